# revision 2
# baseline (speedup 1.0000x reference)
import sys

sys.path.insert(0, "/opt/trn_rl_repo")

from contextlib import ExitStack

import numpy as np

import concourse.bacc as bacc
import concourse.mybir as mybir
from concourse import tile
from concourse.bass_utils import run_bass_kernel_spmd

F32 = mybir.dt.float32
F32R = mybir.dt.float32r
AL = mybir.AluOpType
AF = mybir.ActivationFunctionType

C = 256
H = W = 64
NC = 8  # cores / batch shards


# ---------------------------------------------------------------- host prep
def host_prep(inp):
    """Rearrange all weights into [partition, free] layouts matching SBUF tiles."""
    d = {}
    f = np.float32

    # conditioning nets (dsc1, dsc2)
    for i, pre in ((0, "dsc1"), (1, "dsc2")):
        w1 = np.asarray(inp[f"{pre}_w1"], f)  # [64, 256]
        b1 = np.asarray(inp[f"{pre}_b1"], f)  # [64]
        w2 = np.asarray(inp[f"{pre}_w2"], f)  # [2304, 64]
        b2 = np.asarray(inp[f"{pre}_b2"], f)  # [2304]
        # lhsT for gm matmul: [k_local, chunk, m]; fold the 1/(H*W) mean here
        d[f"w1T{i}"] = np.ascontiguousarray(
            (w1.T / (H * W)).reshape(2, 128, 64).transpose(1, 0, 2)
        ).reshape(128, 128)
        d[f"b1_{i}"] = b1.reshape(64, 1).copy()
        # lhsT for wts matmul: [j, chunk, k, c_local]; fold gelu's 0.5 here
        d[f"w2r{i}"] = np.ascontiguousarray(
            (0.5 * w2).reshape(2, 128, 9, 64).transpose(3, 0, 2, 1)
        ).reshape(64, 2304)
        d[f"b2r{i}"] = np.ascontiguousarray(
            b2.reshape(2, 128, 9).transpose(1, 0, 2)
        ).reshape(128, 18)

    # identity for building runtime diagonal depthwise weight matrices
    d["ident"] = np.ascontiguousarray(np.eye(128, dtype=f))

    # channel_align 1x1: [k_local, kc, mc, m]
    aw = np.asarray(inp["align_w"], f)[:, :, 0, 0]  # [256, 512]
    d["alignw"] = np.ascontiguousarray(
        aw.reshape(2, 128, 4, 128).transpose(3, 2, 0, 1)
    ).reshape(128, 1024)
    d["alignb"] = np.ascontiguousarray(
        np.asarray(inp["align_b"], f).reshape(2, 128).T
    )  # [128, 2]

    # up conv1 3x3 C->4C with pixel-shuffle phase reorder:
    # new channel (p, g) -> orig channel 4g + p   (p = 2r+s)
    uw = np.asarray(inp["up_w1"], f)  # [1024, 256, 3, 3]
    a = uw.reshape(256, 4, 2, 128, 9)  # [g, p, kc, k_local, tap]
    a = a.reshape(2, 128, 4, 2, 128, 9)  # [mcin, m, p, kc, k_local, tap]
    d["up1w"] = np.ascontiguousarray(a.transpose(4, 2, 0, 5, 3, 1)).reshape(
        128, 4 * 2 * 9 * 2 * 128
    )  # [k_local, (p, mcin, tap, kc, m)]
    ub = np.asarray(inp["up_b1"], f)
    d["up1b"] = np.ascontiguousarray(
        ub.reshape(2, 128, 4).transpose(1, 2, 0)
    ).reshape(128, 8)  # [m, (p, mcin)]

    # up conv2 1x1 C->C/2 (per-phase): [k_local, kc, m]
    u2 = np.asarray(inp["up_w2"], f)[:, :, 0, 0]  # [128, 256]
    d["up2w"] = np.ascontiguousarray(
        u2.reshape(128, 2, 128).transpose(2, 1, 0)
    ).reshape(128, 256)
    d["up2b"] = np.asarray(inp["up_b2"], f).reshape(128, 1).copy()

    # ---- polyphase re_enhance ----
    # phase/tap -> (in-phase, sub-shift) mapping
    def split(v):  # v = r + dy - 1
        rp = v % 2
        return rp, (v - rp) // 2

    r1w = np.asarray(inp["re_w1"], f)  # [32, 128, 3, 3]
    keymap = {}
    for p in range(4):
        r, s = p // 2, p % 2
        for dy in range(3):
            for dx in range(3):
                rp, qy = split(r + dy - 1)
                sp, qx = split(s + dx - 1)
                keymap.setdefault((2 * rp + sp, qy, qx), []).append((p, dy, dx))
    keys = sorted(keymap.keys(), key=lambda k: (k[1] != 0 or k[2] != 0, k))
    re1_keys = keys  # list of (p_in, qy, qx); all-(0,0) shifts first
    re1w = np.zeros((128, 16, 128), f)
    for ki, key in enumerate(keys):
        for (p, dy, dx) in keymap[key]:
            re1w[:, ki, p * 32 : (p + 1) * 32] = r1w[:, :, dy, dx].T
    d["re1w"] = re1w.reshape(128, 2048)
    d["re1b"] = np.tile(np.asarray(inp["re_b1"], f), 4).reshape(128, 1)

    r2w = np.asarray(inp["re_w2"], f)  # [128, 32, 3, 3]
    re2_q = []  # per out-phase list of shifts, (0,0) first
    re2w = np.zeros((128, 4, 4, 128), f)
    for p in range(4):
        r, s = p // 2, p % 2
        qys = sorted({split(r + dy - 1)[1] for dy in range(3)}, key=lambda q: q != 0)
        qxs = sorted({split(s + dx - 1)[1] for dx in range(3)}, key=lambda q: q != 0)
        qs = [(qy, qx) for qy in qys for qx in qxs]
        qs.sort(key=lambda q: q != (0, 0))
        re2_q.append(qs)
        for qi, (qy, qx) in enumerate(qs):
            for pp in range(4):
                rp, sp = pp // 2, pp % 2
                dy = 2 * qy + rp - r + 1
                dx = 2 * qx + sp - s + 1
                if 0 <= dy < 3 and 0 <= dx < 3:
                    re2w[pp * 32 : (pp + 1) * 32, p, qi, :] = r2w[:, :, dy, dx].T
    d["re2w"] = re2w.reshape(128, 2048)
    d["re2b"] = np.asarray(inp["re_b2"], f).reshape(128, 1).copy()

    return d, re1_keys, re2_q


RE1_KEYS = None
RE2_Q = None


def _mapping():
    global RE1_KEYS, RE2_Q
    if RE1_KEYS is None:
        zeros = {k: np.zeros(v) for k, v in [
            ("dsc1_w1", (64, 256)), ("dsc1_b1", (64,)), ("dsc1_w2", (2304, 64)),
            ("dsc1_b2", (2304,)), ("dsc2_w1", (64, 256)), ("dsc2_b1", (64,)),
            ("dsc2_w2", (2304, 64)), ("dsc2_b2", (2304,)),
            ("align_w", (256, 512, 1, 1)), ("align_b", (256,)),
            ("up_w1", (1024, 256, 3, 3)), ("up_b1", (1024,)),
            ("up_w2", (128, 256, 1, 1)), ("up_b2", (128,)),
            ("re_w1", (32, 128, 3, 3)), ("re_b1", (32,)),
            ("re_w2", (128, 32, 3, 3)), ("re_b2", (128,)),
        ]}
        _, RE1_KEYS, RE2_Q = host_prep(zeros)
    return RE1_KEYS, RE2_Q


# ---------------------------------------------------------------- bass build
def pimg(ap):
    """View of a column-padded [128, 64*66] image (zero cols at x=0 and x=65)."""
    return ap.rearrange("p (y x) -> p y x", x=66)


def mm_views(src_ap, psum_ap, sy, sx, n):
    """(psum_out, rhs) for 8-row chunk n of a shifted conv tap on a col-padded
    source. Row range restricted by sy; columns handled by the zero pad.
    psum out is a contiguous 2D region."""
    y0 = max(0, -sy)
    y1 = 64 + min(0, -sy)
    r0 = max(8 * n, y0)
    r1 = min(8 * n + 8, y1)
    if r1 <= r0:
        return None, None
    v = pimg(src_ap)
    rhs = v[:, r0 + sy : r1 + sy, 1 + sx : 65 + sx]
    out = psum_ap[:, (r0 - 8 * n) * 64 : (r1 - 8 * n) * 64]
    return out, rhs


def build():
    re1_keys, re2_q = _mapping()
    nc = bacc.Bacc(trn_type="TRN2", target_bir_lowering=False, debug=False)

    x_d = [nc.dram_tensor(n, [256, 4096], F32R, kind="ExternalInput") for n in ("x1", "x2")]
    wd = {}
    for name, shape, dt in [
        ("w1T0", [128, 128], F32), ("w1T1", [128, 128], F32),
        ("b1_0", [64, 1], F32), ("b1_1", [64, 1], F32),
        ("w2r0", [64, 2304], F32), ("w2r1", [64, 2304], F32),
        ("b2r0", [128, 18], F32), ("b2r1", [128, 18], F32),
        ("ident", [128, 128], F32),
        ("alignw", [128, 1024], F32R), ("alignb", [128, 2], F32),
        ("up1w", [128, 18432], F32R), ("up1b", [128, 8], F32),
        ("up2w", [128, 256], F32R), ("up2b", [128, 1], F32),
        ("re1w", [128, 2048], F32R), ("re1b", [128, 1], F32),
        ("re2w", [128, 2048], F32R), ("re2b", [128, 1], F32),
    ]:
        wd[name] = nc.dram_tensor(name, shape, dt, kind="ExternalInput")
    out_d = nc.dram_tensor("out", [128, 16384], F32, kind="ExternalOutput")

    with tile.TileContext(nc) as tc, ExitStack() as ctx:
        wpool = ctx.enter_context(tc.tile_pool(name="w", bufs=1))
        u1pool = ctx.enter_context(tc.tile_pool(name="u1w", bufs=4))
        big = ctx.enter_context(tc.tile_pool(name="big", bufs=8))
        dpool = ctx.enter_context(tc.tile_pool(name="dg", bufs=3))
        stg = ctx.enter_context(tc.tile_pool(name="stg", bufs=2))
        bnd = ctx.enter_context(tc.tile_pool(name="bnd", bufs=2))
        tiny = ctx.enter_context(tc.tile_pool(name="tiny", bufs=5))
        ps = ctx.enter_context(tc.tile_pool(name="ps", bufs=6, space="PSUM"))
        psc = ctx.enter_context(tc.tile_pool(name="psc", bufs=1, space="PSUM"))

        # persistent small weights
        wt = {}
        for name in ("w1T0", "w1T1", "b1_0", "b1_1", "b2r0", "b2r1", "ident",
                     "alignb", "up2w", "up2b",
                     "re1b", "re2b", "up1b"):
            t = wpool.tile(list(wd[name].shape), wd[name].dtype, tag=name)
            nc.sync.dma_start(t[:], wd[name].ap())
            wt[name] = t

        def padded_img_tile(tag="big"):
            """[128, 64*66] F32R image tile with pad cols x=0,65 zeroed."""
            t = big.tile([128, 4224], F32R, tag=tag)
            v = pimg(t[:].bitcast(F32))
            nc.gpsimd.memset(v[:, :, 0:1], 0.0)
            nc.gpsimd.memset(v[:, :, 65:66], 0.0)
            return t

        # x in (column-padded layout)
        xin = []  # [input][chunk] -> padded tile
        for i in range(2):
            pair = []
            for c in range(2):
                t = padded_img_tile()
                nc.sync.dma_start(pimg(t[:])[:, :, 1:65],
                                  x_d[i].ap()[c * 128 : (c + 1) * 128, :])
                pair.append(t)
            xin.append(pair)

        # streamed conditioning w2r (one dsc at a time)
        w2rt = {}

        def load_w2r(d):
            if d not in w2rt:
                t = u1pool.tile([64, 2304], F32, tag="u1w")
                nc.sync.dma_start(t[:], wd[f"w2r{d}"].ap())
                w2rt[d] = t
            return w2rt[d]

        # ---------------- dsc stage ----------------
        d_init = [0]  # number of dpool buffers memset so far

        def conditioning(d, gms):
            """gms: two [128,1] per-chunk channel-sum tiles -> per-chunk
            diagonal-tap matrices D [128, 9*128] (f32r) for the PE depthwise."""
            pg = psc.tile([64, 1], F32, tag="psc_g")
            for c in range(2):
                nc.tensor.matmul(pg[:], wt[f"w1T{d}"][:, c * 64 : (c + 1) * 64],
                                 gms[c][:], start=(c == 0), stop=(c == 1))
            u = tiny.tile([64, 1], F32, tag="u")
            nc.scalar.activation(u[:], pg[:], AF.Identity, bias=wt[f"b1_{d}"][:])
            sq = tiny.tile([64, 1], F32, tag="sq")
            nc.scalar.activation(sq[:], u[:], AF.Square)
            v3 = tiny.tile([64, 1], F32, tag="v3")
            nc.vector.scalar_tensor_tensor(v3[:], sq[:], 0.044715, u[:], AL.mult, AL.mult)
            w_ = tiny.tile([64, 1], F32, tag="w_")
            nc.vector.tensor_tensor(w_[:], v3[:], u[:], AL.add)
            th = tiny.tile([64, 1], F32, tag="th")
            nc.scalar.activation(th[:], w_[:], AF.Tanh, scale=0.7978845608028654)
            hv = tiny.tile([64, 1], F32, tag="hv")
            nc.vector.scalar_tensor_tensor(hv[:], th[:], 1.0, u[:], AL.add, AL.mult)

            w2r = load_w2r(d)
            D_l = []
            for c in range(2):
                pw = psc.tile([128, 9], F32, tag="psc_w")
                for k in range(9):
                    nc.tensor.matmul(pw[:, k : k + 1],
                                     w2r[:, (c * 9 + k) * 128 : (c * 9 + k + 1) * 128],
                                     hv[:], start=True, stop=True)
                raw = tiny.tile([128, 9], F32, tag="raw")
                for k in range(9):
                    nc.scalar.activation(raw[:, k : k + 1], pw[:, k : k + 1], AF.Identity,
                                         bias=wt[f"b2r{d}"][:, c * 9 + k : c * 9 + k + 1])
                mx = tiny.tile([128, 1], F32, tag="mx")
                nc.vector.tensor_reduce(mx[:], raw[:], axis=mybir.AxisListType.X, op=AL.max)
                ngm = tiny.tile([128, 1], F32, tag="ngm")
                nc.vector.tensor_scalar_mul(ngm[:], mx[:], -1.0)
                ex = tiny.tile([128, 9], F32, tag="ex")
                ssum = tiny.tile([128, 1], F32, tag="ssum")
                nc.scalar.activation(ex[:], raw[:], AF.Exp, bias=ngm[:], accum_out=ssum[:])
                rec = tiny.tile([128, 1], F32, tag="rec")
                nc.vector.reciprocal(rec[:], ssum[:])
                wts = tiny.tile([128, 9], F32, tag="wts")
                nc.vector.tensor_scalar_mul(wts[:], ex[:], rec[:])
                w4p = tiny.tile([128, 1], F32, tag="w4p")
                nc.vector.tensor_scalar_add(w4p[:], wts[:, 4:5], 1.0)
                # diagonal tap matrices (center tap has +1 residual folded in)
                D = dpool.tile([128, 1152], F32R, tag="D")
                if d_init[0] < 3:
                    nc.gpsimd.memset(D[:].bitcast(F32), 0.0)
                    d_init[0] += 1
                Dv = D[:].bitcast(F32)
                for k in range(9):
                    ptr = w4p[:] if k == 4 else wts[:, k : k + 1]
                    nc.vector.tensor_scalar_mul(
                        Dv[:, k * 128 : (k + 1) * 128], wt["ident"][:], ptr)
                D_l.append(D)
            return D_l

        TAPS = (4, 0, 1, 2, 3, 5, 6, 7, 8)  # center first: full psum coverage

        def dw_pe(src, dst, D, relu, gacc):
            """dst = depthwise(src) (+x via center tap) on PE; evict via Act.
            src: padded f32r; dst: padded f32r (relu) or flat [128,4096] (not).
            gacc: [128,8] per-chunk accum target or None."""
            for n in range(8):
                p = ps.tile([128, 512], F32, tag="ps")
                first = True
                for k in TAPS:
                    sy, sx = k // 3 - 1, k % 3 - 1
                    o, rhs = mm_views(src[:], p[:], sy, sx, n)
                    if o is None:
                        continue
                    nc.tensor.matmul(o, D[:, k * 128 : (k + 1) * 128], rhs,
                                     start=first, stop=(k == TAPS[-1]),
                                     skip_group_check=True)
                    first = False
                pv = p[:].rearrange("p (y x) -> p y x", y=8)
                if relu:
                    nc.scalar.activation(pimg(dst[:])[:, n * 8 : (n + 1) * 8, 1:65],
                                         pv, AF.Relu, accum_out=gacc[:, n : n + 1])
                else:
                    nc.scalar.activation(dst[:, n * 512 : (n + 1) * 512].bitcast(F32),
                                         pv, AF.Identity)

        # per-chunk channel sums of the inputs
        gms1 = [[None, None], [None, None]]
        for i in range(2):
            for c in range(2):
                g = tiny.tile([128, 1], F32, tag="gms")
                nc.vector.tensor_reduce(g[:], xin[i][c][:].bitcast(F32),
                                        axis=mybir.AxisListType.X, op=AL.add)
                gms1[i][c] = g

        mid = [[None, None], [None, None]]
        gacc = [[None, None], [None, None]]
        gms2 = [[None, None], [None, None]]
        y = [[None, None], [None, None]]

        def alloc_mid(i):
            for c in range(2):
                mid[i][c] = padded_img_tile()
                gacc[i][c] = tiny.tile([128, 8], F32, tag="gacc")

        def reduce_gms2(i):
            for c in range(2):
                g2 = tiny.tile([128, 1], F32, tag="gms")
                nc.vector.tensor_reduce(g2[:], gacc[i][c][:],
                                        axis=mybir.AxisListType.X, op=AL.add)
                gms2[i][c] = g2

        # interleave conditioning (PE-tiny + DVE/Act chain) between the big
        # PE depthwise apps so PE never waits on a conditioning chain.
        D1x1 = conditioning(0, gms1[0])
        alloc_mid(0)
        dw_pe(xin[0][0], mid[0][0], D1x1[0], True, gacc[0][0])
        D1x2 = conditioning(0, gms1[1])
        alloc_mid(1)
        dw_pe(xin[0][1], mid[0][1], D1x1[1], True, gacc[0][1])
        dw_pe(xin[1][0], mid[1][0], D1x2[0], True, gacc[1][0])
        reduce_gms2(0)
        D2x1 = conditioning(1, gms2[0])
        dw_pe(xin[1][1], mid[1][1], D1x2[1], True, gacc[1][1])
        reduce_gms2(1)
        for c in range(2):
            y[0][c] = big.tile([128, 4096], F32R, tag="big")
        dw_pe(mid[0][0], y[0][0], D2x1[0], False, None)
        D2x2 = conditioning(1, gms2[1])
        dw_pe(mid[0][1], y[0][1], D2x1[1], False, None)
        for c in range(2):
            y[1][c] = big.tile([128, 4096], F32R, tag="big")
        dw_pe(mid[1][0], y[1][0], D2x2[0], False, None)
        dw_pe(mid[1][1], y[1][1], D2x2[1], False, None)

        # ---------------- align 1x1 (2C -> C) ----------------
        awt = u1pool.tile([128, 1024], F32R, tag="u1w")
        nc.sync.dma_start(awt[:], wd["alignw"].ap())
        wt["alignw"] = awt
        fus = []
        for mc in range(2):
            fus.append(padded_img_tile())
        for mc in range(2):
            for n in range(8):
                p = ps.tile([128, 512], F32, tag="ps")
                for kc in range(4):
                    rhs = y[kc // 2][kc % 2][:, n * 512 : (n + 1) * 512]
                    nc.tensor.matmul(
                        p[:], wt["alignw"][:, (kc * 2 + mc) * 128 : (kc * 2 + mc + 1) * 128],
                        rhs, start=(kc == 0), stop=(kc == 3))
                hview = pimg(fus[mc][:])[:, n * 8 : (n + 1) * 8, 1:65]
                nc.scalar.activation(hview, p[:].rearrange("p (y x) -> p y x", y=8),
                                     AF.Identity, bias=wt["alignb"][:, mc : mc + 1])

        # ---------------- up1 (3x3 C->4C, phase-ordered) + up2 (1x1) ----------------
        taps = [(1, 1)] + [(dy, dx) for dy in range(3) for dx in range(3) if (dy, dx) != (1, 1)]
        up2p = []
        for p4 in range(4):
            up2p.append(padded_img_tile())
        for p4 in range(4):
            u1t = []
            for mcin in range(2):
                th = u1pool.tile([128, 2304], F32R, tag="u1w")
                off = (p4 * 2 + mcin) * 2304
                nc.sync.dma_start(th[:], wd["up1w"].ap()[:, off : off + 2304])
                u1t.append(th)
            for n in range(8):
                stage = []
                for mcin in range(2):
                    p = ps.tile([128, 512], F32, tag="ps")
                    first = True
                    for (dy, dx) in taps:
                        sy, sx = dy - 1, dx - 1
                        for kc in range(2):
                            o, rhs = mm_views(fus[kc][:], p[:], sy, sx, n)
                            if o is None:
                                continue
                            lhsT = u1t[mcin][:, ((dy * 3 + dx) * 2 + kc) * 128 :
                                             ((dy * 3 + dx) * 2 + kc + 1) * 128]
                            nc.tensor.matmul(o, lhsT, rhs, start=first,
                                             stop=((dy, dx) == taps[-1] and kc == 1),
                                             skip_group_check=True)
                            first = False
                    st = stg.tile([128, 512], F32R, tag="stg")
                    nc.scalar.activation(st[:].bitcast(F32), p[:], AF.Identity,
                                         bias=wt["up1b"][:, p4 * 2 + mcin : p4 * 2 + mcin + 1])
                    stage.append(st)
                p2 = ps.tile([128, 512], F32, tag="ps")
                for kc in range(2):
                    nc.tensor.matmul(p2[:], wt["up2w"][:, kc * 128 : (kc + 1) * 128],
                                     stage[kc][:], start=(kc == 0), stop=(kc == 1))
                nc.scalar.activation(pimg(up2p[p4][:])[:, n * 8 : (n + 1) * 8, 1:65],
                                     p2[:].rearrange("p (y x) -> p y x", y=8),
                                     AF.Identity, bias=wt["up2b"][:])

        # ---------------- re1 (polyphase 3x3, M-packed) ----------------
        re1t = padded_img_tile()
        for name in ("re1w", "re2w"):
            t = u1pool.tile([128, 2048], F32R, tag="u1w")
            nc.sync.dma_start(t[:], wd[name].ap())
            wt[name] = t
        for n in range(8):
            p = ps.tile([128, 512], F32, tag="ps")
            for ki, (pin, qy, qx) in enumerate(re1_keys):
                o, rhs = mm_views(up2p[pin][:], p[:], qy, qx, n)
                if o is None:
                    continue
                nc.tensor.matmul(o, wt["re1w"][:, ki * 128 : (ki + 1) * 128], rhs,
                                 start=(ki == 0), stop=(ki == len(re1_keys) - 1),
                                 skip_group_check=True)
            nc.scalar.activation(pimg(re1t[:])[:, n * 8 : (n + 1) * 8, 1:65],
                                 p[:].rearrange("p (y x) -> p y x", y=8),
                                 AF.Relu, bias=wt["re1b"][:])

        # ---------------- re2 (polyphase 3x3) + residual + interleave + out ----------------
        for n in range(8):
            pss = []
            for p4 in range(4):
                p = ps.tile([128, 512], F32, tag="ps")
                for qi, (qy, qx) in enumerate(re2_q[p4]):
                    o, rhs = mm_views(re1t[:], p[:], qy, qx, n)
                    if o is None:
                        continue
                    nc.tensor.matmul(o, wt["re2w"][:, (p4 * 4 + qi) * 128 :
                                                   (p4 * 4 + qi + 1) * 128], rhs,
                                     start=(qi == 0), stop=(qi == len(re2_q[p4]) - 1),
                                     skip_group_check=True)
                pss.append(p)
            for hb in range(2):  # half-bands of 8 output rows (4 phase rows)
                band = bnd.tile([128, 1024], F32, tag="bnd")
                bv = band[:].rearrange("p (y r x s) -> p y r x s", y=4, r=2, s=2)
                for p4 in range(4):
                    r, s = p4 // 2, p4 % 2
                    y0 = n * 8 + hb * 4
                    up_v = pimg(up2p[p4][:].bitcast(F32))[:, y0 : y0 + 4, 1:65]
                    nc.vector.scalar_tensor_tensor(
                        bv[:, :, r, :, s],
                        pss[p4][:, hb * 256 : (hb + 1) * 256].rearrange(
                            "p (y x) -> p y x", y=4),
                        wt["re2b"][:], up_v, AL.add, AL.add)
                nc.sync.dma_start(
                    out_d.ap()[:, (2 * n + hb) * 1024 : (2 * n + hb + 1) * 1024],
                    band[:])

    nc.compile()
    return nc


_NC = None


def _get_nc():
    global _NC
    if _NC is None:
        _NC = build()
    return _NC


def make_in_maps(inputs):
    w, _, _ = host_prep(inputs)
    x1 = np.ascontiguousarray(np.asarray(inputs["x1"], np.float32).reshape(NC, 256, 4096))
    x2 = np.ascontiguousarray(np.asarray(inputs["x2"], np.float32).reshape(NC, 256, 4096))
    in_maps = []
    for i in range(NC):
        m = {"x1": x1[i], "x2": x2[i]}
        m.update(w)
        in_maps.append(m)
    return in_maps


def kernel(**inputs):
    nc = _get_nc()
    in_maps = make_in_maps(inputs)
    res = run_bass_kernel_spmd(nc, in_maps, core_ids=list(range(NC)))
    out = np.stack([res.results[i]["out"].reshape(128, 128, 128) for i in range(NC)])
    return out.astype(np.float32)


# revision 14
# speedup vs baseline: 2.5616x; 2.5616x over previous
import sys

sys.path.insert(0, "/opt/trn_rl_repo")

from contextlib import ExitStack

import numpy as np

import concourse.bacc as bacc
import concourse.mybir as mybir
from concourse import tile
from concourse.bass_utils import run_bass_kernel_spmd

F32 = mybir.dt.float32
F32R = mybir.dt.float32r
AL = mybir.AluOpType
AF = mybir.ActivationFunctionType

C = 256
H = W = 64
NC = 8  # cores / batch shards


# ---------------------------------------------------------------- host prep
def host_prep(inp):
    """Rearrange all weights into [partition, free] layouts matching SBUF tiles."""
    d = {}
    f = np.float32

    # conditioning nets (dsc1, dsc2)
    for i, pre in ((0, "dsc1"), (1, "dsc2")):
        w1 = np.asarray(inp[f"{pre}_w1"], f)  # [64, 256]
        b1 = np.asarray(inp[f"{pre}_b1"], f)  # [64]
        w2 = np.asarray(inp[f"{pre}_w2"], f)  # [2304, 64]
        b2 = np.asarray(inp[f"{pre}_b2"], f)  # [2304]
        # lhsT for gm matmul: [k_local, chunk, m]; fold the 1/(H*W) mean here
        d[f"w1T{i}"] = np.ascontiguousarray(
            (w1.T / (H * W)).reshape(2, 128, 64).transpose(1, 0, 2)
        ).reshape(128, 128)
        d[f"b1_{i}"] = b1.reshape(64, 1).copy()
        # lhsT for wts matmul: [j, chunk, k, c_local]; fold gelu's 0.5 here
        d[f"w2r{i}"] = np.ascontiguousarray(
            (0.5 * w2).reshape(2, 128, 9, 64).transpose(3, 0, 2, 1)
        ).reshape(64, 2304)
        d[f"b2r{i}"] = np.ascontiguousarray(
            b2.reshape(2, 128, 9).transpose(1, 0, 2)
        ).reshape(128, 18)

    # identity for building runtime diagonal depthwise weight matrices
    d["ident"] = np.ascontiguousarray(np.eye(128, dtype=f))

    # channel_align 1x1: [k_local, kc, mc, m]
    aw = np.asarray(inp["align_w"], f)[:, :, 0, 0]  # [256, 512]
    d["alignw"] = np.ascontiguousarray(
        aw.reshape(2, 128, 4, 128).transpose(3, 2, 0, 1)
    ).reshape(128, 1024)
    d["alignb"] = np.ascontiguousarray(
        np.asarray(inp["align_b"], f).reshape(2, 128).T
    )  # [128, 2]

    # up conv1 3x3 C->4C with pixel-shuffle phase reorder:
    # new channel (p, g) -> orig channel 4g + p   (p = 2r+s)
    uw = np.asarray(inp["up_w1"], f)  # [1024, 256, 3, 3]
    a = uw.reshape(256, 4, 2, 128, 9)  # [g, p, kc, k_local, tap]
    a = a.reshape(2, 128, 4, 2, 128, 9)  # [mcin, m, p, kc, k_local, tap]
    d["up1w"] = np.ascontiguousarray(a.transpose(4, 2, 0, 5, 3, 1)).reshape(
        128, 4 * 2 * 9 * 2 * 128
    )  # [k_local, (p, mcin, tap, kc, m)]
    ub = np.asarray(inp["up_b1"], f)
    d["up1b"] = np.ascontiguousarray(
        ub.reshape(2, 128, 4).transpose(1, 2, 0)
    ).reshape(128, 8)  # [m, (p, mcin)]

    # up conv2 1x1 C->C/2 (per-phase): [k_local, kc, m]
    u2 = np.asarray(inp["up_w2"], f)[:, :, 0, 0]  # [128, 256]
    d["up2w"] = np.ascontiguousarray(
        u2.reshape(128, 2, 128).transpose(2, 1, 0)
    ).reshape(128, 256)
    d["up2b"] = np.asarray(inp["up_b2"], f).reshape(128, 1).copy()

    # ---- polyphase re_enhance ----
    # phase/tap -> (in-phase, sub-shift) mapping
    def split(v):  # v = r + dy - 1
        rp = v % 2
        return rp, (v - rp) // 2

    r1w = np.asarray(inp["re_w1"], f)  # [32, 128, 3, 3]
    keymap = {}
    for p in range(4):
        r, s = p // 2, p % 2
        for dy in range(3):
            for dx in range(3):
                rp, qy = split(r + dy - 1)
                sp, qx = split(s + dx - 1)
                keymap.setdefault((2 * rp + sp, qy, qx), []).append((p, dy, dx))
    keys = sorted(keymap.keys(), key=lambda k: (k[1] != 0 or k[2] != 0, k))
    re1_keys = keys  # list of (p_in, qy, qx); all-(0,0) shifts first
    re1w = np.zeros((128, 16, 128), f)
    for ki, key in enumerate(keys):
        for (p, dy, dx) in keymap[key]:
            re1w[:, ki, p * 32 : (p + 1) * 32] = r1w[:, :, dy, dx].T
    d["re1w"] = re1w.reshape(128, 2048)
    d["re1b"] = np.tile(np.asarray(inp["re_b1"], f), 4).reshape(128, 1)

    r2w = np.asarray(inp["re_w2"], f)  # [128, 32, 3, 3]
    re2_q = []  # per out-phase list of shifts, (0,0) first
    re2w = np.zeros((128, 4, 4, 128), f)
    for p in range(4):
        r, s = p // 2, p % 2
        qys = sorted({split(r + dy - 1)[1] for dy in range(3)}, key=lambda q: q != 0)
        qxs = sorted({split(s + dx - 1)[1] for dx in range(3)}, key=lambda q: q != 0)
        qs = [(qy, qx) for qy in qys for qx in qxs]
        qs.sort(key=lambda q: q != (0, 0))
        re2_q.append(qs)
        for qi, (qy, qx) in enumerate(qs):
            for pp in range(4):
                rp, sp = pp // 2, pp % 2
                dy = 2 * qy + rp - r + 1
                dx = 2 * qx + sp - s + 1
                if 0 <= dy < 3 and 0 <= dx < 3:
                    re2w[pp * 32 : (pp + 1) * 32, p, qi, :] = r2w[:, :, dy, dx].T
    d["re2w"] = re2w.reshape(128, 2048)
    d["re2b"] = np.asarray(inp["re_b2"], f).reshape(128, 1).copy()

    return d, re1_keys, re2_q


RE1_KEYS = None
RE2_Q = None


def _mapping():
    global RE1_KEYS, RE2_Q
    if RE1_KEYS is None:
        zeros = {k: np.zeros(v) for k, v in [
            ("dsc1_w1", (64, 256)), ("dsc1_b1", (64,)), ("dsc1_w2", (2304, 64)),
            ("dsc1_b2", (2304,)), ("dsc2_w1", (64, 256)), ("dsc2_b1", (64,)),
            ("dsc2_w2", (2304, 64)), ("dsc2_b2", (2304,)),
            ("align_w", (256, 512, 1, 1)), ("align_b", (256,)),
            ("up_w1", (1024, 256, 3, 3)), ("up_b1", (1024,)),
            ("up_w2", (128, 256, 1, 1)), ("up_b2", (128,)),
            ("re_w1", (32, 128, 3, 3)), ("re_b1", (32,)),
            ("re_w2", (128, 32, 3, 3)), ("re_b2", (128,)),
        ]}
        _, RE1_KEYS, RE2_Q = host_prep(zeros)
    return RE1_KEYS, RE2_Q


# ---------------------------------------------------------------- bass build
def pimg(ap):
    """View of a column-padded [128, 64*66] image (zero cols at x=0 and x=65)."""
    return ap.rearrange("p (y x) -> p y x", x=66)


def mm_views(src_ap, psum_ap, sy, sx, n):
    """(psum_out, rhs) for 8-row chunk n of a shifted conv tap on a col-padded
    source. Row range restricted by sy; columns handled by the zero pad.
    psum out is a contiguous 2D region."""
    y0 = max(0, -sy)
    y1 = 64 + min(0, -sy)
    r0 = max(8 * n, y0)
    r1 = min(8 * n + 8, y1)
    if r1 <= r0:
        return None, None
    v = pimg(src_ap)
    rhs = v[:, r0 + sy : r1 + sy, 1 + sx : 65 + sx]
    out = psum_ap[:, (r0 - 8 * n) * 64 : (r1 - 8 * n) * 64]
    return out, rhs


def build():
    re1_keys, re2_q = _mapping()
    nc = bacc.Bacc(trn_type="TRN2", target_bir_lowering=False, debug=False)

    x_d = [nc.dram_tensor(n, [256, 4096], F32R, kind="ExternalInput") for n in ("x1", "x2")]
    wd = {}
    for name, shape, dt in [
        ("w1T0", [128, 128], F32), ("w1T1", [128, 128], F32),
        ("b1_0", [64, 1], F32), ("b1_1", [64, 1], F32),
        ("w2r0", [64, 2304], F32), ("w2r1", [64, 2304], F32),
        ("b2r0", [128, 18], F32), ("b2r1", [128, 18], F32),
        ("ident", [128, 128], F32R),
        ("alignw", [128, 1024], F32R), ("alignb", [128, 2], F32),
        ("up1w", [128, 18432], F32R), ("up1b", [128, 8], F32),
        ("up2w", [128, 256], F32R), ("up2b", [128, 1], F32),
        ("re1w", [128, 2048], F32R), ("re1b", [128, 1], F32),
        ("re2w", [128, 2048], F32R), ("re2b", [128, 1], F32),
    ]:
        wd[name] = nc.dram_tensor(name, shape, dt, kind="ExternalInput")
    out_d = nc.dram_tensor("out", [128, 16384], F32, kind="ExternalOutput")

    with tile.TileContext(nc) as tc, ExitStack() as ctx:
        wpool = ctx.enter_context(tc.tile_pool(name="w", bufs=1))
        u1pool = ctx.enter_context(tc.tile_pool(name="u1w", bufs=4))
        big = ctx.enter_context(tc.tile_pool(name="big", bufs=8))
        dpool = ctx.enter_context(tc.tile_pool(name="dg", bufs=3))
        stg = ctx.enter_context(tc.tile_pool(name="stg", bufs=2))
        bnd = ctx.enter_context(tc.tile_pool(name="bnd", bufs=2))
        tiny = ctx.enter_context(tc.tile_pool(name="tiny", bufs=5))
        ps = ctx.enter_context(tc.tile_pool(name="ps", bufs=6, space="PSUM"))
        psc = ctx.enter_context(tc.tile_pool(name="psc", bufs=1, space="PSUM"))

        # persistent small weights
        wt = {}
        for name in ("w1T0", "w1T1", "b1_0", "b1_1", "b2r0", "b2r1", "ident",
                     "alignb", "up2w", "up2b",
                     "re1b", "re2b", "up1b"):
            t = wpool.tile(list(wd[name].shape), wd[name].dtype, tag=name)
            nc.sync.dma_start(t[:], wd[name].ap())
            wt[name] = t

        def padded_img_tile(tag="big"):
            """[128, 64*66] F32R image tile with pad cols x=0,65 zeroed."""
            t = big.tile([128, 4224], F32R, tag=tag)
            v = pimg(t[:].bitcast(F32))
            nc.gpsimd.memset(v[:, :, 0:1], 0.0)
            nc.gpsimd.memset(v[:, :, 65:66], 0.0)
            return t

        # conditioning w2r weights first: they gate the first PE work
        w2rt = {}

        def load_w2r(d):
            if d not in w2rt:
                t = u1pool.tile([64, 2304], F32, tag="u1w")
                nc.sync.dma_start(t[:], wd[f"w2r{d}"].ap())
                w2rt[d] = t
            return w2rt[d]

        load_w2r(0)
        load_w2r(1)

        # x in (column-padded layout); x1 chunks first (cond1(x1) gates PE)
        xin = [[None, None], [None, None]]
        for i in range(2):
            for c in range(2):
                t = padded_img_tile()
                nc.sync.dma_start(pimg(t[:])[:, :, 1:65],
                                  x_d[i].ap()[c * 128 : (c + 1) * 128, :])
                xin[i][c] = t

        # ---------------- dsc stage ----------------
        d_init = [0]  # number of dpool buffers memset so far

        def conditioning(d, gms):
            """gms: two [128,1] per-chunk channel-sum tiles -> per-chunk
            diagonal-tap matrices D [128, 9*128] (f32r) for the PE depthwise."""
            pg = psc.tile([64, 1], F32, tag="psc_g")
            for c in range(2):
                nc.tensor.matmul(pg[:], wt[f"w1T{d}"][:, c * 64 : (c + 1) * 64],
                                 gms[c][:], start=(c == 0), stop=(c == 1))
            u = tiny.tile([64, 1], F32, tag="u")
            nc.scalar.activation(u[:], pg[:], AF.Identity, bias=wt[f"b1_{d}"][:])
            sq = tiny.tile([64, 1], F32, tag="sq")
            nc.scalar.activation(sq[:], u[:], AF.Square)
            v3 = tiny.tile([64, 1], F32, tag="v3")
            nc.vector.scalar_tensor_tensor(v3[:], sq[:], 0.044715, u[:], AL.mult, AL.mult)
            w_ = tiny.tile([64, 1], F32, tag="w_")
            nc.vector.tensor_tensor(w_[:], v3[:], u[:], AL.add)
            th = tiny.tile([64, 1], F32, tag="th")
            nc.scalar.activation(th[:], w_[:], AF.Tanh, scale=0.7978845608028654)
            hv = tiny.tile([64, 1], F32, tag="hv")
            nc.vector.scalar_tensor_tensor(hv[:], th[:], 1.0, u[:], AL.add, AL.mult)

            w2r = load_w2r(d)
            D_l = []
            for c in range(2):
                pw = psc.tile([128, 9], F32, tag="psc_w")
                for k in range(9):
                    nc.tensor.matmul(pw[:, k : k + 1],
                                     w2r[:, (c * 9 + k) * 128 : (c * 9 + k + 1) * 128],
                                     hv[:], start=True, stop=True)
                raw = tiny.tile([128, 9], F32, tag="raw")
                nc.vector.tensor_tensor(raw[:], pw[:],
                                        wt[f"b2r{d}"][:, c * 9 : (c + 1) * 9], AL.add)
                mx = tiny.tile([128, 1], F32, tag="mx")
                nc.vector.tensor_reduce(mx[:], raw[:], axis=mybir.AxisListType.X, op=AL.max)
                ngm = tiny.tile([128, 1], F32, tag="ngm")
                nc.vector.tensor_scalar_mul(ngm[:], mx[:], -1.0)
                ex = tiny.tile([128, 9], F32, tag="ex")
                ssum = tiny.tile([128, 1], F32, tag="ssum")
                nc.scalar.activation(ex[:], raw[:], AF.Exp, bias=ngm[:], accum_out=ssum[:])
                rec = tiny.tile([128, 1], F32, tag="rec")
                nc.vector.reciprocal(rec[:], ssum[:])
                wts = tiny.tile([128, 9], F32, tag="wts")
                nc.vector.tensor_scalar_mul(wts[:], ex[:], rec[:])
                w4p = tiny.tile([128, 1], F32, tag="w4p")
                nc.vector.tensor_scalar_add(w4p[:], wts[:, 4:5], 1.0)
                # diagonal tap matrices (center tap has +1 residual folded in)
                D = dpool.tile([128, 1152], F32R, tag="D")
                if d_init[0] < 3:
                    nc.gpsimd.memset(D[:].bitcast(F32), 0.0)
                    d_init[0] += 1
                deng = nc.vector if c == 0 else nc.gpsimd
                for k in range(9):
                    ptr = w4p[:] if k == 4 else wts[:, k : k + 1]
                    deng.tensor_scalar_mul(
                        D[:, k * 128 : (k + 1) * 128], wt["ident"][:], ptr)
                D_l.append(D)
            return D_l

        TAPS = (4, 0, 1, 2, 3, 5, 6, 7, 8)  # center first: full psum coverage

        def dw_pe(src, dst, D, relu, gacc):
            """dst = depthwise(src) (+x via center tap) on PE; evict via Act.
            src: padded f32r; dst: padded f32r (relu) or flat [128,4096] (not).
            gacc: [128,8] per-chunk accum target or None."""
            for n in range(8):
                p = ps.tile([128, 512], F32, tag="ps")
                first = True
                for k in TAPS:
                    sy, sx = k // 3 - 1, k % 3 - 1
                    o, rhs = mm_views(src[:], p[:], sy, sx, n)
                    if o is None:
                        continue
                    nc.tensor.matmul(o, D[:, k * 128 : (k + 1) * 128], rhs,
                                     start=first, stop=(k == TAPS[-1]),
                                     skip_group_check=True)
                    first = False
                pv = p[:].rearrange("p (y x) -> p y x", y=8)
                if relu:
                    nc.scalar.activation(pimg(dst[:])[:, n * 8 : (n + 1) * 8, 1:65],
                                         pv, AF.Relu, accum_out=gacc[:, n : n + 1])
                else:
                    nc.scalar.activation(dst[:, n * 512 : (n + 1) * 512],
                                         pv, AF.Identity)

        # per-chunk channel sums of the inputs; x2's are emitted later so they
        # don't delay cond1(x1)'s D builds on the in-order DVE
        gms1 = [[None, None], [None, None]]

        def reduce_gms1(i):
            for c in range(2):
                g = tiny.tile([128, 1], F32, tag="gms")
                nc.vector.tensor_reduce(g[:], xin[i][c][:].bitcast(F32),
                                        axis=mybir.AxisListType.X, op=AL.add)
                gms1[i][c] = g

        reduce_gms1(0)

        mid = [[None, None], [None, None]]
        gacc = [[None, None], [None, None]]
        gms2 = [[None, None], [None, None]]
        y = [[None, None], [None, None]]

        def alloc_mid(i):
            for c in range(2):
                mid[i][c] = padded_img_tile()
                gacc[i][c] = tiny.tile([128, 8], F32, tag="gacc", name="gacc")

        def reduce_gms2(i):
            for c in range(2):
                g2 = tiny.tile([128, 1], F32, tag="gms")
                nc.vector.tensor_reduce(g2[:], gacc[i][c][:],
                                        axis=mybir.AxisListType.X, op=AL.add)
                gms2[i][c] = g2

        # interleave conditioning (PE-tiny + DVE/Act chain) between the big
        # PE depthwise apps so PE never waits on a conditioning chain.
        D1x1 = conditioning(0, gms1[0])
        alloc_mid(0)
        reduce_gms1(1)
        dw_pe(xin[0][0], mid[0][0], D1x1[0], True, gacc[0][0])
        D1x2 = conditioning(0, gms1[1])
        alloc_mid(1)
        dw_pe(xin[0][1], mid[0][1], D1x1[1], True, gacc[0][1])
        dw_pe(xin[1][0], mid[1][0], D1x2[0], True, gacc[1][0])
        reduce_gms2(0)
        D2x1 = conditioning(1, gms2[0])
        dw_pe(xin[1][1], mid[1][1], D1x2[1], True, gacc[1][1])
        reduce_gms2(1)
        for c in range(2):
            y[0][c] = big.tile([128, 4096], F32R, tag="big", name="yt")
        dw_pe(mid[0][0], y[0][0], D2x1[0], False, None)
        D2x2 = conditioning(1, gms2[1])
        dw_pe(mid[0][1], y[0][1], D2x1[1], False, None)
        for c in range(2):
            y[1][c] = big.tile([128, 4096], F32R, tag="big", name="yt")
        dw_pe(mid[1][0], y[1][0], D2x2[0], False, None)
        dw_pe(mid[1][1], y[1][1], D2x2[1], False, None)

        # ---------------- align 1x1 (2C -> C) ----------------
        awt = u1pool.tile([128, 1024], F32R, tag="u1w")
        nc.sync.dma_start(awt[:], wd["alignw"].ap())
        wt["alignw"] = awt
        fus = []
        for mc in range(2):
            fus.append(padded_img_tile())
        for mc in range(2):
            for n in range(8):
                p = ps.tile([128, 512], F32, tag="ps")
                for kc in range(4):
                    rhs = y[kc // 2][kc % 2][:, n * 512 : (n + 1) * 512]
                    nc.tensor.matmul(
                        p[:], wt["alignw"][:, (kc * 2 + mc) * 128 : (kc * 2 + mc + 1) * 128],
                        rhs, start=(kc == 0), stop=(kc == 3))
                hview = pimg(fus[mc][:])[:, n * 8 : (n + 1) * 8, 1:65]
                nc.scalar.activation(hview, p[:].rearrange("p (y x) -> p y x", y=8),
                                     AF.Identity, bias=wt["alignb"][:, mc : mc + 1])

        # ---------------- up1 (3x3 C->4C, phase-ordered) + up2 (1x1) ----------------
        taps = [(1, 1)] + [(dy, dx) for dy in range(3) for dx in range(3) if (dy, dx) != (1, 1)]
        up2p = []
        for p4 in range(4):
            up2p.append(padded_img_tile())
        for p4 in range(4):
            u1t = []
            for mcin in range(2):
                th = u1pool.tile([128, 2304], F32R, tag="u1w")
                off = (p4 * 2 + mcin) * 2304
                nc.sync.dma_start(th[:], wd["up1w"].ap()[:, off : off + 2304])
                u1t.append(th)
            for n in range(8):
                stage = []
                for mcin in range(2):
                    p = ps.tile([128, 512], F32, tag="ps")
                    first = True
                    for (dy, dx) in taps:
                        sy, sx = dy - 1, dx - 1
                        for kc in range(2):
                            o, rhs = mm_views(fus[kc][:], p[:], sy, sx, n)
                            if o is None:
                                continue
                            lhsT = u1t[mcin][:, ((dy * 3 + dx) * 2 + kc) * 128 :
                                             ((dy * 3 + dx) * 2 + kc + 1) * 128]
                            nc.tensor.matmul(o, lhsT, rhs, start=first,
                                             stop=((dy, dx) == taps[-1] and kc == 1),
                                             skip_group_check=True)
                            first = False
                    st = stg.tile([128, 512], F32R, tag="stg")
                    nc.scalar.activation(st[:], p[:], AF.Identity,
                                         bias=wt["up1b"][:, p4 * 2 + mcin : p4 * 2 + mcin + 1])
                    stage.append(st)
                p2 = ps.tile([128, 512], F32, tag="ps")
                for kc in range(2):
                    nc.tensor.matmul(p2[:], wt["up2w"][:, kc * 128 : (kc + 1) * 128],
                                     stage[kc][:], start=(kc == 0), stop=(kc == 1))
                nc.scalar.activation(pimg(up2p[p4][:])[:, n * 8 : (n + 1) * 8, 1:65],
                                     p2[:].rearrange("p (y x) -> p y x", y=8),
                                     AF.Identity, bias=wt["up2b"][:])

        # ---------------- re1 (polyphase 3x3, M-packed) ----------------
        re1t = padded_img_tile()
        for name in ("re1w", "re2w"):
            t = u1pool.tile([128, 2048], F32R, tag="u1w")
            nc.sync.dma_start(t[:], wd[name].ap())
            wt[name] = t
        for n in range(8):
            p = ps.tile([128, 512], F32, tag="ps")
            for ki, (pin, qy, qx) in enumerate(re1_keys):
                o, rhs = mm_views(up2p[pin][:], p[:], qy, qx, n)
                if o is None:
                    continue
                nc.tensor.matmul(o, wt["re1w"][:, ki * 128 : (ki + 1) * 128], rhs,
                                 start=(ki == 0), stop=(ki == len(re1_keys) - 1),
                                 skip_group_check=True)
            nc.scalar.activation(pimg(re1t[:])[:, n * 8 : (n + 1) * 8, 1:65],
                                 p[:].rearrange("p (y x) -> p y x", y=8),
                                 AF.Relu, bias=wt["re1b"][:])

        # ---------------- re2 (polyphase 3x3) + residual + interleave + out ----------------
        for n in range(8):
            pss = []
            for p4 in range(4):
                p = ps.tile([128, 512], F32, tag="ps")
                for qi, (qy, qx) in enumerate(re2_q[p4]):
                    o, rhs = mm_views(re1t[:], p[:], qy, qx, n)
                    if o is None:
                        continue
                    nc.tensor.matmul(o, wt["re2w"][:, (p4 * 4 + qi) * 128 :
                                                   (p4 * 4 + qi + 1) * 128], rhs,
                                     start=(qi == 0), stop=(qi == len(re2_q[p4]) - 1),
                                     skip_group_check=True)
                pss.append(p)
            for hb in range(2):  # half-bands of 8 output rows (4 phase rows)
                band = bnd.tile([128, 1024], F32, tag="bnd")
                bv = band[:].rearrange("p (y r x s) -> p y r x s", y=4, r=2, s=2)
                for p4 in range(4):
                    r, s = p4 // 2, p4 % 2
                    y0 = n * 8 + hb * 4
                    up_v = pimg(up2p[p4][:].bitcast(F32))[:, y0 : y0 + 4, 1:65]
                    nc.vector.scalar_tensor_tensor(
                        bv[:, :, r, :, s],
                        pss[p4][:, hb * 256 : (hb + 1) * 256].rearrange(
                            "p (y x) -> p y x", y=4),
                        wt["re2b"][:], up_v, AL.add, AL.add)
                nc.sync.dma_start(
                    out_d.ap()[:, (2 * n + hb) * 1024 : (2 * n + hb + 1) * 1024],
                    band[:])

    nc.compile()
    return nc


_NC = None


def _get_nc():
    global _NC
    if _NC is None:
        _NC = build()
    return _NC


def make_in_maps(inputs):
    w, _, _ = host_prep(inputs)
    x1 = np.ascontiguousarray(np.asarray(inputs["x1"], np.float32).reshape(NC, 256, 4096))
    x2 = np.ascontiguousarray(np.asarray(inputs["x2"], np.float32).reshape(NC, 256, 4096))
    in_maps = []
    for i in range(NC):
        m = {"x1": x1[i], "x2": x2[i]}
        m.update(w)
        in_maps.append(m)
    return in_maps


def kernel(**inputs):
    nc = _get_nc()
    in_maps = make_in_maps(inputs)
    res = run_bass_kernel_spmd(nc, in_maps, core_ids=list(range(NC)))
    out = np.stack([res.results[i]["out"].reshape(128, 128, 128) for i in range(NC)])
    return out.astype(np.float32)


# revision 31
# speedup vs baseline: 2.8298x; 1.1047x over previous
import sys

sys.path.insert(0, "/opt/trn_rl_repo")

from contextlib import ExitStack

import numpy as np

import concourse.bacc as bacc
import concourse.mybir as mybir
from concourse import tile
from concourse.bass_utils import run_bass_kernel_spmd

F32 = mybir.dt.float32
F32R = mybir.dt.float32r
AL = mybir.AluOpType
AF = mybir.ActivationFunctionType

C = 256
H = W = 64
NC = 8  # cores / batch shards


# ---------------------------------------------------------------- host prep
def host_prep(inp):
    """Rearrange all weights into [partition, free] layouts matching SBUF tiles."""
    d = {}
    f = np.float32

    # conditioning nets (dsc1, dsc2)
    for i, pre in ((0, "dsc1"), (1, "dsc2")):
        w1 = np.asarray(inp[f"{pre}_w1"], f)  # [64, 256]
        b1 = np.asarray(inp[f"{pre}_b1"], f)  # [64]
        w2 = np.asarray(inp[f"{pre}_w2"], f)  # [2304, 64]
        b2 = np.asarray(inp[f"{pre}_b2"], f)  # [2304]
        # lhsT for gm matmul: [k_local, chunk, m]; fold the 1/(H*W) mean here
        d[f"w1T{i}"] = np.ascontiguousarray(
            (w1.T / (H * W)).reshape(2, 128, 64).transpose(1, 0, 2)
        ).reshape(128, 128)
        d[f"b1_{i}"] = b1.reshape(64, 1).copy()
        # lhsT for wts matmul: [j, chunk, k, c_local]; fold gelu's 0.5 here
        d[f"w2r{i}"] = np.ascontiguousarray(
            (0.5 * w2).reshape(2, 128, 9, 64).transpose(3, 0, 2, 1)
        ).reshape(64, 2304)
        d[f"b2r{i}"] = np.ascontiguousarray(
            b2.reshape(2, 128, 9).transpose(1, 0, 2)
        ).reshape(128, 18)

    # identity for building runtime diagonal depthwise weight matrices
    d["ident"] = np.ascontiguousarray(np.eye(128, dtype=f))

    # channel_align 1x1: [k_local, kc, mc, m]
    aw = np.asarray(inp["align_w"], f)[:, :, 0, 0]  # [256, 512]
    d["alignw"] = np.ascontiguousarray(
        aw.reshape(2, 128, 4, 128).transpose(3, 2, 0, 1)
    ).reshape(128, 1024)
    d["alignb"] = np.ascontiguousarray(
        np.asarray(inp["align_b"], f).reshape(2, 128).T
    )  # [128, 2]

    # up conv1 3x3 C->4C with pixel-shuffle phase reorder:
    # new channel (p, g) -> orig channel 4g + p   (p = 2r+s)
    uw = np.asarray(inp["up_w1"], f)  # [1024, 256, 3, 3]
    a = uw.reshape(256, 4, 2, 128, 9)  # [g, p, kc, k_local, tap]
    a = a.reshape(2, 128, 4, 2, 128, 9)  # [mcin, m, p, kc, k_local, tap]
    d["up1w"] = np.ascontiguousarray(a.transpose(4, 2, 0, 5, 3, 1)).reshape(
        128, 4 * 2 * 9 * 2 * 128
    )  # [k_local, (p, mcin, tap, kc, m)]
    ub = np.asarray(inp["up_b1"], f)
    d["up1b"] = np.ascontiguousarray(
        ub.reshape(2, 128, 4).transpose(1, 2, 0)
    ).reshape(128, 8)  # [m, (p, mcin)]

    # up conv2 1x1 C->C/2 (per-phase): [k_local, kc, m]
    u2 = np.asarray(inp["up_w2"], f)[:, :, 0, 0]  # [128, 256]
    d["up2w"] = np.ascontiguousarray(
        u2.reshape(128, 2, 128).transpose(2, 1, 0)
    ).reshape(128, 256)
    d["up2b"] = np.asarray(inp["up_b2"], f).reshape(128, 1).copy()

    # ---- polyphase re_enhance ----
    # phase/tap -> (in-phase, sub-shift) mapping
    def split(v):  # v = r + dy - 1
        rp = v % 2
        return rp, (v - rp) // 2

    r1w = np.asarray(inp["re_w1"], f)  # [32, 128, 3, 3]
    keymap = {}
    for p in range(4):
        r, s = p // 2, p % 2
        for dy in range(3):
            for dx in range(3):
                rp, qy = split(r + dy - 1)
                sp, qx = split(s + dx - 1)
                keymap.setdefault((2 * rp + sp, qy, qx), []).append((p, dy, dx))
    keys = sorted(keymap.keys(), key=lambda k: (k[1] != 0 or k[2] != 0, k))
    re1_keys = keys  # list of (p_in, qy, qx); all-(0,0) shifts first
    re1w = np.zeros((128, 16, 128), f)
    for ki, key in enumerate(keys):
        for (p, dy, dx) in keymap[key]:
            re1w[:, ki, p * 32 : (p + 1) * 32] = r1w[:, :, dy, dx].T
    d["re1w"] = re1w.reshape(128, 2048)
    d["re1b"] = np.tile(np.asarray(inp["re_b1"], f), 4).reshape(128, 1)

    r2w = np.asarray(inp["re_w2"], f)  # [128, 32, 3, 3]
    re2_q = []  # per out-phase list of shifts, (0,0) first
    re2w = np.zeros((128, 4, 4, 128), f)
    for p in range(4):
        r, s = p // 2, p % 2
        qys = sorted({split(r + dy - 1)[1] for dy in range(3)}, key=lambda q: q != 0)
        qxs = sorted({split(s + dx - 1)[1] for dx in range(3)}, key=lambda q: q != 0)
        qs = [(qy, qx) for qy in qys for qx in qxs]
        qs.sort(key=lambda q: q != (0, 0))
        re2_q.append(qs)
        for qi, (qy, qx) in enumerate(qs):
            for pp in range(4):
                rp, sp = pp // 2, pp % 2
                dy = 2 * qy + rp - r + 1
                dx = 2 * qx + sp - s + 1
                if 0 <= dy < 3 and 0 <= dx < 3:
                    re2w[pp * 32 : (pp + 1) * 32, p, qi, :] = r2w[:, :, dy, dx].T
    d["re2w"] = re2w.reshape(128, 2048)
    d["re2b"] = np.asarray(inp["re_b2"], f).reshape(128, 1).copy()

    return d, re1_keys, re2_q


RE1_KEYS = None
RE2_Q = None


def _mapping():
    global RE1_KEYS, RE2_Q
    if RE1_KEYS is None:
        zeros = {k: np.zeros(v) for k, v in [
            ("dsc1_w1", (64, 256)), ("dsc1_b1", (64,)), ("dsc1_w2", (2304, 64)),
            ("dsc1_b2", (2304,)), ("dsc2_w1", (64, 256)), ("dsc2_b1", (64,)),
            ("dsc2_w2", (2304, 64)), ("dsc2_b2", (2304,)),
            ("align_w", (256, 512, 1, 1)), ("align_b", (256,)),
            ("up_w1", (1024, 256, 3, 3)), ("up_b1", (1024,)),
            ("up_w2", (128, 256, 1, 1)), ("up_b2", (128,)),
            ("re_w1", (32, 128, 3, 3)), ("re_b1", (32,)),
            ("re_w2", (128, 32, 3, 3)), ("re_b2", (128,)),
        ]}
        _, RE1_KEYS, RE2_Q = host_prep(zeros)
    return RE1_KEYS, RE2_Q


# ---------------------------------------------------------------- bass build
def img(ap):
    """[128, 64, 64] view of a flat [128, 4096] image."""
    return ap.rearrange("p (y x) -> p y x", y=64)


def pimg(ap):
    """View of a column-padded [128, 64*66] image (zero cols at x=0 and x=65)."""
    return ap.rearrange("p (y x) -> p y x", x=66)


def mm_views(src_ap, psum_ap, sy, sx, n):
    """(psum_out, rhs) for 8-row chunk n of a shifted conv tap on a col-padded
    source. Rows clipped by sy; columns handled by the zero pad. psum out is a
    contiguous 2D region (an f32r-matmul ISA requirement)."""
    y0 = max(0, -sy)
    y1 = 64 + min(0, -sy)
    r0 = max(8 * n, y0)
    r1 = min(8 * n + 8, y1)
    if r1 <= r0:
        return None, None
    v = pimg(src_ap)
    rhs = v[:, r0 + sy : r1 + sy, 1 + sx : 65 + sx]
    out = psum_ap[:, (r0 - 8 * n) * 64 : (r1 - 8 * n) * 64]
    return out, rhs


def build():
    re1_keys, re2_q = _mapping()
    nc = bacc.Bacc(trn_type="TRN2", target_bir_lowering=False, debug=False)

    x_d = [nc.dram_tensor(n, [256, 4224], F32R, kind="ExternalInput") for n in ("x1", "x2")]
    wd = {}
    for name, shape, dt in [
        ("w1T0", [128, 128], F32), ("w1T1", [128, 128], F32),
        ("b1_0", [64, 1], F32), ("b1_1", [64, 1], F32),
        ("w2r0", [64, 2304], F32), ("w2r1", [64, 2304], F32),
        ("b2r0", [128, 18], F32), ("b2r1", [128, 18], F32),
        ("ident", [128, 128], F32R),
        ("alignw", [128, 1024], F32R), ("alignb", [128, 2], F32),
        ("up1w", [128, 18432], F32R), ("up1b", [128, 8], F32),
        ("up2w", [128, 256], F32R), ("up2b", [128, 1], F32),
        ("re1w", [128, 2048], F32R), ("re1b", [128, 1], F32),
        ("re2w", [128, 2048], F32R), ("re2b", [128, 1], F32),
    ]:
        wd[name] = nc.dram_tensor(name, shape, dt, kind="ExternalInput")
    out_d = nc.dram_tensor("out", [128, 16384], F32, kind="ExternalOutput")

    with tile.TileContext(nc) as tc, ExitStack() as ctx:
        wpool = ctx.enter_context(tc.tile_pool(name="w", bufs=1))
        u1pool = ctx.enter_context(tc.tile_pool(name="u1w", bufs=4))
        big = ctx.enter_context(tc.tile_pool(name="big", bufs=8))
        dpool = ctx.enter_context(tc.tile_pool(name="dg", bufs=2))
        stg = ctx.enter_context(tc.tile_pool(name="stg", bufs=2))
        bnd = ctx.enter_context(tc.tile_pool(name="bnd", bufs=3))
        tiny = ctx.enter_context(tc.tile_pool(name="tiny", bufs=5))
        ps = ctx.enter_context(tc.tile_pool(name="ps", bufs=7, space="PSUM"))
        psc = ctx.enter_context(tc.tile_pool(name="psc", bufs=1, space="PSUM"))

        wt = {}

        def load_w(name):
            t = wpool.tile(list(wd[name].shape), wd[name].dtype, tag=name, name=name)
            nc.sync.dma_start(t[:], wd[name].ap())
            wt[name] = t

        def img_tile(name="it"):
            return big.tile([128, 4096], F32R, tag="big", name=name)

        def pad_tile(name="pt"):
            """[128, 64*66] tile; interior written by evictions, pad cols
            zeroed here (x DMAs bring zeros from the host instead)."""
            t = big.tile([128, 4224], F32R, tag="big", name=name)
            v = pimg(t[:].bitcast(F32))
            nc.gpsimd.memset(v[:, :, 0:1], 0.0)
            nc.gpsimd.memset(v[:, :, 65:66], 0.0)
            return t

        w2rt = {}

        def load_w2r(d):
            if d not in w2rt:
                t = u1pool.tile([64, 2304], F32, tag="u1w", name="w2rt")
                nc.sync.dma_start(t[:], wd[f"w2r{d}"].ap())
                w2rt[d] = t
            return w2rt[d]

        # DMA order is the startup critical path: x1 chunks and the weights
        # cond1(x1) needs come first, everything else after.
        xin = [[None, None], [None, None]]

        def load_x(i, c):
            t = big.tile([128, 4224], F32R, tag="big", name="xin")
            nc.sync.dma_start(t[:], x_d[i].ap()[c * 128 : (c + 1) * 128, :])
            xin[i][c] = t

        load_x(0, 0)
        load_x(0, 1)
        for name in ("w1T0", "b1_0", "b2r0", "ident"):
            load_w(name)
        load_w2r(0)
        for name in ("w1T1", "b1_1", "b2r1", "alignb", "up2w", "up2b",
                     "re1b", "re2b", "up1b"):
            load_w(name)
        load_w2r(1)
        load_x(1, 0)
        load_x(1, 1)

        # ---------------- dsc stage ----------------
        d_init = [0]  # number of dpool buffers memset so far

        def conditioning(d, gms):
            """gms: two [128,1] per-chunk channel-sum tiles -> per-chunk
            diagonal-tap matrices D [128, 9*128] (f32r) for the PE depthwise."""
            pgt = psc.tile([128, 9], F32, tag="psc", name="pgt")
            pg = pgt[0:64, 0:1]
            for c in range(2):
                nc.tensor.matmul(pg, wt[f"w1T{d}"][:, c * 64 : (c + 1) * 64],
                                 gms[c][:], start=(c == 0), stop=(c == 1))
            u = tiny.tile([64, 1], F32, tag="u")
            nc.scalar.activation(u[:], pg, AF.Identity, bias=wt[f"b1_{d}"][:])
            sq = tiny.tile([64, 1], F32, tag="sq")
            nc.scalar.activation(sq[:], u[:], AF.Square)
            v3 = tiny.tile([64, 1], F32, tag="v3")
            nc.vector.scalar_tensor_tensor(v3[:], sq[:], 0.044715, u[:], AL.mult, AL.mult)
            w_ = tiny.tile([64, 1], F32, tag="w_")
            nc.vector.tensor_tensor(w_[:], v3[:], u[:], AL.add)
            th = tiny.tile([64, 1], F32, tag="th")
            nc.scalar.activation(th[:], w_[:], AF.Tanh, scale=0.7978845608028654)
            hv = tiny.tile([64, 1], F32, tag="hv")
            nc.vector.scalar_tensor_tensor(hv[:], th[:], 1.0, u[:], AL.add, AL.mult)

            w2r = load_w2r(d)
            D_l, wts_l = [], []
            for c in range(2):
                pw = psc.tile([128, 9], F32, tag="psc", name="pw")
                for k in range(9):
                    nc.tensor.matmul(pw[:, k : k + 1],
                                     w2r[:, (c * 9 + k) * 128 : (c * 9 + k + 1) * 128],
                                     hv[:], start=True, stop=True)
                raw = tiny.tile([128, 9], F32, tag="raw")
                nc.vector.tensor_tensor(raw[:], pw[:],
                                        wt[f"b2r{d}"][:, c * 9 : (c + 1) * 9], AL.add)
                mx = tiny.tile([128, 1], F32, tag="mx")
                nc.vector.tensor_reduce(mx[:], raw[:], axis=mybir.AxisListType.X, op=AL.max)
                ngm = tiny.tile([128, 1], F32, tag="ngm")
                nc.vector.tensor_scalar_mul(ngm[:], mx[:], -1.0)
                ex = tiny.tile([128, 9], F32, tag="ex")
                ssum = tiny.tile([128, 1], F32, tag="ssum")
                nc.scalar.activation(ex[:], raw[:], AF.Exp, bias=ngm[:], accum_out=ssum[:])
                rec = tiny.tile([128, 1], F32, tag="rec")
                nc.vector.reciprocal(rec[:], ssum[:])
                wts = tiny.tile([128, 9], F32, tag="wts")
                nc.vector.tensor_scalar_mul(wts[:], ex[:], rec[:])
                w4p = tiny.tile([128, 1], F32, tag="w4p")
                nc.vector.tensor_scalar_add(w4p[:], wts[:, 4:5], 1.0)
                # diagonal tap matrices (center tap has +1 residual folded in)
                D = dpool.tile([128, 1152], F32R, tag="D")
                if d_init[0] < 2:
                    nc.gpsimd.memset(D[:].bitcast(F32), 0.0)
                    d_init[0] += 1
                for k in range(9):
                    ptr = w4p[:] if k == 4 else wts[:, k : k + 1]
                    nc.vector.tensor_scalar_mul(
                        D[:, k * 128 : (k + 1) * 128], wt["ident"][:], ptr)
                D_l.append(D)
                wts_l.append(wts)
            return D_l, wts_l

        TAPS = (4, 0, 1, 2, 3, 5, 6, 7, 8)  # center first: full psum coverage

        def dw_pe(src, dst, D, relu, gacc, wts=None):
            """dst = depthwise(src) (+x via center tap) on PE; evict via Act.
            With wts given (no-relu apps only), taps 0,1 run on DVE and tap 2
            on Pool as post-eviction MACs into dst, shrinking the PE share."""
            off = () if wts is None else (0, 1)
            for n in range(8):
                p = ps.tile([128, 512], F32, tag="ps")
                first = True
                for k in TAPS:
                    if k in off:
                        continue
                    sy, sx = k // 3 - 1, k % 3 - 1
                    o, rhs = mm_views(src[:], p[:], sy, sx, n)
                    if o is None:
                        continue
                    nc.tensor.matmul(o, D[:, k * 128 : (k + 1) * 128], rhs,
                                     start=first, stop=(k == TAPS[-1]),
                                     skip_group_check=True)
                    first = False
                if relu:
                    nc.scalar.activation(pimg(dst[:])[:, n * 8 : (n + 1) * 8, 1:65],
                                         p[:].rearrange("p (y x) -> p y x", y=8),
                                         AF.Relu, accum_out=gacc[:, n : n + 1])
                else:
                    nc.scalar.activation(dst[:, n * 512 : (n + 1) * 512], p[:],
                                         AF.Identity)
            for k in off:
                # per-partition-scalar MACs are DVE-only on hardware (Pool
                # lacks TensorScalarPtr); src pad columns supply the x-shift
                # zeros
                sy, sx = k // 3 - 1, k % 3 - 1
                sv = pimg(src[:])[:, 1 + sy : 64 + sy, 1 + sx : 65 + sx]
                dv = img(dst[:])[:, 1:64, :]
                nc.vector.scalar_tensor_tensor(dv, sv, wts[:, k : k + 1], dv,
                                               AL.mult, AL.add)

        # per-chunk channel sums of the inputs; x2's are emitted later so they
        # don't delay cond1(x1)'s D builds on the in-order DVE
        gms1 = [[None, None], [None, None]]

        def reduce_gms1(i, use_act=False):
            for c in range(2):
                g = tiny.tile([128, 1], F32, tag="gms")
                if use_act and c == 1:
                    # idle Act engine: in-place copy whose accumulator is the
                    # channel sum; runs concurrently with DVE's c0 reduce
                    nc.scalar.activation(xin[i][c][:], xin[i][c][:], AF.Copy,
                                         accum_out=g[:])
                else:
                    nc.vector.tensor_reduce(g[:], xin[i][c][:].bitcast(F32),
                                            axis=mybir.AxisListType.X, op=AL.add)
                gms1[i][c] = g

        reduce_gms1(0, use_act=True)

        mid = [[None, None], [None, None]]
        gacc = [[None, None], [None, None]]
        gms2 = [[None, None], [None, None]]
        y = [[None, None], [None, None]]

        def alloc_mid(i):
            for c in range(2):
                mid[i][c] = pad_tile(name="midt")
                gacc[i][c] = tiny.tile([128, 8], F32, tag="gacc", name="gacc")

        def reduce_gms2(i):
            for c in range(2):
                g2 = tiny.tile([128, 1], F32, tag="gms")
                nc.vector.tensor_reduce(g2[:], gacc[i][c][:],
                                        axis=mybir.AxisListType.X, op=AL.add)
                gms2[i][c] = g2

        # interleave conditioning (PE-tiny + DVE/Act chain) between the big
        # PE depthwise apps so PE never waits on a conditioning chain.
        D1x1, _ = conditioning(0, gms1[0])
        alloc_mid(0)
        reduce_gms1(1)
        dw_pe(xin[0][0], mid[0][0], D1x1[0], True, gacc[0][0])
        D1x2, _ = conditioning(0, gms1[1])
        alloc_mid(1)
        dw_pe(xin[0][1], mid[0][1], D1x1[1], True, gacc[0][1])
        dw_pe(xin[1][0], mid[1][0], D1x2[0], True, gacc[1][0])
        reduce_gms2(0)
        D2x1, wts2x1 = conditioning(1, gms2[0])
        dw_pe(xin[1][1], mid[1][1], D1x2[1], True, gacc[1][1])
        reduce_gms2(1)
        for c in range(2):
            y[0][c] = big.tile([128, 4096], F32R, tag="big", name="yt")
        dw_pe(mid[0][0], y[0][0], D2x1[0], False, None, wts=wts2x1[0][:])
        D2x2, wts2x2 = conditioning(1, gms2[1])
        dw_pe(mid[0][1], y[0][1], D2x1[1], False, None, wts=wts2x1[1][:])
        for c in range(2):
            y[1][c] = big.tile([128, 4096], F32R, tag="big", name="yt")
        dw_pe(mid[1][0], y[1][0], D2x2[0], False, None, wts=wts2x2[0][:])
        dw_pe(mid[1][1], y[1][1], D2x2[1], False, None)

        # ---------------- align 1x1 (2C -> C) ----------------
        awt = u1pool.tile([128, 1024], F32R, tag="u1w")
        nc.sync.dma_start(awt[:], wd["alignw"].ap())
        wt["alignw"] = awt
        fus = []
        for mc in range(2):
            fus.append(pad_tile(name="fus"))
        for mc in range(2):
            for n in range(8):
                p = ps.tile([128, 512], F32, tag="ps")
                for kc in range(4):
                    rhs = y[kc // 2][kc % 2][:, n * 512 : (n + 1) * 512]
                    nc.tensor.matmul(
                        p[:], wt["alignw"][:, (kc * 2 + mc) * 128 : (kc * 2 + mc + 1) * 128],
                        rhs, start=(kc == 0), stop=(kc == 3))
                nc.scalar.activation(pimg(fus[mc][:])[:, n * 8 : (n + 1) * 8, 1:65],
                                     p[:].rearrange("p (y x) -> p y x", y=8),
                                     AF.Identity, bias=wt["alignb"][:, mc : mc + 1])

        # ---------------- up1 (3x3 C->4C, phase-ordered) + up2 (1x1) ----------------
        taps = [(1, 1)] + [(dy, dx) for dy in range(3) for dx in range(3) if (dy, dx) != (1, 1)]
        up2p = []
        for p4 in range(4):
            up2p.append(pad_tile(name="up2p"))
        for p4 in range(4):
            u1t = []
            for mcin in range(2):
                th = u1pool.tile([128, 2304], F32R, tag="u1w")
                off = (p4 * 2 + mcin) * 2304
                nc.sync.dma_start(th[:], wd["up1w"].ap()[:, off : off + 2304])
                u1t.append(th)
            for n in range(8):
                stage = []
                for mcin in range(2):
                    p = ps.tile([128, 512], F32, tag="ps")
                    first = True
                    for (dy, dx) in taps:
                        sy, sx = dy - 1, dx - 1
                        for kc in range(2):
                            o, rhs = mm_views(fus[kc][:], p[:], sy, sx, n)
                            if o is None:
                                continue
                            lhsT = u1t[mcin][:, ((dy * 3 + dx) * 2 + kc) * 128 :
                                             ((dy * 3 + dx) * 2 + kc + 1) * 128]
                            nc.tensor.matmul(o, lhsT, rhs, start=first,
                                             stop=((dy, dx) == taps[-1] and kc == 1),
                                             skip_group_check=True)
                            first = False
                    st = stg.tile([128, 512], F32R, tag="stg")
                    nc.scalar.activation(st[:], p[:], AF.Identity,
                                         bias=wt["up1b"][:, p4 * 2 + mcin : p4 * 2 + mcin + 1])
                    stage.append(st)
                p2 = ps.tile([128, 512], F32, tag="ps")
                for kc in range(2):
                    nc.tensor.matmul(p2[:], wt["up2w"][:, kc * 128 : (kc + 1) * 128],
                                     stage[kc][:], start=(kc == 0), stop=(kc == 1))
                nc.scalar.activation(pimg(up2p[p4][:])[:, n * 8 : (n + 1) * 8, 1:65],
                                     p2[:].rearrange("p (y x) -> p y x", y=8),
                                     AF.Identity, bias=wt["up2b"][:])

        # ---------------- re1 (polyphase 3x3, M-packed) ----------------
        re1t = pad_tile(name="re1t")
        for name in ("re1w", "re2w"):
            t = u1pool.tile([128, 2048], F32R, tag="u1w")
            nc.sync.dma_start(t[:], wd[name].ap())
            wt[name] = t
        for n in range(8):
            p = ps.tile([128, 512], F32, tag="ps")
            for ki, (pin, qy, qx) in enumerate(re1_keys):
                o, rhs = mm_views(up2p[pin][:], p[:], qy, qx, n)
                if o is None:
                    continue
                nc.tensor.matmul(o, wt["re1w"][:, ki * 128 : (ki + 1) * 128], rhs,
                                 start=(ki == 0), stop=(ki == len(re1_keys) - 1),
                                 skip_group_check=True)
            nc.scalar.activation(pimg(re1t[:])[:, n * 8 : (n + 1) * 8, 1:65],
                                 p[:].rearrange("p (y x) -> p y x", y=8),
                                 AF.Relu, bias=wt["re1b"][:])

        # ---------------- re2 (polyphase 3x3) + residual + interleave + out ----------------
        for n in range(8):
            pss = []
            for p4 in range(4):
                p = ps.tile([128, 512], F32, tag="ps")
                for qi, (qy, qx) in enumerate(re2_q[p4]):
                    o, rhs = mm_views(re1t[:], p[:], qy, qx, n)
                    if o is None:
                        continue
                    nc.tensor.matmul(o, wt["re2w"][:, (p4 * 4 + qi) * 128 :
                                                   (p4 * 4 + qi + 1) * 128], rhs,
                                     start=(qi == 0), stop=(qi == len(re2_q[p4]) - 1),
                                     skip_group_check=True)
                pss.append(p)
            # p4=3 detours via Act (psum evict + bias) so Pool (no PSUM
            # access) can do its residual add from SBUF; p4 0-2 are DVE
            # STTs straight from psum. Keeps every engine under PE's pace.
            ret = {}
            for p4 in (2, 3):
                t = stg.tile([128, 512], F32, tag="ret", name="ret", bufs=4)
                nc.scalar.activation(t[:], pss[p4][:], AF.Identity,
                                     bias=wt["re2b"][:])
                ret[p4] = t
            for hb in range(2):  # half-bands of 8 output rows (4 phase rows)
                band = bnd.tile([128, 1024], F32, tag="bnd")
                bv = band[:].rearrange("p (y r x s) -> p y r x s", y=4, r=2, s=2)
                for p4 in range(4):
                    r, s = p4 // 2, p4 % 2
                    y0 = n * 8 + hb * 4
                    up_v = pimg(up2p[p4][:].bitcast(F32))[:, y0 : y0 + 4, 1:65]
                    if p4 >= 2:
                        nc.gpsimd.tensor_tensor(
                            bv[:, :, r, :, s],
                            ret[p4][:, hb * 256 : (hb + 1) * 256].rearrange(
                                "p (y x) -> p y x", y=4),
                            up_v, AL.add)
                    else:
                        nc.vector.scalar_tensor_tensor(
                            bv[:, :, r, :, s],
                            pss[p4][:, hb * 256 : (hb + 1) * 256].rearrange(
                                "p (y x) -> p y x", y=4),
                            wt["re2b"][:], up_v, AL.add, AL.add)
                nc.sync.dma_start(
                    out_d.ap()[:, (2 * n + hb) * 1024 : (2 * n + hb + 1) * 1024],
                    band[:])

    nc.compile()
    return nc


_NC = None


def _get_nc():
    global _NC
    if _NC is None:
        _NC = build()
    return _NC


def make_in_maps(inputs):
    w, _, _ = host_prep(inputs)
    def hostpad(x):
        x = np.asarray(x, np.float32).reshape(NC, 256, 64, 64)
        xp = np.zeros((NC, 256, 64, 66), np.float32)
        xp[:, :, :, 1:65] = x
        return np.ascontiguousarray(xp.reshape(NC, 256, 4224))

    x1 = hostpad(inputs["x1"])
    x2 = hostpad(inputs["x2"])
    in_maps = []
    for i in range(NC):
        m = {"x1": x1[i], "x2": x2[i]}
        m.update(w)
        in_maps.append(m)
    return in_maps


def kernel(**inputs):
    nc = _get_nc()
    in_maps = make_in_maps(inputs)
    res = run_bass_kernel_spmd(nc, in_maps, core_ids=list(range(NC)))
    out = np.stack([res.results[i]["out"].reshape(128, 128, 128) for i in range(NC)])
    return out.astype(np.float32)


# revision 39
# speedup vs baseline: 3.3441x; 1.1817x over previous
import sys

sys.path.insert(0, "/opt/trn_rl_repo")

from contextlib import ExitStack

import numpy as np

import concourse.bacc as bacc
import concourse.mybir as mybir
from concourse import tile
from concourse.bass_utils import run_bass_kernel_spmd

F32 = mybir.dt.float32
F32R = mybir.dt.float32r
BF16 = mybir.dt.bfloat16
AL = mybir.AluOpType
AF = mybir.ActivationFunctionType

C = 256
H = W = 64
NC = 8  # cores / batch shards


# ---------------------------------------------------------------- host prep
def host_prep(inp):
    """Rearrange all weights into [partition, free] layouts matching SBUF tiles."""
    d = {}
    f = np.float32

    # conditioning nets (dsc1, dsc2)
    for i, pre in ((0, "dsc1"), (1, "dsc2")):
        w1 = np.asarray(inp[f"{pre}_w1"], f)  # [64, 256]
        b1 = np.asarray(inp[f"{pre}_b1"], f)  # [64]
        w2 = np.asarray(inp[f"{pre}_w2"], f)  # [2304, 64]
        b2 = np.asarray(inp[f"{pre}_b2"], f)  # [2304]
        # lhsT for gm matmul: [k_local, chunk, m]; fold the 1/(H*W) mean here
        d[f"w1T{i}"] = np.ascontiguousarray(
            (w1.T / (H * W)).reshape(2, 128, 64).transpose(1, 0, 2)
        ).reshape(128, 128)
        d[f"b1_{i}"] = b1.reshape(64, 1).copy()
        # lhsT for wts matmul: [j, chunk, k, c_local]; fold gelu's 0.5 here
        d[f"w2r{i}"] = np.ascontiguousarray(
            (0.5 * w2).reshape(2, 128, 9, 64).transpose(3, 0, 2, 1)
        ).reshape(64, 2304).astype(mybir.dt.np(mybir.dt.bfloat16))
        d[f"b2r{i}"] = np.ascontiguousarray(
            b2.reshape(2, 128, 9).transpose(1, 0, 2)
        ).reshape(128, 18)

    # identity for building runtime diagonal depthwise weight matrices
    d["ident"] = np.ascontiguousarray(np.eye(128, dtype=f))

    # channel_align 1x1: [k_local, kc, mc, m]
    aw = np.asarray(inp["align_w"], f)[:, :, 0, 0]  # [256, 512]
    d["alignw"] = np.ascontiguousarray(
        aw.reshape(2, 128, 4, 128).transpose(3, 2, 0, 1)
    ).reshape(128, 1024)
    d["alignb"] = np.ascontiguousarray(
        np.asarray(inp["align_b"], f).reshape(2, 128).T
    )  # [128, 2]

    # up conv1 3x3 C->4C, pixel-shuffle phase reorder + 1D-Winograd F(2,3)
    # along x: 4 position weights per (dy, kc) replace the 3 dx taps.
    # new channel (p, g) -> orig channel 4g + p   (p = 2r+s)
    uw = np.asarray(inp["up_w1"], f)  # [1024, 256, 3, 3]
    w6 = uw.reshape(256, 4, 2, 128, 3, 3)
    w6 = w6.reshape(2, 128, 4, 2, 128, 3, 3)  # [mcin, m, p, kc, k, dy, dx]
    g0, g1, g2 = w6[..., 0], w6[..., 1], w6[..., 2]
    q = np.stack([g0, 0.5 * (g0 + g1 + g2), -0.5 * (g0 - g1 + g2), g2])
    # q: [pos, mcin, m, p, kc, k, dy] -> lhsT [k, (p, mcin, pos, dy, kc, m)]
    d["up1w"] = np.ascontiguousarray(q.transpose(5, 3, 1, 0, 6, 4, 2)).reshape(
        128, 4 * 2 * 4 * 3 * 2 * 128
    ).astype(mybir.dt.np(mybir.dt.bfloat16))
    ub = np.asarray(inp["up_b1"], f)
    d["up1b"] = np.ascontiguousarray(
        ub.reshape(2, 128, 4).transpose(1, 2, 0)
    ).reshape(128, 8)  # [m, (p, mcin)]

    # up conv2 1x1 C->C/2 (per-phase): [k_local, kc, m]
    u2 = np.asarray(inp["up_w2"], f)[:, :, 0, 0]  # [128, 256]
    d["up2w"] = np.ascontiguousarray(
        u2.reshape(128, 2, 128).transpose(2, 1, 0)
    ).reshape(128, 256)
    d["up2b"] = np.asarray(inp["up_b2"], f).reshape(128, 1).copy()

    # ---- polyphase re_enhance ----
    # phase/tap -> (in-phase, sub-shift) mapping
    def split(v):  # v = r + dy - 1
        rp = v % 2
        return rp, (v - rp) // 2

    r1w = np.asarray(inp["re_w1"], f)  # [32, 128, 3, 3]
    keymap = {}
    for p in range(4):
        r, s = p // 2, p % 2
        for dy in range(3):
            for dx in range(3):
                rp, qy = split(r + dy - 1)
                sp, qx = split(s + dx - 1)
                keymap.setdefault((2 * rp + sp, qy, qx), []).append((p, dy, dx))
    keys = sorted(keymap.keys(), key=lambda k: (k[1] != 0 or k[2] != 0, k))
    re1_keys = keys  # list of (p_in, qy, qx); all-(0,0) shifts first
    re1w = np.zeros((128, 16, 128), f)
    for ki, key in enumerate(keys):
        for (p, dy, dx) in keymap[key]:
            re1w[:, ki, p * 32 : (p + 1) * 32] = r1w[:, :, dy, dx].T
    d["re1w"] = re1w.reshape(128, 2048)
    d["re1b"] = np.tile(np.asarray(inp["re_b1"], f), 4).reshape(128, 1)

    r2w = np.asarray(inp["re_w2"], f)  # [128, 32, 3, 3]
    re2_q = []  # per out-phase list of shifts, (0,0) first
    re2w = np.zeros((128, 4, 4, 128), f)
    for p in range(4):
        r, s = p // 2, p % 2
        qys = sorted({split(r + dy - 1)[1] for dy in range(3)}, key=lambda q: q != 0)
        qxs = sorted({split(s + dx - 1)[1] for dx in range(3)}, key=lambda q: q != 0)
        qs = [(qy, qx) for qy in qys for qx in qxs]
        qs.sort(key=lambda q: q != (0, 0))
        re2_q.append(qs)
        for qi, (qy, qx) in enumerate(qs):
            for pp in range(4):
                rp, sp = pp // 2, pp % 2
                dy = 2 * qy + rp - r + 1
                dx = 2 * qx + sp - s + 1
                if 0 <= dy < 3 and 0 <= dx < 3:
                    re2w[pp * 32 : (pp + 1) * 32, p, qi, :] = r2w[:, :, dy, dx].T
    d["re2w"] = re2w.reshape(128, 2048)
    d["re2b"] = np.asarray(inp["re_b2"], f).reshape(128, 1).copy()

    return d, re1_keys, re2_q


RE1_KEYS = None
RE2_Q = None


def _mapping():
    global RE1_KEYS, RE2_Q
    if RE1_KEYS is None:
        zeros = {k: np.zeros(v) for k, v in [
            ("dsc1_w1", (64, 256)), ("dsc1_b1", (64,)), ("dsc1_w2", (2304, 64)),
            ("dsc1_b2", (2304,)), ("dsc2_w1", (64, 256)), ("dsc2_b1", (64,)),
            ("dsc2_w2", (2304, 64)), ("dsc2_b2", (2304,)),
            ("align_w", (256, 512, 1, 1)), ("align_b", (256,)),
            ("up_w1", (1024, 256, 3, 3)), ("up_b1", (1024,)),
            ("up_w2", (128, 256, 1, 1)), ("up_b2", (128,)),
            ("re_w1", (32, 128, 3, 3)), ("re_b1", (32,)),
            ("re_w2", (128, 32, 3, 3)), ("re_b2", (128,)),
        ]}
        _, RE1_KEYS, RE2_Q = host_prep(zeros)
    return RE1_KEYS, RE2_Q


# ---------------------------------------------------------------- bass build
def img(ap):
    """[128, 64, 64] view of a flat [128, 4096] image."""
    return ap.rearrange("p (y x) -> p y x", y=64)


def pimg(ap):
    """View of a column-padded [128, 64*66] image (zero cols at x=0 and x=65)."""
    return ap.rearrange("p (y x) -> p y x", x=66)


def mm_views(src_ap, psum_ap, sy, sx, n):
    """(psum_out, rhs) for 8-row chunk n of a shifted conv tap on a col-padded
    source. Rows clipped by sy; columns handled by the zero pad. psum out is a
    contiguous 2D region (an f32r-matmul ISA requirement)."""
    y0 = max(0, -sy)
    y1 = 64 + min(0, -sy)
    r0 = max(8 * n, y0)
    r1 = min(8 * n + 8, y1)
    if r1 <= r0:
        return None, None
    v = pimg(src_ap)
    rhs = v[:, r0 + sy : r1 + sy, 1 + sx : 65 + sx]
    out = psum_ap[:, (r0 - 8 * n) * 64 : (r1 - 8 * n) * 64]
    return out, rhs


def build():
    re1_keys, re2_q = _mapping()
    nc = bacc.Bacc(trn_type="TRN2", target_bir_lowering=False, debug=False)

    x_d = [nc.dram_tensor(n, [256, 4224], F32R, kind="ExternalInput") for n in ("x1", "x2")]
    wd = {}
    for name, shape, dt in [
        ("w1T0", [128, 128], F32), ("w1T1", [128, 128], F32),
        ("b1_0", [64, 1], F32), ("b1_1", [64, 1], F32),
        ("w2r0", [64, 2304], BF16), ("w2r1", [64, 2304], BF16),
        ("b2r0", [128, 18], F32), ("b2r1", [128, 18], F32),
        ("ident", [128, 128], F32R),
        ("alignw", [128, 1024], F32R), ("alignb", [128, 2], F32),
        ("up1w", [128, 24576], BF16), ("up1b", [128, 8], F32),
        ("up2w", [128, 256], F32R), ("up2b", [128, 1], F32),
        ("re1w", [128, 2048], F32R), ("re1b", [128, 1], F32),
        ("re2w", [128, 2048], F32R), ("re2b", [128, 1], F32),
    ]:
        wd[name] = nc.dram_tensor(name, shape, dt, kind="ExternalInput")
    out_d = nc.dram_tensor("out", [128, 16384], F32, kind="ExternalOutput")

    with tile.TileContext(nc) as tc, ExitStack() as ctx:
        wpool = ctx.enter_context(tc.tile_pool(name="w", bufs=1))
        u1pool = ctx.enter_context(tc.tile_pool(name="u1w", bufs=4))
        big = ctx.enter_context(tc.tile_pool(name="big", bufs=6))
        upool = ctx.enter_context(tc.tile_pool(name="upos", bufs=8))
        dpool = ctx.enter_context(tc.tile_pool(name="dg", bufs=2))
        stg = ctx.enter_context(tc.tile_pool(name="stg", bufs=2))
        bnd = ctx.enter_context(tc.tile_pool(name="bnd", bufs=3))
        tiny = ctx.enter_context(tc.tile_pool(name="tiny", bufs=4))
        ps = ctx.enter_context(tc.tile_pool(name="ps", bufs=7, space="PSUM"))
        psc = ctx.enter_context(tc.tile_pool(name="psc", bufs=1, space="PSUM"))

        wt = {}

        def load_w(name):
            t = wpool.tile(list(wd[name].shape), wd[name].dtype, tag=name, name=name)
            nc.sync.dma_start(t[:], wd[name].ap())
            wt[name] = t

        def img_tile(name="it"):
            return big.tile([128, 4096], F32R, tag="big", name=name)

        def pad_tile(name="pt"):
            """[128, 64*66] tile; interior written by evictions, pad cols
            zeroed here (x DMAs bring zeros from the host instead)."""
            t = big.tile([128, 4224], F32R, tag="big", name=name)
            v = pimg(t[:].bitcast(F32))
            nc.gpsimd.memset(v[:, :, 0:1], 0.0)
            nc.gpsimd.memset(v[:, :, 65:66], 0.0)
            return t

        w2rt = {}

        def load_w2r(d):
            if d not in w2rt:
                t = u1pool.tile([64, 2304], BF16, tag="u1w", name="w2rt")
                nc.sync.dma_start(t[:], wd[f"w2r{d}"].ap())
                w2rt[d] = t
            return w2rt[d]

        # DMA order is the startup critical path: x1 chunks and the weights
        # cond1(x1) needs come first, everything else after.
        xin = [[None, None], [None, None]]

        def load_x(i, c):
            t = big.tile([128, 4224], F32R, tag="big", name="xin")
            nc.sync.dma_start(t[:], x_d[i].ap()[c * 128 : (c + 1) * 128, :])
            xin[i][c] = t

        load_x(0, 0)
        load_x(0, 1)
        for name in ("w1T0", "b1_0", "b2r0", "ident"):
            load_w(name)
        load_w2r(0)
        for name in ("w1T1", "b1_1", "b2r1", "alignb", "up2w", "up2b",
                     "re1b", "re2b", "up1b"):
            load_w(name)
        load_w2r(1)
        load_x(1, 0)
        load_x(1, 1)

        # ---------------- dsc stage ----------------
        d_init = [0]  # number of dpool buffers memset so far

        def conditioning(d, gms):
            """gms: two [128,1] per-chunk channel-sum tiles -> per-chunk
            diagonal-tap matrices D [128, 9*128] (f32r) for the PE depthwise."""
            pgt = psc.tile([128, 9], F32, tag="psc", name="pgt")
            pg = pgt[0:64, 0:1]
            for c in range(2):
                nc.tensor.matmul(pg, wt[f"w1T{d}"][:, c * 64 : (c + 1) * 64],
                                 gms[c][:], start=(c == 0), stop=(c == 1))
            u = tiny.tile([64, 1], F32, tag="u")
            nc.scalar.activation(u[:], pg, AF.Identity, bias=wt[f"b1_{d}"][:])
            sq = tiny.tile([64, 1], F32, tag="sq")
            nc.scalar.activation(sq[:], u[:], AF.Square)
            v3 = tiny.tile([64, 1], F32, tag="v3")
            nc.vector.scalar_tensor_tensor(v3[:], sq[:], 0.044715, u[:], AL.mult, AL.mult)
            w_ = tiny.tile([64, 1], F32, tag="w_")
            nc.vector.tensor_tensor(w_[:], v3[:], u[:], AL.add)
            th = tiny.tile([64, 1], F32, tag="th")
            nc.scalar.activation(th[:], w_[:], AF.Tanh, scale=0.7978845608028654)
            hv = tiny.tile([64, 1], BF16, tag="hv")
            nc.vector.scalar_tensor_tensor(hv[:], th[:], 1.0, u[:], AL.add, AL.mult)

            w2r = load_w2r(d)
            D_l, wts_l = [], []
            for c in range(2):
                pw = psc.tile([128, 9], F32, tag="psc", name="pw")
                for k in range(9):
                    nc.tensor.matmul(pw[:, k : k + 1],
                                     w2r[:, (c * 9 + k) * 128 : (c * 9 + k + 1) * 128],
                                     hv[:], start=True, stop=True)
                raw = tiny.tile([128, 9], F32, tag="raw")
                nc.vector.tensor_tensor(raw[:], pw[:],
                                        wt[f"b2r{d}"][:, c * 9 : (c + 1) * 9], AL.add)
                mx = tiny.tile([128, 1], F32, tag="mx")
                nc.vector.tensor_reduce(mx[:], raw[:], axis=mybir.AxisListType.X, op=AL.max)
                ngm = tiny.tile([128, 1], F32, tag="ngm")
                nc.vector.tensor_scalar_mul(ngm[:], mx[:], -1.0)
                ex = tiny.tile([128, 9], F32, tag="ex")
                ssum = tiny.tile([128, 1], F32, tag="ssum")
                nc.scalar.activation(ex[:], raw[:], AF.Exp, bias=ngm[:], accum_out=ssum[:])
                rec = tiny.tile([128, 1], F32, tag="rec")
                nc.vector.reciprocal(rec[:], ssum[:])
                wts = tiny.tile([128, 9], F32, tag="wts")
                nc.vector.tensor_scalar_mul(wts[:], ex[:], rec[:])
                w4p = tiny.tile([128, 1], F32, tag="w4p")
                nc.vector.tensor_scalar_add(w4p[:], wts[:, 4:5], 1.0)
                # diagonal tap matrices (center tap has +1 residual folded in)
                D = dpool.tile([128, 1152], F32R, tag="D")
                if d_init[0] < 2:
                    nc.gpsimd.memset(D[:].bitcast(F32), 0.0)
                    d_init[0] += 1
                for k in range(9):
                    ptr = w4p[:] if k == 4 else wts[:, k : k + 1]
                    nc.vector.tensor_scalar_mul(
                        D[:, k * 128 : (k + 1) * 128], wt["ident"][:], ptr)
                D_l.append(D)
                wts_l.append(wts)
            return D_l, wts_l

        TAPS = (4, 0, 1, 2, 3, 5, 6, 7, 8)  # center first: full psum coverage

        def dw_pe(src, dst, D, relu, gacc, wts=None):
            """dst = depthwise(src) (+x via center tap) on PE; evict via Act.
            With wts given (no-relu apps only), taps 0,1 run on DVE and tap 2
            on Pool as post-eviction MACs into dst, shrinking the PE share."""
            off = () if wts is None else (0, 1)
            for n in range(8):
                p = ps.tile([128, 512], F32, tag="ps")
                first = True
                for k in TAPS:
                    if k in off:
                        continue
                    sy, sx = k // 3 - 1, k % 3 - 1
                    o, rhs = mm_views(src[:], p[:], sy, sx, n)
                    if o is None:
                        continue
                    nc.tensor.matmul(o, D[:, k * 128 : (k + 1) * 128], rhs,
                                     start=first, stop=(k == TAPS[-1]),
                                     skip_group_check=True)
                    first = False
                if relu:
                    nc.scalar.activation(pimg(dst[:])[:, n * 8 : (n + 1) * 8, 1:65],
                                         p[:].rearrange("p (y x) -> p y x", y=8),
                                         AF.Relu, accum_out=gacc[:, n : n + 1])
                else:
                    nc.scalar.activation(dst[:, n * 512 : (n + 1) * 512], p[:],
                                         AF.Identity)
            for k in off:
                # per-partition-scalar MACs are DVE-only on hardware (Pool
                # lacks TensorScalarPtr); src pad columns supply the x-shift
                # zeros
                sy, sx = k // 3 - 1, k % 3 - 1
                sv = pimg(src[:])[:, 1 + sy : 64 + sy, 1 + sx : 65 + sx]
                dv = img(dst[:])[:, 1:64, :]
                nc.vector.scalar_tensor_tensor(dv, sv, wts[:, k : k + 1], dv,
                                               AL.mult, AL.add)

        # per-chunk channel sums of the inputs; x2's are emitted later so they
        # don't delay cond1(x1)'s D builds on the in-order DVE
        gms1 = [[None, None], [None, None]]

        def reduce_gms1(i, use_act=False):
            for c in range(2):
                g = tiny.tile([128, 1], F32, tag="gms")
                if use_act and c == 1:
                    # idle Act engine: in-place copy whose accumulator is the
                    # channel sum; runs concurrently with DVE's c0 reduce
                    nc.scalar.activation(xin[i][c][:], xin[i][c][:], AF.Copy,
                                         accum_out=g[:])
                else:
                    nc.vector.tensor_reduce(g[:], xin[i][c][:].bitcast(F32),
                                            axis=mybir.AxisListType.X, op=AL.add)
                gms1[i][c] = g

        reduce_gms1(0, use_act=True)

        mid = [[None, None], [None, None]]
        gacc = [[None, None], [None, None]]
        gms2 = [[None, None], [None, None]]
        y = [[None, None], [None, None]]

        def alloc_mid(i):
            for c in range(2):
                mid[i][c] = pad_tile(name="midt")
                gacc[i][c] = tiny.tile([128, 8], F32, tag="gacc", name="gacc")

        def reduce_gms2(i):
            for c in range(2):
                g2 = tiny.tile([128, 1], F32, tag="gms")
                nc.vector.tensor_reduce(g2[:], gacc[i][c][:],
                                        axis=mybir.AxisListType.X, op=AL.add)
                gms2[i][c] = g2

        # interleave conditioning (PE-tiny + DVE/Act chain) between the big
        # PE depthwise apps so PE never waits on a conditioning chain.
        D1x1, _ = conditioning(0, gms1[0])
        alloc_mid(0)
        reduce_gms1(1)
        dw_pe(xin[0][0], mid[0][0], D1x1[0], True, gacc[0][0])
        D1x2, _ = conditioning(0, gms1[1])
        alloc_mid(1)
        dw_pe(xin[0][1], mid[0][1], D1x1[1], True, gacc[0][1])
        dw_pe(xin[1][0], mid[1][0], D1x2[0], True, gacc[1][0])
        reduce_gms2(0)
        D2x1, wts2x1 = conditioning(1, gms2[0])
        dw_pe(xin[1][1], mid[1][1], D1x2[1], True, gacc[1][1])
        reduce_gms2(1)
        for c in range(2):
            y[0][c] = big.tile([128, 4096], F32R, tag="big", name="yt")
        dw_pe(mid[0][0], y[0][0], D2x1[0], False, None, wts=wts2x1[0][:])
        D2x2, wts2x2 = conditioning(1, gms2[1])
        dw_pe(mid[0][1], y[0][1], D2x1[1], False, None, wts=wts2x1[1][:])
        for c in range(2):
            y[1][c] = big.tile([128, 4096], F32R, tag="big", name="yt")
        dw_pe(mid[1][0], y[1][0], D2x2[0], False, None, wts=wts2x2[0][:])
        dw_pe(mid[1][1], y[1][1], D2x2[1], False, None)

        # ---------------- align 1x1 (2C -> C) + Winograd x-transform ----------------
        # U0 = odd_{j-1}-odd_j, U1 = even+odd, U2 = even-odd, U3 = even_j-even_{j+1}
        # (signs folded into the position weights), built per eviction chunk so
        # up1 can start as soon as the first rows exist.
        awt = u1pool.tile([128, 1024], F32R, tag="u1w", name="awt")
        nc.sync.dma_start(awt[:], wd["alignw"].ap())
        wt["alignw"] = awt
        U = [[upool.tile([128, 2048], BF16, tag="U", name="U") for _ in range(4)]
             for _ in range(2)]
        for mc in range(2):
            for n in range(8):
                p = ps.tile([128, 512], F32, tag="ps")
                for kc in range(4):
                    rhs = y[kc // 2][kc % 2][:, n * 512 : (n + 1) * 512]
                    nc.tensor.matmul(
                        p[:], wt["alignw"][:, (kc * 2 + mc) * 128 : (kc * 2 + mc + 1) * 128],
                        rhs, start=(kc == 0), stop=(kc == 3))
                aev = stg.tile([128, 512], F32, tag="cmb", name="aev", bufs=2)
                nc.scalar.activation(aev[:], p[:], AF.Identity,
                                     bias=wt["alignb"][:, mc : mc + 1])
                v = aev[:].rearrange("p (y j t) -> p y j t", j=32, t=2)
                ev, od = v[:, :, :, 0:1], v[:, :, :, 1:2]
                Uv = [U[mc][q][:].rearrange("p (y j t) -> p y j t", j=32, t=1)
                      [:, n * 8 : (n + 1) * 8, :, :] for q in range(4)]
                nc.gpsimd.tensor_tensor(Uv[1], ev, od, AL.add)
                nc.vector.tensor_tensor(Uv[2], ev, od, AL.subtract)
                nc.vector.tensor_tensor(Uv[0][:, :, 1:32], od[:, :, 0:31],
                                        od[:, :, 1:32], AL.subtract)
                nc.vector.tensor_scalar_mul(Uv[0][:, :, 0:1], od[:, :, 0:1], -1.0)
                nc.vector.tensor_tensor(Uv[3][:, :, 0:31], ev[:, :, 0:31],
                                        ev[:, :, 1:32], AL.subtract)
                nc.vector.tensor_scalar_mul(Uv[3][:, :, 31:32], ev[:, :, 31:32], 1.0)

        # ---------------- up1 (Winograd positions) + up2 (1x1) ----------------
        def wmm_views(U_t, ps_t, sl, sy, n):
            y0 = max(0, -sy)
            y1 = 64 + min(0, -sy)
            r0 = max(8 * n, y0)
            r1 = min(8 * n + 8, y1)
            if r1 <= r0:
                return None, None
            v = U_t[:].rearrange("p (y j) -> p y j", j=32)
            rhs = v[:, r0 + sy : r1 + sy, :]
            out = ps_t[:, sl + (r0 - 8 * n) * 32 : sl + (r1 - 8 * n) * 32]
            return out, rhs

        up2p = []
        for p4 in range(4):
            up2p.append(pad_tile(name="up2p"))
        for p4 in range(4):
            u1t = []
            for mcin in range(2):
                th = u1pool.tile([128, 3072], BF16, tag="u1w", name="u1t")
                off = (p4 * 2 + mcin) * 3072
                nc.sync.dma_start(th[:], wd["up1w"].ap()[:, off : off + 3072])
                u1t.append(th)
            for n in range(8):
                stage = []
                for mcin in range(2):
                    psA = ps.tile([128, 512], F32, tag="ps")
                    psB = ps.tile([128, 512], F32, tag="ps")
                    for pos in range(4):
                        # psA = (m0 | m3), psB = (m1 | m2): the combine then
                        # needs only one PSUM operand per instruction
                        tgt = psA if pos in (0, 3) else psB
                        sl = 256 * (0 if pos in (0, 1) else 1)
                        first = True
                        for dy in (1, 0, 2):  # sy=0 first: full slice coverage
                            sy = dy - 1
                            for kc in range(2):
                                o, rhs = wmm_views(U[kc][pos], tgt[:], sl, sy, n)
                                if o is None:
                                    continue
                                lhsT = u1t[mcin][:, ((pos * 3 + dy) * 2 + kc) * 128 :
                                                 ((pos * 3 + dy) * 2 + kc + 1) * 128]
                                nc.tensor.matmul(o, lhsT, rhs, start=first,
                                                 stop=(dy == 2 and kc == 1),
                                                 skip_group_check=True)
                                first = False
                    # out0 = m0+(m1+m2)+b, out1 = (m1-m2)+b-m3, x-interleaved.
                    # Act evicts both psum pairs (frees the banks fast; engine
                    # ops may read at most one PSUM operand anyway), then the
                    # combine is SBUF-only on Pool/DVE.
                    sb = stg.tile([128, 512], F32, tag="cmb", name="cmb", bufs=2)
                    nc.scalar.activation(sb[:], psB[:], AF.Identity)
                    sbA = stg.tile([128, 512], F32, tag="ret", name="sbA", bufs=3)
                    nc.scalar.activation(sbA[:], psA[:], AF.Identity)
                    ta = stg.tile([128, 256], F32, tag="t01", name="ta", bufs=2)
                    nc.gpsimd.tensor_tensor(ta[:], sb[:, 0:256], sb[:, 256:512],
                                            AL.add)
                    tb = stg.tile([128, 256], F32, tag="t12", name="tb", bufs=2)
                    nc.vector.tensor_tensor(tb[:], sb[:, 0:256], sb[:, 256:512],
                                            AL.subtract)
                    st = stg.tile([128, 512], F32R, tag="stg")
                    stv = st[:].rearrange("p (a t) -> p a t", t=2)
                    c1 = lambda ap: ap.rearrange("p (a t) -> p a t", t=1)
                    bptr = wt["up1b"][:, p4 * 2 + mcin : p4 * 2 + mcin + 1]
                    nc.vector.scalar_tensor_tensor(stv[:, :, 0:1], c1(ta[:]), bptr,
                                                   c1(sbA[:, 0:256]), AL.add, AL.add)
                    nc.vector.scalar_tensor_tensor(stv[:, :, 1:2], c1(tb[:]), bptr,
                                                   c1(sbA[:, 256:512]), AL.add,
                                                   AL.subtract)
                    stage.append(st)
                p2 = ps.tile([128, 512], F32, tag="ps")
                for kc in range(2):
                    nc.tensor.matmul(p2[:], wt["up2w"][:, kc * 128 : (kc + 1) * 128],
                                     stage[kc][:], start=(kc == 0), stop=(kc == 1))
                nc.scalar.activation(pimg(up2p[p4][:])[:, n * 8 : (n + 1) * 8, 1:65],
                                     p2[:].rearrange("p (y x) -> p y x", y=8),
                                     AF.Identity, bias=wt["up2b"][:])

        # ---------------- re1 (polyphase 3x3, M-packed) ----------------
        re1t = pad_tile(name="re1t")
        for name in ("re1w", "re2w"):
            t = u1pool.tile([128, 2048], F32R, tag="u1w")
            nc.sync.dma_start(t[:], wd[name].ap())
            wt[name] = t
        for n in range(8):
            p = ps.tile([128, 512], F32, tag="ps")
            for ki, (pin, qy, qx) in enumerate(re1_keys):
                o, rhs = mm_views(up2p[pin][:], p[:], qy, qx, n)
                if o is None:
                    continue
                nc.tensor.matmul(o, wt["re1w"][:, ki * 128 : (ki + 1) * 128], rhs,
                                 start=(ki == 0), stop=(ki == len(re1_keys) - 1),
                                 skip_group_check=True)
            nc.scalar.activation(pimg(re1t[:])[:, n * 8 : (n + 1) * 8, 1:65],
                                 p[:].rearrange("p (y x) -> p y x", y=8),
                                 AF.Relu, bias=wt["re1b"][:])

        # ---------------- re2 (polyphase 3x3) + residual + interleave + out ----------------
        for n in range(8):
            pss = []
            for p4 in range(4):
                p = ps.tile([128, 512], F32, tag="ps")
                for qi, (qy, qx) in enumerate(re2_q[p4]):
                    o, rhs = mm_views(re1t[:], p[:], qy, qx, n)
                    if o is None:
                        continue
                    nc.tensor.matmul(o, wt["re2w"][:, (p4 * 4 + qi) * 128 :
                                                   (p4 * 4 + qi + 1) * 128], rhs,
                                     start=(qi == 0), stop=(qi == len(re2_q[p4]) - 1),
                                     skip_group_check=True)
                pss.append(p)
            # p4=3 detours via Act (psum evict + bias) so Pool (no PSUM
            # access) can do its residual add from SBUF; p4 0-2 are DVE
            # STTs straight from psum. Keeps every engine under PE's pace.
            ret = {}
            for p4 in (2, 3):
                t = stg.tile([128, 512], F32, tag="ret", name="ret", bufs=3)
                nc.scalar.activation(t[:], pss[p4][:], AF.Identity,
                                     bias=wt["re2b"][:])
                ret[p4] = t
            for hb in range(2):  # half-bands of 8 output rows (4 phase rows)
                band = bnd.tile([128, 1024], F32, tag="bnd")
                bv = band[:].rearrange("p (y r x s) -> p y r x s", y=4, r=2, s=2)
                for p4 in range(4):
                    r, s = p4 // 2, p4 % 2
                    y0 = n * 8 + hb * 4
                    up_v = pimg(up2p[p4][:].bitcast(F32))[:, y0 : y0 + 4, 1:65]
                    if p4 >= 2:
                        nc.gpsimd.tensor_tensor(
                            bv[:, :, r, :, s],
                            ret[p4][:, hb * 256 : (hb + 1) * 256].rearrange(
                                "p (y x) -> p y x", y=4),
                            up_v, AL.add)
                    else:
                        nc.vector.scalar_tensor_tensor(
                            bv[:, :, r, :, s],
                            pss[p4][:, hb * 256 : (hb + 1) * 256].rearrange(
                                "p (y x) -> p y x", y=4),
                            wt["re2b"][:], up_v, AL.add, AL.add)
                nc.sync.dma_start(
                    out_d.ap()[:, (2 * n + hb) * 1024 : (2 * n + hb + 1) * 1024],
                    band[:])

    nc.compile()
    return nc


_NC = None


def _get_nc():
    global _NC
    if _NC is None:
        _NC = build()
    return _NC


def make_in_maps(inputs):
    w, _, _ = host_prep(inputs)
    def hostpad(x):
        x = np.asarray(x, np.float32).reshape(NC, 256, 64, 64)
        xp = np.zeros((NC, 256, 64, 66), np.float32)
        xp[:, :, :, 1:65] = x
        return np.ascontiguousarray(xp.reshape(NC, 256, 4224))

    x1 = hostpad(inputs["x1"])
    x2 = hostpad(inputs["x2"])
    in_maps = []
    for i in range(NC):
        m = {"x1": x1[i], "x2": x2[i]}
        m.update(w)
        in_maps.append(m)
    return in_maps


def kernel(**inputs):
    nc = _get_nc()
    in_maps = make_in_maps(inputs)
    res = run_bass_kernel_spmd(nc, in_maps, core_ids=list(range(NC)))
    out = np.stack([res.results[i]["out"].reshape(128, 128, 128) for i in range(NC)])
    return out.astype(np.float32)


# revision 40
# speedup vs baseline: 3.4065x; 1.0186x over previous
import sys

sys.path.insert(0, "/opt/trn_rl_repo")

from contextlib import ExitStack

import numpy as np

import concourse.bacc as bacc
import concourse.mybir as mybir
from concourse import tile
from concourse.bass_utils import run_bass_kernel_spmd

F32 = mybir.dt.float32
F32R = mybir.dt.float32r
BF16 = mybir.dt.bfloat16
AL = mybir.AluOpType
AF = mybir.ActivationFunctionType

C = 256
H = W = 64
NC = 8  # cores / batch shards


# ---------------------------------------------------------------- host prep
def host_prep(inp):
    """Rearrange all weights into [partition, free] layouts matching SBUF tiles."""
    d = {}
    f = np.float32

    # conditioning nets (dsc1, dsc2)
    for i, pre in ((0, "dsc1"), (1, "dsc2")):
        w1 = np.asarray(inp[f"{pre}_w1"], f)  # [64, 256]
        b1 = np.asarray(inp[f"{pre}_b1"], f)  # [64]
        w2 = np.asarray(inp[f"{pre}_w2"], f)  # [2304, 64]
        b2 = np.asarray(inp[f"{pre}_b2"], f)  # [2304]
        # lhsT for gm matmul: [k_local, chunk, m]; fold the 1/(H*W) mean here
        d[f"w1T{i}"] = np.ascontiguousarray(
            (w1.T / (H * W)).reshape(2, 128, 64).transpose(1, 0, 2)
        ).reshape(128, 128)
        d[f"b1_{i}"] = b1.reshape(64, 1).copy()
        # lhsT for wts matmul: [j, chunk, k, c_local]; fold gelu's 0.5 here
        d[f"w2r{i}"] = np.ascontiguousarray(
            (0.5 * w2).reshape(2, 128, 9, 64).transpose(3, 0, 2, 1)
        ).reshape(64, 2304).astype(mybir.dt.np(mybir.dt.bfloat16))
        d[f"b2r{i}"] = np.ascontiguousarray(
            b2.reshape(2, 128, 9).transpose(1, 0, 2)
        ).reshape(128, 18)

    # identity for building runtime diagonal depthwise weight matrices
    d["ident"] = np.ascontiguousarray(np.eye(128, dtype=f))

    # channel_align 1x1: [k_local, kc, mc, m]
    aw = np.asarray(inp["align_w"], f)[:, :, 0, 0]  # [256, 512]
    d["alignw"] = np.ascontiguousarray(
        aw.reshape(2, 128, 4, 128).transpose(3, 2, 0, 1)
    ).reshape(128, 1024)
    d["alignb"] = np.ascontiguousarray(
        np.asarray(inp["align_b"], f).reshape(2, 128).T
    )  # [128, 2]

    # up conv1 3x3 C->4C, pixel-shuffle phase reorder + 1D-Winograd F(2,3)
    # along x: 4 position weights per (dy, kc) replace the 3 dx taps.
    # new channel (p, g) -> orig channel 4g + p   (p = 2r+s)
    uw = np.asarray(inp["up_w1"], f)  # [1024, 256, 3, 3]
    w6 = uw.reshape(256, 4, 2, 128, 3, 3)
    w6 = w6.reshape(2, 128, 4, 2, 128, 3, 3)  # [mcin, m, p, kc, k, dy, dx]
    g0, g1, g2 = w6[..., 0], w6[..., 1], w6[..., 2]
    q = np.stack([g0, 0.5 * (g0 + g1 + g2), -0.5 * (g0 - g1 + g2), g2])
    # q: [pos, mcin, m, p, kc, k, dy] -> lhsT [k, (p, mcin, pos, dy, kc, m)]
    d["up1w"] = np.ascontiguousarray(q.transpose(5, 3, 1, 0, 6, 4, 2)).reshape(
        128, 4 * 2 * 4 * 3 * 2 * 128
    ).astype(mybir.dt.np(mybir.dt.bfloat16))
    ub = np.asarray(inp["up_b1"], f)
    d["up1b"] = np.ascontiguousarray(
        ub.reshape(2, 128, 4).transpose(1, 2, 0)
    ).reshape(128, 8)  # [m, (p, mcin)]

    # up conv2 1x1 C->C/2 (per-phase): [k_local, kc, m]
    u2 = np.asarray(inp["up_w2"], f)[:, :, 0, 0]  # [128, 256]
    d["up2w"] = np.ascontiguousarray(
        u2.reshape(128, 2, 128).transpose(2, 1, 0)
    ).reshape(128, 256)
    d["up2b"] = np.asarray(inp["up_b2"], f).reshape(128, 1).copy()

    # ---- polyphase re_enhance ----
    # phase/tap -> (in-phase, sub-shift) mapping
    def split(v):  # v = r + dy - 1
        rp = v % 2
        return rp, (v - rp) // 2

    r1w = np.asarray(inp["re_w1"], f)  # [32, 128, 3, 3]
    keymap = {}
    for p in range(4):
        r, s = p // 2, p % 2
        for dy in range(3):
            for dx in range(3):
                rp, qy = split(r + dy - 1)
                sp, qx = split(s + dx - 1)
                keymap.setdefault((2 * rp + sp, qy, qx), []).append((p, dy, dx))
    keys = sorted(keymap.keys(), key=lambda k: (k[1] != 0 or k[2] != 0, k))
    re1_keys = keys  # list of (p_in, qy, qx); all-(0,0) shifts first
    re1w = np.zeros((128, 16, 128), f)
    for ki, key in enumerate(keys):
        for (p, dy, dx) in keymap[key]:
            re1w[:, ki, p * 32 : (p + 1) * 32] = r1w[:, :, dy, dx].T
    d["re1w"] = re1w.reshape(128, 2048)
    d["re1b"] = np.tile(np.asarray(inp["re_b1"], f), 4).reshape(128, 1)

    r2w = np.asarray(inp["re_w2"], f)  # [128, 32, 3, 3]
    re2_q = []  # per out-phase list of shifts, (0,0) first
    re2w = np.zeros((128, 4, 4, 128), f)
    for p in range(4):
        r, s = p // 2, p % 2
        qys = sorted({split(r + dy - 1)[1] for dy in range(3)}, key=lambda q: q != 0)
        qxs = sorted({split(s + dx - 1)[1] for dx in range(3)}, key=lambda q: q != 0)
        qs = [(qy, qx) for qy in qys for qx in qxs]
        qs.sort(key=lambda q: q != (0, 0))
        re2_q.append(qs)
        for qi, (qy, qx) in enumerate(qs):
            for pp in range(4):
                rp, sp = pp // 2, pp % 2
                dy = 2 * qy + rp - r + 1
                dx = 2 * qx + sp - s + 1
                if 0 <= dy < 3 and 0 <= dx < 3:
                    re2w[pp * 32 : (pp + 1) * 32, p, qi, :] = r2w[:, :, dy, dx].T
    d["re2w"] = re2w.reshape(128, 2048)
    d["re2b"] = np.asarray(inp["re_b2"], f).reshape(128, 1).copy()

    return d, re1_keys, re2_q


RE1_KEYS = None
RE2_Q = None


def _mapping():
    global RE1_KEYS, RE2_Q
    if RE1_KEYS is None:
        zeros = {k: np.zeros(v) for k, v in [
            ("dsc1_w1", (64, 256)), ("dsc1_b1", (64,)), ("dsc1_w2", (2304, 64)),
            ("dsc1_b2", (2304,)), ("dsc2_w1", (64, 256)), ("dsc2_b1", (64,)),
            ("dsc2_w2", (2304, 64)), ("dsc2_b2", (2304,)),
            ("align_w", (256, 512, 1, 1)), ("align_b", (256,)),
            ("up_w1", (1024, 256, 3, 3)), ("up_b1", (1024,)),
            ("up_w2", (128, 256, 1, 1)), ("up_b2", (128,)),
            ("re_w1", (32, 128, 3, 3)), ("re_b1", (32,)),
            ("re_w2", (128, 32, 3, 3)), ("re_b2", (128,)),
        ]}
        _, RE1_KEYS, RE2_Q = host_prep(zeros)
    return RE1_KEYS, RE2_Q


# ---------------------------------------------------------------- bass build
def img(ap):
    """[128, 64, 64] view of a flat [128, 4096] image."""
    return ap.rearrange("p (y x) -> p y x", y=64)


def pimg(ap):
    """View of a column-padded [128, 64*66] image (zero cols at x=0 and x=65)."""
    return ap.rearrange("p (y x) -> p y x", x=66)


def mm_views(src_ap, psum_ap, sy, sx, n):
    """(psum_out, rhs) for 8-row chunk n of a shifted conv tap on a col-padded
    source. Rows clipped by sy; columns handled by the zero pad. psum out is a
    contiguous 2D region (an f32r-matmul ISA requirement)."""
    y0 = max(0, -sy)
    y1 = 64 + min(0, -sy)
    r0 = max(8 * n, y0)
    r1 = min(8 * n + 8, y1)
    if r1 <= r0:
        return None, None
    v = pimg(src_ap)
    rhs = v[:, r0 + sy : r1 + sy, 1 + sx : 65 + sx]
    out = psum_ap[:, (r0 - 8 * n) * 64 : (r1 - 8 * n) * 64]
    return out, rhs


def build():
    re1_keys, re2_q = _mapping()
    nc = bacc.Bacc(trn_type="TRN2", target_bir_lowering=False, debug=False)

    x_d = [nc.dram_tensor(n, [256, 4224], F32R, kind="ExternalInput") for n in ("x1", "x2")]
    wd = {}
    for name, shape, dt in [
        ("w1T0", [128, 128], F32), ("w1T1", [128, 128], F32),
        ("b1_0", [64, 1], F32), ("b1_1", [64, 1], F32),
        ("w2r0", [64, 2304], BF16), ("w2r1", [64, 2304], BF16),
        ("b2r0", [128, 18], F32), ("b2r1", [128, 18], F32),
        ("ident", [128, 128], F32R),
        ("alignw", [128, 1024], F32R), ("alignb", [128, 2], F32),
        ("up1w", [128, 24576], BF16), ("up1b", [128, 8], F32),
        ("up2w", [128, 256], F32R), ("up2b", [128, 1], F32),
        ("re1w", [128, 2048], F32R), ("re1b", [128, 1], F32),
        ("re2w", [128, 2048], F32R), ("re2b", [128, 1], F32),
    ]:
        wd[name] = nc.dram_tensor(name, shape, dt, kind="ExternalInput")
    out_d = nc.dram_tensor("out", [128, 16384], F32, kind="ExternalOutput")

    with tile.TileContext(nc) as tc, ExitStack() as ctx:
        wpool = ctx.enter_context(tc.tile_pool(name="w", bufs=1))
        u1pool = ctx.enter_context(tc.tile_pool(name="u1w", bufs=4))
        big = ctx.enter_context(tc.tile_pool(name="big", bufs=6))
        upool = ctx.enter_context(tc.tile_pool(name="upos", bufs=8))
        dpool = ctx.enter_context(tc.tile_pool(name="dg", bufs=2))
        stg = ctx.enter_context(tc.tile_pool(name="stg", bufs=2))
        bnd = ctx.enter_context(tc.tile_pool(name="bnd", bufs=3))
        tiny = ctx.enter_context(tc.tile_pool(name="tiny", bufs=4))
        ps = ctx.enter_context(tc.tile_pool(name="ps", bufs=7, space="PSUM"))
        psc = ctx.enter_context(tc.tile_pool(name="psc", bufs=1, space="PSUM"))

        wt = {}

        def load_w(name):
            t = wpool.tile(list(wd[name].shape), wd[name].dtype, tag=name, name=name)
            nc.sync.dma_start(t[:], wd[name].ap())
            wt[name] = t

        def img_tile(name="it"):
            return big.tile([128, 4096], F32R, tag="big", name=name)

        def pad_tile(name="pt"):
            """[128, 64*66] tile; interior written by evictions, pad cols
            zeroed here (x DMAs bring zeros from the host instead)."""
            t = big.tile([128, 4224], F32R, tag="big", name=name)
            v = pimg(t[:].bitcast(F32))
            nc.gpsimd.memset(v[:, :, 0:1], 0.0)
            nc.gpsimd.memset(v[:, :, 65:66], 0.0)
            return t

        w2rt = {}

        def load_w2r(d):
            if d not in w2rt:
                t = u1pool.tile([64, 2304], BF16, tag="u1w", name="w2rt")
                nc.sync.dma_start(t[:], wd[f"w2r{d}"].ap())
                w2rt[d] = t
            return w2rt[d]

        # DMA order is the startup critical path: x1 chunks and the weights
        # cond1(x1) needs come first, everything else after.
        xin = [[None, None], [None, None]]

        def load_x(i, c):
            t = big.tile([128, 4224], F32R, tag="big", name="xin")
            nc.sync.dma_start(t[:], x_d[i].ap()[c * 128 : (c + 1) * 128, :])
            xin[i][c] = t

        load_x(0, 0)
        load_x(0, 1)
        for name in ("w1T0", "b1_0", "b2r0", "ident"):
            load_w(name)
        load_w2r(0)
        for name in ("w1T1", "b1_1", "b2r1", "alignb", "up2w", "up2b",
                     "re1b", "re2b", "up1b"):
            load_w(name)
        load_w2r(1)
        load_x(1, 0)
        load_x(1, 1)

        # ---------------- dsc stage ----------------
        d_init = [0]  # number of dpool buffers memset so far

        def conditioning(d, gms):
            """gms: two [128,1] per-chunk channel-sum tiles -> per-chunk
            diagonal-tap matrices D [128, 9*128] (f32r) for the PE depthwise."""
            pgt = psc.tile([128, 9], F32, tag="psc", name="pgt")
            pg = pgt[0:64, 0:1]
            for c in range(2):
                nc.tensor.matmul(pg, wt[f"w1T{d}"][:, c * 64 : (c + 1) * 64],
                                 gms[c][:], start=(c == 0), stop=(c == 1))
            u = tiny.tile([64, 1], F32, tag="u")
            nc.scalar.activation(u[:], pg, AF.Identity, bias=wt[f"b1_{d}"][:])
            sq = tiny.tile([64, 1], F32, tag="sq")
            nc.scalar.activation(sq[:], u[:], AF.Square)
            v3 = tiny.tile([64, 1], F32, tag="v3")
            nc.vector.scalar_tensor_tensor(v3[:], sq[:], 0.044715, u[:], AL.mult, AL.mult)
            w_ = tiny.tile([64, 1], F32, tag="w_")
            nc.vector.tensor_tensor(w_[:], v3[:], u[:], AL.add)
            th = tiny.tile([64, 1], F32, tag="th")
            nc.scalar.activation(th[:], w_[:], AF.Tanh, scale=0.7978845608028654)
            hv = tiny.tile([64, 1], BF16, tag="hv")
            nc.vector.scalar_tensor_tensor(hv[:], th[:], 1.0, u[:], AL.add, AL.mult)

            w2r = load_w2r(d)
            D_l, wts_l = [], []
            for c in range(2):
                pw = psc.tile([128, 9], F32, tag="psc", name="pw")
                for k in range(9):
                    nc.tensor.matmul(pw[:, k : k + 1],
                                     w2r[:, (c * 9 + k) * 128 : (c * 9 + k + 1) * 128],
                                     hv[:], start=True, stop=True)
                raw = tiny.tile([128, 9], F32, tag="raw")
                nc.vector.tensor_tensor(raw[:], pw[:],
                                        wt[f"b2r{d}"][:, c * 9 : (c + 1) * 9], AL.add)
                mx = tiny.tile([128, 1], F32, tag="mx")
                nc.vector.tensor_reduce(mx[:], raw[:], axis=mybir.AxisListType.X, op=AL.max)
                ngm = tiny.tile([128, 1], F32, tag="ngm")
                nc.vector.tensor_scalar_mul(ngm[:], mx[:], -1.0)
                ex = tiny.tile([128, 9], F32, tag="ex")
                ssum = tiny.tile([128, 1], F32, tag="ssum")
                nc.scalar.activation(ex[:], raw[:], AF.Exp, bias=ngm[:], accum_out=ssum[:])
                rec = tiny.tile([128, 1], F32, tag="rec")
                nc.vector.reciprocal(rec[:], ssum[:])
                wts = tiny.tile([128, 9], F32, tag="wts")
                nc.vector.tensor_scalar_mul(wts[:], ex[:], rec[:])
                w4p = tiny.tile([128, 1], F32, tag="w4p")
                nc.vector.tensor_scalar_add(w4p[:], wts[:, 4:5], 1.0)
                # diagonal tap matrices (center tap has +1 residual folded in)
                D = dpool.tile([128, 1152], F32R, tag="D")
                if d_init[0] < 2:
                    nc.gpsimd.memset(D[:].bitcast(F32), 0.0)
                    d_init[0] += 1
                for k in range(9):
                    ptr = w4p[:] if k == 4 else wts[:, k : k + 1]
                    nc.vector.tensor_scalar_mul(
                        D[:, k * 128 : (k + 1) * 128], wt["ident"][:], ptr)
                D_l.append(D)
                wts_l.append(wts)
            return D_l, wts_l

        TAPS = (4, 0, 1, 2, 3, 5, 6, 7, 8)  # center first: full psum coverage

        def dw_pe(src, dst, D, relu, gacc, wts=None):
            """dst = depthwise(src) (+x via center tap) on PE; evict via Act.
            With wts given (no-relu apps only), taps 0,1 run on DVE and tap 2
            on Pool as post-eviction MACs into dst, shrinking the PE share."""
            off = () if wts is None else (0, 1)
            for n in range(8):
                p = ps.tile([128, 512], F32, tag="ps")
                first = True
                for k in TAPS:
                    if k in off:
                        continue
                    sy, sx = k // 3 - 1, k % 3 - 1
                    o, rhs = mm_views(src[:], p[:], sy, sx, n)
                    if o is None:
                        continue
                    nc.tensor.matmul(o, D[:, k * 128 : (k + 1) * 128], rhs,
                                     start=first, stop=(k == TAPS[-1]),
                                     skip_group_check=True)
                    first = False
                if relu:
                    nc.scalar.activation(pimg(dst[:])[:, n * 8 : (n + 1) * 8, 1:65],
                                         p[:].rearrange("p (y x) -> p y x", y=8),
                                         AF.Relu, accum_out=gacc[:, n : n + 1])
                else:
                    nc.scalar.activation(dst[:, n * 512 : (n + 1) * 512], p[:],
                                         AF.Identity)
            for k in off:
                # per-partition-scalar MACs are DVE-only on hardware (Pool
                # lacks TensorScalarPtr); src pad columns supply the x-shift
                # zeros
                sy, sx = k // 3 - 1, k % 3 - 1
                sv = pimg(src[:])[:, 1 + sy : 64 + sy, 1 + sx : 65 + sx]
                dv = img(dst[:])[:, 1:64, :]
                nc.vector.scalar_tensor_tensor(dv, sv, wts[:, k : k + 1], dv,
                                               AL.mult, AL.add)

        # per-chunk channel sums of the inputs; x2's are emitted later so they
        # don't delay cond1(x1)'s D builds on the in-order DVE
        gms1 = [[None, None], [None, None]]

        def reduce_gms1(i, use_act=False):
            for c in range(2):
                g = tiny.tile([128, 1], F32, tag="gms")
                if use_act and c == 1:
                    # idle Act engine: in-place copy whose accumulator is the
                    # channel sum; runs concurrently with DVE's c0 reduce
                    nc.scalar.activation(xin[i][c][:], xin[i][c][:], AF.Copy,
                                         accum_out=g[:])
                else:
                    nc.vector.tensor_reduce(g[:], xin[i][c][:].bitcast(F32),
                                            axis=mybir.AxisListType.X, op=AL.add)
                gms1[i][c] = g

        reduce_gms1(0, use_act=True)

        mid = [[None, None], [None, None]]
        gacc = [[None, None], [None, None]]
        gms2 = [[None, None], [None, None]]
        y = [[None, None], [None, None]]

        def alloc_mid(i):
            for c in range(2):
                mid[i][c] = pad_tile(name="midt")
                gacc[i][c] = tiny.tile([128, 8], F32, tag="gacc", name="gacc")

        def reduce_gms2(i):
            for c in range(2):
                g2 = tiny.tile([128, 1], F32, tag="gms")
                nc.vector.tensor_reduce(g2[:], gacc[i][c][:],
                                        axis=mybir.AxisListType.X, op=AL.add)
                gms2[i][c] = g2

        # interleave conditioning (PE-tiny + DVE/Act chain) between the big
        # PE depthwise apps so PE never waits on a conditioning chain.
        D1x1, _ = conditioning(0, gms1[0])
        alloc_mid(0)
        reduce_gms1(1)
        dw_pe(xin[0][0], mid[0][0], D1x1[0], True, gacc[0][0])
        D1x2, _ = conditioning(0, gms1[1])
        alloc_mid(1)
        dw_pe(xin[0][1], mid[0][1], D1x1[1], True, gacc[0][1])
        dw_pe(xin[1][0], mid[1][0], D1x2[0], True, gacc[1][0])
        reduce_gms2(0)
        D2x1, wts2x1 = conditioning(1, gms2[0])
        dw_pe(xin[1][1], mid[1][1], D1x2[1], True, gacc[1][1])
        reduce_gms2(1)
        for c in range(2):
            y[0][c] = big.tile([128, 4096], F32R, tag="big", name="yt")
        dw_pe(mid[0][0], y[0][0], D2x1[0], False, None, wts=wts2x1[0][:])
        D2x2, wts2x2 = conditioning(1, gms2[1])
        dw_pe(mid[0][1], y[0][1], D2x1[1], False, None, wts=wts2x1[1][:])
        for c in range(2):
            y[1][c] = big.tile([128, 4096], F32R, tag="big", name="yt")
        dw_pe(mid[1][0], y[1][0], D2x2[0], False, None, wts=wts2x2[0][:])
        dw_pe(mid[1][1], y[1][1], D2x2[1], False, None)

        # ---------------- align 1x1 (2C -> C) + Winograd x-transform ----------------
        # U0 = odd_{j-1}-odd_j, U1 = even+odd, U2 = even-odd, U3 = even_j-even_{j+1}
        # (signs folded into the position weights), built per eviction chunk so
        # up1 can start as soon as the first rows exist.
        awt = u1pool.tile([128, 1024], F32R, tag="u1w", name="awt")
        nc.sync.dma_start(awt[:], wd["alignw"].ap())
        wt["alignw"] = awt
        U = [[upool.tile([128, 2048], BF16, tag="U", name="U") for _ in range(4)]
             for _ in range(2)]
        for mc in range(2):
            for n in range(8):
                p = ps.tile([128, 512], F32, tag="ps")
                for kc in range(4):
                    rhs = y[kc // 2][kc % 2][:, n * 512 : (n + 1) * 512]
                    nc.tensor.matmul(
                        p[:], wt["alignw"][:, (kc * 2 + mc) * 128 : (kc * 2 + mc + 1) * 128],
                        rhs, start=(kc == 0), stop=(kc == 3))
                aev = stg.tile([128, 512], F32, tag="cmb", name="aev", bufs=2)
                nc.scalar.activation(aev[:], p[:], AF.Identity,
                                     bias=wt["alignb"][:, mc : mc + 1])
                v = aev[:].rearrange("p (y j t) -> p y j t", j=32, t=2)
                ev, od = v[:, :, :, 0:1], v[:, :, :, 1:2]
                Uv = [U[mc][q][:].rearrange("p (y j t) -> p y j t", j=32, t=1)
                      [:, n * 8 : (n + 1) * 8, :, :] for q in range(4)]
                nc.gpsimd.tensor_tensor(Uv[1], ev, od, AL.add)
                nc.vector.tensor_tensor(Uv[2], ev, od, AL.subtract)
                nc.vector.tensor_tensor(Uv[0][:, :, 1:32], od[:, :, 0:31],
                                        od[:, :, 1:32], AL.subtract)
                nc.vector.tensor_scalar_mul(Uv[0][:, :, 0:1], od[:, :, 0:1], -1.0)
                nc.vector.tensor_tensor(Uv[3][:, :, 0:31], ev[:, :, 0:31],
                                        ev[:, :, 1:32], AL.subtract)
                nc.vector.tensor_scalar_mul(Uv[3][:, :, 31:32], ev[:, :, 31:32], 1.0)

        # ---------------- up1 (Winograd positions) + up2 (1x1) ----------------
        def wmm_views(U_t, ps_t, sl, sy, n):
            y0 = max(0, -sy)
            y1 = 64 + min(0, -sy)
            r0 = max(8 * n, y0)
            r1 = min(8 * n + 8, y1)
            if r1 <= r0:
                return None, None
            v = U_t[:].rearrange("p (y j) -> p y j", j=32)
            rhs = v[:, r0 + sy : r1 + sy, :]
            out = ps_t[:, sl + (r0 - 8 * n) * 32 : sl + (r1 - 8 * n) * 32]
            return out, rhs

        up2p = []
        for p4 in range(4):
            up2p.append(pad_tile(name="up2p"))

        def emit_up2(p4, n, stage):
            """up2 1x1 for chunk n; emitted one chunk late so PE never waits
            on the stage-combine chain."""
            p2 = ps.tile([128, 512], F32, tag="ps", name="p2")
            for kc in range(2):
                nc.tensor.matmul(p2[:], wt["up2w"][:, kc * 128 : (kc + 1) * 128],
                                 stage[kc][:], start=(kc == 0), stop=(kc == 1))
            nc.scalar.activation(pimg(up2p[p4][:])[:, n * 8 : (n + 1) * 8, 1:65],
                                 p2[:].rearrange("p (y x) -> p y x", y=8),
                                 AF.Identity, bias=wt["up2b"][:])

        pend = []
        for p4 in range(4):
            u1t = []
            for mcin in range(2):
                th = u1pool.tile([128, 3072], BF16, tag="u1w", name="u1t")
                off = (p4 * 2 + mcin) * 3072
                nc.sync.dma_start(th[:], wd["up1w"].ap()[:, off : off + 3072])
                u1t.append(th)
            for n in range(8):
                stage = []
                for mcin in range(2):
                    psA = ps.tile([128, 512], F32, tag="ps")
                    psB = ps.tile([128, 512], F32, tag="ps")
                    for pos in range(4):
                        # psA = (m0 | m3), psB = (m1 | m2): the combine then
                        # needs only one PSUM operand per instruction
                        tgt = psA if pos in (0, 3) else psB
                        sl = 256 * (0 if pos in (0, 1) else 1)
                        first = True
                        for dy in (1, 0, 2):  # sy=0 first: full slice coverage
                            sy = dy - 1
                            for kc in range(2):
                                o, rhs = wmm_views(U[kc][pos], tgt[:], sl, sy, n)
                                if o is None:
                                    continue
                                lhsT = u1t[mcin][:, ((pos * 3 + dy) * 2 + kc) * 128 :
                                                 ((pos * 3 + dy) * 2 + kc + 1) * 128]
                                nc.tensor.matmul(o, lhsT, rhs, start=first,
                                                 stop=(dy == 2 and kc == 1),
                                                 skip_group_check=True)
                                first = False
                    # out0 = m0+(m1+m2)+b, out1 = (m1-m2)+b-m3, x-interleaved.
                    # Act evicts both psum pairs (frees the banks fast; engine
                    # ops may read at most one PSUM operand anyway), then the
                    # combine is SBUF-only on Pool/DVE.
                    sb = stg.tile([128, 512], F32, tag="cmb", name="cmb", bufs=2)
                    nc.scalar.activation(sb[:], psB[:], AF.Identity)
                    sbA = stg.tile([128, 512], F32, tag="ret", name="sbA", bufs=3)
                    nc.scalar.activation(sbA[:], psA[:], AF.Identity)
                    ta = stg.tile([128, 256], F32, tag="t01", name="ta", bufs=2)
                    nc.gpsimd.tensor_tensor(ta[:], sb[:, 0:256], sb[:, 256:512],
                                            AL.add)
                    tb = stg.tile([128, 256], F32, tag="t12", name="tb", bufs=2)
                    nc.vector.tensor_tensor(tb[:], sb[:, 0:256], sb[:, 256:512],
                                            AL.subtract)
                    st = stg.tile([128, 512], F32R, tag="stg")
                    stv = st[:].rearrange("p (a t) -> p a t", t=2)
                    c1 = lambda ap: ap.rearrange("p (a t) -> p a t", t=1)
                    bptr = wt["up1b"][:, p4 * 2 + mcin : p4 * 2 + mcin + 1]
                    nc.vector.scalar_tensor_tensor(stv[:, :, 0:1], c1(ta[:]), bptr,
                                                   c1(sbA[:, 0:256]), AL.add, AL.add)
                    nc.vector.scalar_tensor_tensor(stv[:, :, 1:2], c1(tb[:]), bptr,
                                                   c1(sbA[:, 256:512]), AL.add,
                                                   AL.subtract)
                    stage.append(st)
                pend.append((n, stage))
                if len(pend) > 1:
                    emit_up2(p4, *pend.pop(0))
            while pend:
                emit_up2(p4, *pend.pop(0))

        # ---------------- re1 (polyphase 3x3, M-packed) ----------------
        re1t = pad_tile(name="re1t")
        for name in ("re1w", "re2w"):
            t = u1pool.tile([128, 2048], F32R, tag="u1w")
            nc.sync.dma_start(t[:], wd[name].ap())
            wt[name] = t
        for n in range(8):
            p = ps.tile([128, 512], F32, tag="ps")
            for ki, (pin, qy, qx) in enumerate(re1_keys):
                o, rhs = mm_views(up2p[pin][:], p[:], qy, qx, n)
                if o is None:
                    continue
                nc.tensor.matmul(o, wt["re1w"][:, ki * 128 : (ki + 1) * 128], rhs,
                                 start=(ki == 0), stop=(ki == len(re1_keys) - 1),
                                 skip_group_check=True)
            nc.scalar.activation(pimg(re1t[:])[:, n * 8 : (n + 1) * 8, 1:65],
                                 p[:].rearrange("p (y x) -> p y x", y=8),
                                 AF.Relu, bias=wt["re1b"][:])

        # ---------------- re2 (polyphase 3x3) + residual + interleave + out ----------------
        for n in range(8):
            pss = []
            for p4 in range(4):
                p = ps.tile([128, 512], F32, tag="ps")
                for qi, (qy, qx) in enumerate(re2_q[p4]):
                    o, rhs = mm_views(re1t[:], p[:], qy, qx, n)
                    if o is None:
                        continue
                    nc.tensor.matmul(o, wt["re2w"][:, (p4 * 4 + qi) * 128 :
                                                   (p4 * 4 + qi + 1) * 128], rhs,
                                     start=(qi == 0), stop=(qi == len(re2_q[p4]) - 1),
                                     skip_group_check=True)
                pss.append(p)
            # p4=3 detours via Act (psum evict + bias) so Pool (no PSUM
            # access) can do its residual add from SBUF; p4 0-2 are DVE
            # STTs straight from psum. Keeps every engine under PE's pace.
            ret = {}
            for p4 in (2, 3):
                t = stg.tile([128, 512], F32, tag="ret", name="ret", bufs=3)
                nc.scalar.activation(t[:], pss[p4][:], AF.Identity,
                                     bias=wt["re2b"][:])
                ret[p4] = t
            for hb in range(2):  # half-bands of 8 output rows (4 phase rows)
                band = bnd.tile([128, 1024], F32, tag="bnd")
                bv = band[:].rearrange("p (y r x s) -> p y r x s", y=4, r=2, s=2)
                for p4 in range(4):
                    r, s = p4 // 2, p4 % 2
                    y0 = n * 8 + hb * 4
                    up_v = pimg(up2p[p4][:].bitcast(F32))[:, y0 : y0 + 4, 1:65]
                    if p4 >= 2:
                        nc.gpsimd.tensor_tensor(
                            bv[:, :, r, :, s],
                            ret[p4][:, hb * 256 : (hb + 1) * 256].rearrange(
                                "p (y x) -> p y x", y=4),
                            up_v, AL.add)
                    else:
                        nc.vector.scalar_tensor_tensor(
                            bv[:, :, r, :, s],
                            pss[p4][:, hb * 256 : (hb + 1) * 256].rearrange(
                                "p (y x) -> p y x", y=4),
                            wt["re2b"][:], up_v, AL.add, AL.add)
                nc.sync.dma_start(
                    out_d.ap()[:, (2 * n + hb) * 1024 : (2 * n + hb + 1) * 1024],
                    band[:])

    nc.compile()
    return nc


_NC = None


def _get_nc():
    global _NC
    if _NC is None:
        _NC = build()
    return _NC


def make_in_maps(inputs):
    w, _, _ = host_prep(inputs)
    def hostpad(x):
        x = np.asarray(x, np.float32).reshape(NC, 256, 64, 64)
        xp = np.zeros((NC, 256, 64, 66), np.float32)
        xp[:, :, :, 1:65] = x
        return np.ascontiguousarray(xp.reshape(NC, 256, 4224))

    x1 = hostpad(inputs["x1"])
    x2 = hostpad(inputs["x2"])
    in_maps = []
    for i in range(NC):
        m = {"x1": x1[i], "x2": x2[i]}
        m.update(w)
        in_maps.append(m)
    return in_maps


def kernel(**inputs):
    nc = _get_nc()
    in_maps = make_in_maps(inputs)
    res = run_bass_kernel_spmd(nc, in_maps, core_ids=list(range(NC)))
    out = np.stack([res.results[i]["out"].reshape(128, 128, 128) for i in range(NC)])
    return out.astype(np.float32)


# revision 42
# speedup vs baseline: 3.4131x; 1.0020x over previous
import sys

sys.path.insert(0, "/opt/trn_rl_repo")

from contextlib import ExitStack

import numpy as np

import concourse.bacc as bacc
import concourse.mybir as mybir
from concourse import tile
from concourse.bass_utils import run_bass_kernel_spmd

F32 = mybir.dt.float32
F32R = mybir.dt.float32r
BF16 = mybir.dt.bfloat16
AL = mybir.AluOpType
AF = mybir.ActivationFunctionType

C = 256
H = W = 64
NC = 8  # cores / batch shards


# ---------------------------------------------------------------- host prep
def host_prep(inp):
    """Rearrange all weights into [partition, free] layouts matching SBUF tiles."""
    d = {}
    f = np.float32

    # conditioning nets (dsc1, dsc2)
    for i, pre in ((0, "dsc1"), (1, "dsc2")):
        w1 = np.asarray(inp[f"{pre}_w1"], f)  # [64, 256]
        b1 = np.asarray(inp[f"{pre}_b1"], f)  # [64]
        w2 = np.asarray(inp[f"{pre}_w2"], f)  # [2304, 64]
        b2 = np.asarray(inp[f"{pre}_b2"], f)  # [2304]
        # lhsT for gm matmul: [k_local, chunk, m]; fold the 1/(H*W) mean here
        d[f"w1T{i}"] = np.ascontiguousarray(
            (w1.T / (H * W)).reshape(2, 128, 64).transpose(1, 0, 2)
        ).reshape(128, 128)
        d[f"b1_{i}"] = b1.reshape(64, 1).copy()
        # lhsT for wts matmul: [j, chunk, k, c_local]; fold gelu's 0.5 here
        d[f"w2r{i}"] = np.ascontiguousarray(
            (0.5 * w2).reshape(2, 128, 9, 64).transpose(3, 0, 2, 1)
        ).reshape(64, 2304).astype(mybir.dt.np(mybir.dt.bfloat16))
        d[f"b2r{i}"] = np.ascontiguousarray(
            b2.reshape(2, 128, 9).transpose(1, 0, 2)
        ).reshape(128, 18)

    # identity for building runtime diagonal depthwise weight matrices
    d["ident"] = np.ascontiguousarray(np.eye(128, dtype=f))

    # channel_align 1x1: [k_local, kc, mc, m]
    aw = np.asarray(inp["align_w"], f)[:, :, 0, 0]  # [256, 512]
    d["alignw"] = np.ascontiguousarray(
        aw.reshape(2, 128, 4, 128).transpose(3, 2, 0, 1)
    ).reshape(128, 1024)
    d["alignb"] = np.ascontiguousarray(
        np.asarray(inp["align_b"], f).reshape(2, 128).T
    )  # [128, 2]

    # up conv1 3x3 C->4C, pixel-shuffle phase reorder + 1D-Winograd F(2,3)
    # along x: 4 position weights per (dy, kc) replace the 3 dx taps.
    # new channel (p, g) -> orig channel 4g + p   (p = 2r+s)
    uw = np.asarray(inp["up_w1"], f)  # [1024, 256, 3, 3]
    w6 = uw.reshape(256, 4, 2, 128, 3, 3)
    w6 = w6.reshape(2, 128, 4, 2, 128, 3, 3)  # [mcin, m, p, kc, k, dy, dx]
    g0, g1, g2 = w6[..., 0], w6[..., 1], w6[..., 2]
    q = np.stack([g0, 0.5 * (g0 + g1 + g2), -0.5 * (g0 - g1 + g2), g2])
    # q: [pos, mcin, m, p, kc, k, dy] -> lhsT [k, (p, mcin, pos, dy, kc, m)]
    d["up1w"] = np.ascontiguousarray(q.transpose(5, 3, 1, 0, 6, 4, 2)).reshape(
        128, 4 * 2 * 4 * 3 * 2 * 128
    ).astype(mybir.dt.np(mybir.dt.bfloat16))
    ub = np.asarray(inp["up_b1"], f)
    d["up1b"] = np.ascontiguousarray(
        ub.reshape(2, 128, 4).transpose(1, 2, 0)
    ).reshape(128, 8)  # [m, (p, mcin)]

    # up conv2 1x1 C->C/2 (per-phase): [k_local, kc, m]
    u2 = np.asarray(inp["up_w2"], f)[:, :, 0, 0]  # [128, 256]
    d["up2w"] = np.ascontiguousarray(
        u2.reshape(128, 2, 128).transpose(2, 1, 0)
    ).reshape(128, 256)
    d["up2b"] = np.asarray(inp["up_b2"], f).reshape(128, 1).copy()

    # ---- polyphase re_enhance ----
    # phase/tap -> (in-phase, sub-shift) mapping
    def split(v):  # v = r + dy - 1
        rp = v % 2
        return rp, (v - rp) // 2

    r1w = np.asarray(inp["re_w1"], f)  # [32, 128, 3, 3]
    keymap = {}
    for p in range(4):
        r, s = p // 2, p % 2
        for dy in range(3):
            for dx in range(3):
                rp, qy = split(r + dy - 1)
                sp, qx = split(s + dx - 1)
                keymap.setdefault((2 * rp + sp, qy, qx), []).append((p, dy, dx))
    keys = sorted(keymap.keys(), key=lambda k: (k[1] != 0 or k[2] != 0, k))
    re1_keys = keys  # list of (p_in, qy, qx); all-(0,0) shifts first
    re1w = np.zeros((128, 16, 128), f)
    for ki, key in enumerate(keys):
        for (p, dy, dx) in keymap[key]:
            re1w[:, ki, p * 32 : (p + 1) * 32] = r1w[:, :, dy, dx].T
    d["re1w"] = re1w.reshape(128, 2048)
    d["re1b"] = np.tile(np.asarray(inp["re_b1"], f), 4).reshape(128, 1)

    r2w = np.asarray(inp["re_w2"], f)  # [128, 32, 3, 3]
    re2_q = []  # per out-phase list of shifts, (0,0) first
    re2w = np.zeros((128, 4, 4, 128), f)
    for p in range(4):
        r, s = p // 2, p % 2
        qys = sorted({split(r + dy - 1)[1] for dy in range(3)}, key=lambda q: q != 0)
        qxs = sorted({split(s + dx - 1)[1] for dx in range(3)}, key=lambda q: q != 0)
        qs = [(qy, qx) for qy in qys for qx in qxs]
        qs.sort(key=lambda q: q != (0, 0))
        re2_q.append(qs)
        for qi, (qy, qx) in enumerate(qs):
            for pp in range(4):
                rp, sp = pp // 2, pp % 2
                dy = 2 * qy + rp - r + 1
                dx = 2 * qx + sp - s + 1
                if 0 <= dy < 3 and 0 <= dx < 3:
                    re2w[pp * 32 : (pp + 1) * 32, p, qi, :] = r2w[:, :, dy, dx].T
    d["re2w"] = re2w.reshape(128, 2048)
    d["re2b"] = np.asarray(inp["re_b2"], f).reshape(128, 1).copy()

    return d, re1_keys, re2_q


RE1_KEYS = None
RE2_Q = None


def _mapping():
    global RE1_KEYS, RE2_Q
    if RE1_KEYS is None:
        zeros = {k: np.zeros(v) for k, v in [
            ("dsc1_w1", (64, 256)), ("dsc1_b1", (64,)), ("dsc1_w2", (2304, 64)),
            ("dsc1_b2", (2304,)), ("dsc2_w1", (64, 256)), ("dsc2_b1", (64,)),
            ("dsc2_w2", (2304, 64)), ("dsc2_b2", (2304,)),
            ("align_w", (256, 512, 1, 1)), ("align_b", (256,)),
            ("up_w1", (1024, 256, 3, 3)), ("up_b1", (1024,)),
            ("up_w2", (128, 256, 1, 1)), ("up_b2", (128,)),
            ("re_w1", (32, 128, 3, 3)), ("re_b1", (32,)),
            ("re_w2", (128, 32, 3, 3)), ("re_b2", (128,)),
        ]}
        _, RE1_KEYS, RE2_Q = host_prep(zeros)
    return RE1_KEYS, RE2_Q


# ---------------------------------------------------------------- bass build
def img(ap):
    """[128, 64, 64] view of a flat [128, 4096] image."""
    return ap.rearrange("p (y x) -> p y x", y=64)


def pimg(ap):
    """View of a column-padded [128, 64*66] image (zero cols at x=0 and x=65)."""
    return ap.rearrange("p (y x) -> p y x", x=66)


def mm_views(src_ap, psum_ap, sy, sx, n):
    """(psum_out, rhs) for 8-row chunk n of a shifted conv tap on a col-padded
    source. Rows clipped by sy; columns handled by the zero pad. psum out is a
    contiguous 2D region (an f32r-matmul ISA requirement)."""
    y0 = max(0, -sy)
    y1 = 64 + min(0, -sy)
    r0 = max(8 * n, y0)
    r1 = min(8 * n + 8, y1)
    if r1 <= r0:
        return None, None
    v = pimg(src_ap)
    rhs = v[:, r0 + sy : r1 + sy, 1 + sx : 65 + sx]
    out = psum_ap[:, (r0 - 8 * n) * 64 : (r1 - 8 * n) * 64]
    return out, rhs


def build():
    re1_keys, re2_q = _mapping()
    nc = bacc.Bacc(trn_type="TRN2", target_bir_lowering=False, debug=False)

    x_d = [nc.dram_tensor(n, [256, 4224], F32R, kind="ExternalInput") for n in ("x1", "x2")]
    wd = {}
    for name, shape, dt in [
        ("w1T0", [128, 128], F32), ("w1T1", [128, 128], F32),
        ("b1_0", [64, 1], F32), ("b1_1", [64, 1], F32),
        ("w2r0", [64, 2304], BF16), ("w2r1", [64, 2304], BF16),
        ("b2r0", [128, 18], F32), ("b2r1", [128, 18], F32),
        ("ident", [128, 128], F32R),
        ("alignw", [128, 1024], F32R), ("alignb", [128, 2], F32),
        ("up1w", [128, 24576], BF16), ("up1b", [128, 8], F32),
        ("up2w", [128, 256], F32R), ("up2b", [128, 1], F32),
        ("re1w", [128, 2048], F32R), ("re1b", [128, 1], F32),
        ("re2w", [128, 2048], F32R), ("re2b", [128, 1], F32),
    ]:
        wd[name] = nc.dram_tensor(name, shape, dt, kind="ExternalInput")
    out_d = nc.dram_tensor("out", [128, 16384], F32, kind="ExternalOutput")

    with tile.TileContext(nc) as tc, ExitStack() as ctx:
        wpool = ctx.enter_context(tc.tile_pool(name="w", bufs=1))
        u1pool = ctx.enter_context(tc.tile_pool(name="u1w", bufs=4))
        big = ctx.enter_context(tc.tile_pool(name="big", bufs=6))
        upool = ctx.enter_context(tc.tile_pool(name="upos", bufs=8))
        dpool = ctx.enter_context(tc.tile_pool(name="dg", bufs=2))
        stg = ctx.enter_context(tc.tile_pool(name="stg", bufs=2))
        bnd = ctx.enter_context(tc.tile_pool(name="bnd", bufs=3))
        tiny = ctx.enter_context(tc.tile_pool(name="tiny", bufs=4))
        ps = ctx.enter_context(tc.tile_pool(name="ps", bufs=7, space="PSUM"))
        psc = ctx.enter_context(tc.tile_pool(name="psc", bufs=1, space="PSUM"))

        wt = {}

        def load_w(name):
            t = wpool.tile(list(wd[name].shape), wd[name].dtype, tag=name, name=name)
            nc.sync.dma_start(t[:], wd[name].ap())
            wt[name] = t

        def img_tile(name="it"):
            return big.tile([128, 4096], F32R, tag="big", name=name)

        def pad_tile(name="pt"):
            """[128, 64*66] tile; interior written by evictions, pad cols
            zeroed here (x DMAs bring zeros from the host instead)."""
            t = big.tile([128, 4224], F32R, tag="big", name=name)
            v = pimg(t[:].bitcast(F32))
            nc.gpsimd.memset(v[:, :, 0:1], 0.0)
            nc.gpsimd.memset(v[:, :, 65:66], 0.0)
            return t

        w2rt = {}

        def load_w2r(d):
            if d not in w2rt:
                t = u1pool.tile([64, 2304], BF16, tag="u1w", name="w2rt")
                nc.sync.dma_start(t[:], wd[f"w2r{d}"].ap())
                w2rt[d] = t
            return w2rt[d]

        # DMA order is the startup critical path: x1 chunks and the weights
        # cond1(x1) needs come first, everything else after.
        xin = [[None, None], [None, None]]

        def load_x(i, c):
            t = big.tile([128, 4224], F32R, tag="big", name="xin")
            nc.sync.dma_start(t[:], x_d[i].ap()[c * 128 : (c + 1) * 128, :])
            xin[i][c] = t

        load_x(0, 0)
        load_x(0, 1)
        for name in ("w1T0", "b1_0", "b2r0", "ident"):
            load_w(name)
        load_w2r(0)
        for name in ("w1T1", "b1_1", "b2r1", "alignb", "up2w", "up2b",
                     "re1b", "re2b", "up1b"):
            load_w(name)
        load_w2r(1)
        load_x(1, 0)
        load_x(1, 1)

        # ---------------- dsc stage ----------------
        d_init = [0]  # number of dpool buffers memset so far
        TAPS = (4, 0, 1, 2, 3, 5, 6, 7, 8)  # center first: full psum coverage

        def conditioning(d, gms):
            """gms: two [128,1] per-chunk channel-sum tiles -> per-chunk
            diagonal-tap matrices D [128, 9*128] (f32r) for the PE depthwise."""
            pgt = psc.tile([128, 9], F32, tag="psc", name="pgt")
            pg = pgt[0:64, 0:1]
            for c in range(2):
                nc.tensor.matmul(pg, wt[f"w1T{d}"][:, c * 64 : (c + 1) * 64],
                                 gms[c][:], start=(c == 0), stop=(c == 1))
            u = tiny.tile([64, 1], F32, tag="u")
            nc.scalar.activation(u[:], pg, AF.Identity, bias=wt[f"b1_{d}"][:])
            sq = tiny.tile([64, 1], F32, tag="sq")
            nc.scalar.activation(sq[:], u[:], AF.Square)
            v3 = tiny.tile([64, 1], F32, tag="v3")
            nc.vector.scalar_tensor_tensor(v3[:], sq[:], 0.044715, u[:], AL.mult, AL.mult)
            w_ = tiny.tile([64, 1], F32, tag="w_")
            nc.vector.tensor_tensor(w_[:], v3[:], u[:], AL.add)
            th = tiny.tile([64, 1], F32, tag="th")
            nc.scalar.activation(th[:], w_[:], AF.Tanh, scale=0.7978845608028654)
            hv = tiny.tile([64, 1], BF16, tag="hv")
            nc.vector.scalar_tensor_tensor(hv[:], th[:], 1.0, u[:], AL.add, AL.mult)

            w2r = load_w2r(d)
            D_l, wts_l = [], []
            for c in range(2):
                pw = psc.tile([128, 9], F32, tag="psc", name="pw")
                for k in range(9):
                    nc.tensor.matmul(pw[:, k : k + 1],
                                     w2r[:, (c * 9 + k) * 128 : (c * 9 + k + 1) * 128],
                                     hv[:], start=True, stop=True)
                raw = tiny.tile([128, 9], F32, tag="raw")
                nc.vector.tensor_tensor(raw[:], pw[:],
                                        wt[f"b2r{d}"][:, c * 9 : (c + 1) * 9], AL.add)
                mx = tiny.tile([128, 1], F32, tag="mx")
                nc.vector.tensor_reduce(mx[:], raw[:], axis=mybir.AxisListType.X, op=AL.max)
                ngm = tiny.tile([128, 1], F32, tag="ngm")
                nc.vector.tensor_scalar_mul(ngm[:], mx[:], -1.0)
                ex = tiny.tile([128, 9], F32, tag="ex")
                ssum = tiny.tile([128, 1], F32, tag="ssum")
                nc.scalar.activation(ex[:], raw[:], AF.Exp, bias=ngm[:], accum_out=ssum[:])
                rec = tiny.tile([128, 1], F32, tag="rec")
                nc.vector.reciprocal(rec[:], ssum[:])
                wts = tiny.tile([128, 9], F32, tag="wts")
                nc.vector.tensor_scalar_mul(wts[:], ex[:], rec[:])
                w4p = tiny.tile([128, 1], F32, tag="w4p")
                nc.vector.tensor_scalar_add(w4p[:], wts[:, 4:5], 1.0)
                # diagonal tap matrices (center tap has +1 residual folded in)
                D = dpool.tile([128, 1152], F32R, tag="D")
                if d_init[0] < 2:
                    nc.gpsimd.memset(D[:].bitcast(F32), 0.0)
                    d_init[0] += 1
                for k in TAPS:  # tap-consumption order: PE starts on the
                    ptr = w4p[:] if k == 4 else wts[:, k : k + 1]  # center tap
                    nc.vector.tensor_scalar_mul(                   # immediately
                        D[:, k * 128 : (k + 1) * 128], wt["ident"][:], ptr)
                D_l.append(D)
                wts_l.append(wts)
            return D_l, wts_l

        def dw_pe(src, dst, D, relu, gacc, wts=None):
            """dst = depthwise(src) (+x via center tap) on PE; evict via Act.
            With wts given (no-relu apps only), taps 0,1 run on DVE and tap 2
            on Pool as post-eviction MACs into dst, shrinking the PE share."""
            off = () if wts is None else (0, 1)
            for n in range(8):
                p = ps.tile([128, 512], F32, tag="ps")
                first = True
                for k in TAPS:
                    if k in off:
                        continue
                    sy, sx = k // 3 - 1, k % 3 - 1
                    o, rhs = mm_views(src[:], p[:], sy, sx, n)
                    if o is None:
                        continue
                    nc.tensor.matmul(o, D[:, k * 128 : (k + 1) * 128], rhs,
                                     start=first, stop=(k == TAPS[-1]),
                                     skip_group_check=True)
                    first = False
                if relu:
                    nc.scalar.activation(pimg(dst[:])[:, n * 8 : (n + 1) * 8, 1:65],
                                         p[:].rearrange("p (y x) -> p y x", y=8),
                                         AF.Relu, accum_out=gacc[:, n : n + 1])
                else:
                    nc.scalar.activation(dst[:, n * 512 : (n + 1) * 512], p[:],
                                         AF.Identity)
            for k in off:
                # per-partition-scalar MACs are DVE-only on hardware (Pool
                # lacks TensorScalarPtr); src pad columns supply the x-shift
                # zeros
                sy, sx = k // 3 - 1, k % 3 - 1
                sv = pimg(src[:])[:, 1 + sy : 64 + sy, 1 + sx : 65 + sx]
                dv = img(dst[:])[:, 1:64, :]
                nc.vector.scalar_tensor_tensor(dv, sv, wts[:, k : k + 1], dv,
                                               AL.mult, AL.add)

        # per-chunk channel sums of the inputs; x2's are emitted later so they
        # don't delay cond1(x1)'s D builds on the in-order DVE
        gms1 = [[None, None], [None, None]]

        def reduce_gms1(i, use_act=False):
            for c in range(2):
                g = tiny.tile([128, 1], F32, tag="gms")
                if use_act and c == 1:
                    # idle Act engine: in-place copy whose accumulator is the
                    # channel sum; runs concurrently with DVE's c0 reduce
                    nc.scalar.activation(xin[i][c][:], xin[i][c][:], AF.Copy,
                                         accum_out=g[:])
                else:
                    nc.vector.tensor_reduce(g[:], xin[i][c][:].bitcast(F32),
                                            axis=mybir.AxisListType.X, op=AL.add)
                gms1[i][c] = g

        reduce_gms1(0, use_act=True)

        mid = [[None, None], [None, None]]
        gacc = [[None, None], [None, None]]
        gms2 = [[None, None], [None, None]]
        y = [[None, None], [None, None]]

        def alloc_mid(i):
            for c in range(2):
                mid[i][c] = pad_tile(name="midt")
                gacc[i][c] = tiny.tile([128, 8], F32, tag="gacc", name="gacc")

        def reduce_gms2(i):
            for c in range(2):
                g2 = tiny.tile([128, 1], F32, tag="gms")
                nc.vector.tensor_reduce(g2[:], gacc[i][c][:],
                                        axis=mybir.AxisListType.X, op=AL.add)
                gms2[i][c] = g2

        # interleave conditioning (PE-tiny + DVE/Act chain) between the big
        # PE depthwise apps so PE never waits on a conditioning chain.
        D1x1, _ = conditioning(0, gms1[0])
        alloc_mid(0)
        reduce_gms1(1)
        dw_pe(xin[0][0], mid[0][0], D1x1[0], True, gacc[0][0])
        D1x2, _ = conditioning(0, gms1[1])
        alloc_mid(1)
        dw_pe(xin[0][1], mid[0][1], D1x1[1], True, gacc[0][1])
        dw_pe(xin[1][0], mid[1][0], D1x2[0], True, gacc[1][0])
        reduce_gms2(0)
        D2x1, wts2x1 = conditioning(1, gms2[0])
        dw_pe(xin[1][1], mid[1][1], D1x2[1], True, gacc[1][1])
        reduce_gms2(1)
        for c in range(2):
            y[0][c] = big.tile([128, 4096], F32R, tag="big", name="yt")
        dw_pe(mid[0][0], y[0][0], D2x1[0], False, None, wts=wts2x1[0][:])
        D2x2, wts2x2 = conditioning(1, gms2[1])
        dw_pe(mid[0][1], y[0][1], D2x1[1], False, None, wts=wts2x1[1][:])
        for c in range(2):
            y[1][c] = big.tile([128, 4096], F32R, tag="big", name="yt")
        dw_pe(mid[1][0], y[1][0], D2x2[0], False, None, wts=wts2x2[0][:])
        dw_pe(mid[1][1], y[1][1], D2x2[1], False, None)

        # ---------------- align 1x1 (2C -> C) + Winograd x-transform ----------------
        # U0 = odd_{j-1}-odd_j, U1 = even+odd, U2 = even-odd, U3 = even_j-even_{j+1}
        # (signs folded into the position weights), built per eviction chunk so
        # up1 can start as soon as the first rows exist.
        awt = u1pool.tile([128, 1024], F32R, tag="u1w", name="awt")
        nc.sync.dma_start(awt[:], wd["alignw"].ap())
        wt["alignw"] = awt
        U = [[upool.tile([128, 2048], BF16, tag="U", name="U") for _ in range(4)]
             for _ in range(2)]
        for mc in range(2):
            for n in range(8):
                p = ps.tile([128, 512], F32, tag="ps")
                for kc in range(4):
                    rhs = y[kc // 2][kc % 2][:, n * 512 : (n + 1) * 512]
                    nc.tensor.matmul(
                        p[:], wt["alignw"][:, (kc * 2 + mc) * 128 : (kc * 2 + mc + 1) * 128],
                        rhs, start=(kc == 0), stop=(kc == 3))
                aev = stg.tile([128, 512], F32, tag="cmb", name="aev", bufs=2)
                nc.scalar.activation(aev[:], p[:], AF.Identity,
                                     bias=wt["alignb"][:, mc : mc + 1])
                v = aev[:].rearrange("p (y j t) -> p y j t", j=32, t=2)
                ev, od = v[:, :, :, 0:1], v[:, :, :, 1:2]
                Uv = [U[mc][q][:].rearrange("p (y j t) -> p y j t", j=32, t=1)
                      [:, n * 8 : (n + 1) * 8, :, :] for q in range(4)]
                nc.gpsimd.tensor_tensor(Uv[1], ev, od, AL.add)
                nc.vector.tensor_tensor(Uv[2], ev, od, AL.subtract)
                nc.vector.tensor_tensor(Uv[0][:, :, 1:32], od[:, :, 0:31],
                                        od[:, :, 1:32], AL.subtract)
                nc.vector.tensor_scalar_mul(Uv[0][:, :, 0:1], od[:, :, 0:1], -1.0)
                nc.vector.tensor_tensor(Uv[3][:, :, 0:31], ev[:, :, 0:31],
                                        ev[:, :, 1:32], AL.subtract)
                nc.vector.tensor_scalar_mul(Uv[3][:, :, 31:32], ev[:, :, 31:32], 1.0)

        # ---------------- up1 (Winograd positions) + up2 (1x1) ----------------
        def wmm_views(U_t, ps_t, sl, sy, n):
            y0 = max(0, -sy)
            y1 = 64 + min(0, -sy)
            r0 = max(8 * n, y0)
            r1 = min(8 * n + 8, y1)
            if r1 <= r0:
                return None, None
            v = U_t[:].rearrange("p (y j) -> p y j", j=32)
            rhs = v[:, r0 + sy : r1 + sy, :]
            out = ps_t[:, sl + (r0 - 8 * n) * 32 : sl + (r1 - 8 * n) * 32]
            return out, rhs

        up2p = []
        for p4 in range(4):
            up2p.append(pad_tile(name="up2p"))

        def emit_up2(p4, n, stage):
            """up2 1x1 for chunk n; emitted one chunk late so PE never waits
            on the stage-combine chain."""
            p2 = ps.tile([128, 512], F32, tag="ps", name="p2")
            for kc in range(2):
                nc.tensor.matmul(p2[:], wt["up2w"][:, kc * 128 : (kc + 1) * 128],
                                 stage[kc][:], start=(kc == 0), stop=(kc == 1))
            nc.scalar.activation(pimg(up2p[p4][:])[:, n * 8 : (n + 1) * 8, 1:65],
                                 p2[:].rearrange("p (y x) -> p y x", y=8),
                                 AF.Identity, bias=wt["up2b"][:])

        pend = []
        for p4 in range(4):
            u1t = []
            for mcin in range(2):
                th = u1pool.tile([128, 3072], BF16, tag="u1w", name="u1t")
                off = (p4 * 2 + mcin) * 3072
                nc.sync.dma_start(th[:], wd["up1w"].ap()[:, off : off + 3072])
                u1t.append(th)
            for n in range(8):
                stage = []
                for mcin in range(2):
                    psA = ps.tile([128, 512], F32, tag="ps")
                    psB = ps.tile([128, 512], F32, tag="ps")
                    for pos in range(4):
                        # psA = (m0 | m3), psB = (m1 | m2): the combine then
                        # needs only one PSUM operand per instruction
                        tgt = psA if pos in (0, 3) else psB
                        sl = 256 * (0 if pos in (0, 1) else 1)
                        first = True
                        for dy in (1, 0, 2):  # sy=0 first: full slice coverage
                            sy = dy - 1
                            for kc in range(2):
                                o, rhs = wmm_views(U[kc][pos], tgt[:], sl, sy, n)
                                if o is None:
                                    continue
                                lhsT = u1t[mcin][:, ((pos * 3 + dy) * 2 + kc) * 128 :
                                                 ((pos * 3 + dy) * 2 + kc + 1) * 128]
                                nc.tensor.matmul(o, lhsT, rhs, start=first,
                                                 stop=(dy == 2 and kc == 1),
                                                 skip_group_check=True)
                                first = False
                    # out0 = m0+(m1+m2)+b, out1 = (m1-m2)+b-m3, x-interleaved.
                    # Act evicts both psum pairs (frees the banks fast; engine
                    # ops may read at most one PSUM operand anyway), then the
                    # combine is SBUF-only on Pool/DVE.
                    sb = stg.tile([128, 512], F32, tag="cmb", name="cmb", bufs=2)
                    nc.scalar.activation(sb[:], psB[:], AF.Identity)
                    sbA = stg.tile([128, 512], F32, tag="ret", name="sbA", bufs=3)
                    nc.scalar.activation(sbA[:], psA[:], AF.Identity)
                    ta = stg.tile([128, 256], F32, tag="t01", name="ta", bufs=2)
                    nc.gpsimd.tensor_tensor(ta[:], sb[:, 0:256], sb[:, 256:512],
                                            AL.add)
                    tb = stg.tile([128, 256], F32, tag="t12", name="tb", bufs=2)
                    nc.vector.tensor_tensor(tb[:], sb[:, 0:256], sb[:, 256:512],
                                            AL.subtract)
                    st = stg.tile([128, 512], F32R, tag="stg")
                    stv = st[:].rearrange("p (a t) -> p a t", t=2)
                    c1 = lambda ap: ap.rearrange("p (a t) -> p a t", t=1)
                    bptr = wt["up1b"][:, p4 * 2 + mcin : p4 * 2 + mcin + 1]
                    nc.vector.scalar_tensor_tensor(stv[:, :, 0:1], c1(ta[:]), bptr,
                                                   c1(sbA[:, 0:256]), AL.add, AL.add)
                    nc.vector.scalar_tensor_tensor(stv[:, :, 1:2], c1(tb[:]), bptr,
                                                   c1(sbA[:, 256:512]), AL.add,
                                                   AL.subtract)
                    stage.append(st)
                pend.append((n, stage))
                if len(pend) > 1:
                    emit_up2(p4, *pend.pop(0))
            while pend:
                emit_up2(p4, *pend.pop(0))

        # ---------------- re1 (polyphase 3x3, M-packed) ----------------
        re1t = pad_tile(name="re1t")
        for name in ("re1w", "re2w"):
            t = u1pool.tile([128, 2048], F32R, tag="u1w")
            nc.sync.dma_start(t[:], wd[name].ap())
            wt[name] = t
        for n in range(8):
            p = ps.tile([128, 512], F32, tag="ps")
            for ki, (pin, qy, qx) in enumerate(re1_keys):
                o, rhs = mm_views(up2p[pin][:], p[:], qy, qx, n)
                if o is None:
                    continue
                nc.tensor.matmul(o, wt["re1w"][:, ki * 128 : (ki + 1) * 128], rhs,
                                 start=(ki == 0), stop=(ki == len(re1_keys) - 1),
                                 skip_group_check=True)
            nc.scalar.activation(pimg(re1t[:])[:, n * 8 : (n + 1) * 8, 1:65],
                                 p[:].rearrange("p (y x) -> p y x", y=8),
                                 AF.Relu, bias=wt["re1b"][:])

        # ---------------- re2 (polyphase 3x3) + residual + interleave + out ----------------
        for n in range(8):
            pss = []
            for p4 in range(4):
                p = ps.tile([128, 512], F32, tag="ps")
                for qi, (qy, qx) in enumerate(re2_q[p4]):
                    o, rhs = mm_views(re1t[:], p[:], qy, qx, n)
                    if o is None:
                        continue
                    nc.tensor.matmul(o, wt["re2w"][:, (p4 * 4 + qi) * 128 :
                                                   (p4 * 4 + qi + 1) * 128], rhs,
                                     start=(qi == 0), stop=(qi == len(re2_q[p4]) - 1),
                                     skip_group_check=True)
                pss.append(p)
            # p4=3 detours via Act (psum evict + bias) so Pool (no PSUM
            # access) can do its residual add from SBUF; p4 0-2 are DVE
            # STTs straight from psum. Keeps every engine under PE's pace.
            ret = {}
            for p4 in (2, 3):
                t = stg.tile([128, 512], F32, tag="ret", name="ret", bufs=3)
                nc.scalar.activation(t[:], pss[p4][:], AF.Identity,
                                     bias=wt["re2b"][:])
                ret[p4] = t
            for hb in range(2):  # half-bands of 8 output rows (4 phase rows)
                band = bnd.tile([128, 1024], F32, tag="bnd")
                bv = band[:].rearrange("p (y r x s) -> p y r x s", y=4, r=2, s=2)
                for p4 in range(4):
                    r, s = p4 // 2, p4 % 2
                    y0 = n * 8 + hb * 4
                    up_v = pimg(up2p[p4][:].bitcast(F32))[:, y0 : y0 + 4, 1:65]
                    if p4 >= 2:
                        nc.gpsimd.tensor_tensor(
                            bv[:, :, r, :, s],
                            ret[p4][:, hb * 256 : (hb + 1) * 256].rearrange(
                                "p (y x) -> p y x", y=4),
                            up_v, AL.add)
                    else:
                        nc.vector.scalar_tensor_tensor(
                            bv[:, :, r, :, s],
                            pss[p4][:, hb * 256 : (hb + 1) * 256].rearrange(
                                "p (y x) -> p y x", y=4),
                            wt["re2b"][:], up_v, AL.add, AL.add)
                nc.sync.dma_start(
                    out_d.ap()[:, (2 * n + hb) * 1024 : (2 * n + hb + 1) * 1024],
                    band[:])

    nc.compile()
    return nc


_NC = None


def _get_nc():
    global _NC
    if _NC is None:
        _NC = build()
    return _NC


def make_in_maps(inputs):
    w, _, _ = host_prep(inputs)
    def hostpad(x):
        x = np.asarray(x, np.float32).reshape(NC, 256, 64, 64)
        xp = np.zeros((NC, 256, 64, 66), np.float32)
        xp[:, :, :, 1:65] = x
        return np.ascontiguousarray(xp.reshape(NC, 256, 4224))

    x1 = hostpad(inputs["x1"])
    x2 = hostpad(inputs["x2"])
    in_maps = []
    for i in range(NC):
        m = {"x1": x1[i], "x2": x2[i]}
        m.update(w)
        in_maps.append(m)
    return in_maps


def kernel(**inputs):
    nc = _get_nc()
    in_maps = make_in_maps(inputs)
    res = run_bass_kernel_spmd(nc, in_maps, core_ids=list(range(NC)))
    out = np.stack([res.results[i]["out"].reshape(128, 128, 128) for i in range(NC)])
    return out.astype(np.float32)


# revision 43
# speedup vs baseline: 3.4277x; 1.0043x over previous
import sys

sys.path.insert(0, "/opt/trn_rl_repo")

from contextlib import ExitStack

import numpy as np

import concourse.bacc as bacc
import concourse.mybir as mybir
from concourse import tile
from concourse.bass_utils import run_bass_kernel_spmd

F32 = mybir.dt.float32
F32R = mybir.dt.float32r
BF16 = mybir.dt.bfloat16
AL = mybir.AluOpType
AF = mybir.ActivationFunctionType

C = 256
H = W = 64
NC = 8  # cores / batch shards


# ---------------------------------------------------------------- host prep
def host_prep(inp):
    """Rearrange all weights into [partition, free] layouts matching SBUF tiles."""
    d = {}
    f = np.float32

    # conditioning nets (dsc1, dsc2)
    for i, pre in ((0, "dsc1"), (1, "dsc2")):
        w1 = np.asarray(inp[f"{pre}_w1"], f)  # [64, 256]
        b1 = np.asarray(inp[f"{pre}_b1"], f)  # [64]
        w2 = np.asarray(inp[f"{pre}_w2"], f)  # [2304, 64]
        b2 = np.asarray(inp[f"{pre}_b2"], f)  # [2304]
        # lhsT for gm matmul: [k_local, chunk, m]; fold the 1/(H*W) mean here
        d[f"w1T{i}"] = np.ascontiguousarray(
            (w1.T / (H * W)).reshape(2, 128, 64).transpose(1, 0, 2)
        ).reshape(128, 128)
        d[f"b1_{i}"] = b1.reshape(64, 1).copy()
        # lhsT for wts matmul: [j, chunk, k, c_local]; fold gelu's 0.5 here
        d[f"w2r{i}"] = np.ascontiguousarray(
            (0.5 * w2).reshape(2, 128, 9, 64).transpose(3, 0, 2, 1)
        ).reshape(64, 2304).astype(mybir.dt.np(mybir.dt.bfloat16))
        d[f"b2r{i}"] = np.ascontiguousarray(
            b2.reshape(2, 128, 9).transpose(1, 0, 2)
        ).reshape(128, 18)

    # identity for building runtime diagonal depthwise weight matrices
    d["ident"] = np.ascontiguousarray(np.eye(128, dtype=f))

    # channel_align 1x1: [k_local, kc, mc, m]
    aw = np.asarray(inp["align_w"], f)[:, :, 0, 0]  # [256, 512]
    d["alignw"] = np.ascontiguousarray(
        aw.reshape(2, 128, 4, 128).transpose(3, 2, 0, 1)
    ).reshape(128, 1024)
    d["alignb"] = np.ascontiguousarray(
        np.asarray(inp["align_b"], f).reshape(2, 128).T
    )  # [128, 2]

    # up conv1 3x3 C->4C, pixel-shuffle phase reorder + 1D-Winograd F(2,3)
    # along x: 4 position weights per (dy, kc) replace the 3 dx taps.
    # new channel (p, g) -> orig channel 4g + p   (p = 2r+s)
    uw = np.asarray(inp["up_w1"], f)  # [1024, 256, 3, 3]
    w6 = uw.reshape(256, 4, 2, 128, 3, 3)
    w6 = w6.reshape(2, 128, 4, 2, 128, 3, 3)  # [mcin, m, p, kc, k, dy, dx]
    g0, g1, g2 = w6[..., 0], w6[..., 1], w6[..., 2]
    q = np.stack([g0, 0.5 * (g0 + g1 + g2), -0.5 * (g0 - g1 + g2), g2])
    # q: [pos, mcin, m, p, kc, k, dy] -> lhsT [k, (p, mcin, pos, dy, kc, m)]
    d["up1w"] = np.ascontiguousarray(q.transpose(5, 3, 1, 0, 6, 4, 2)).reshape(
        128, 4 * 2 * 4 * 3 * 2 * 128
    ).astype(mybir.dt.np(mybir.dt.bfloat16))
    ub = np.asarray(inp["up_b1"], f)
    d["up1b"] = np.ascontiguousarray(
        ub.reshape(2, 128, 4).transpose(1, 2, 0)
    ).reshape(128, 8)  # [m, (p, mcin)]

    # up conv2 1x1 C->C/2 (per-phase): [k_local, kc, m]
    u2 = np.asarray(inp["up_w2"], f)[:, :, 0, 0]  # [128, 256]
    d["up2w"] = np.ascontiguousarray(
        u2.reshape(128, 2, 128).transpose(2, 1, 0)
    ).reshape(128, 256)
    d["up2b"] = np.asarray(inp["up_b2"], f).reshape(128, 1).copy()

    # ---- polyphase re_enhance ----
    # phase/tap -> (in-phase, sub-shift) mapping
    def split(v):  # v = r + dy - 1
        rp = v % 2
        return rp, (v - rp) // 2

    r1w = np.asarray(inp["re_w1"], f)  # [32, 128, 3, 3]
    keymap = {}
    for p in range(4):
        r, s = p // 2, p % 2
        for dy in range(3):
            for dx in range(3):
                rp, qy = split(r + dy - 1)
                sp, qx = split(s + dx - 1)
                keymap.setdefault((2 * rp + sp, qy, qx), []).append((p, dy, dx))
    keys = sorted(keymap.keys(), key=lambda k: (k[1] != 0 or k[2] != 0, k))
    re1_keys = keys  # list of (p_in, qy, qx); all-(0,0) shifts first
    re1w = np.zeros((128, 16, 128), f)
    for ki, key in enumerate(keys):
        for (p, dy, dx) in keymap[key]:
            re1w[:, ki, p * 32 : (p + 1) * 32] = r1w[:, :, dy, dx].T
    d["re1w"] = re1w.reshape(128, 2048)
    d["re1b"] = np.tile(np.asarray(inp["re_b1"], f), 4).reshape(128, 1)

    r2w = np.asarray(inp["re_w2"], f)  # [128, 32, 3, 3]
    re2_q = []  # per out-phase list of shifts, (0,0) first
    re2w = np.zeros((128, 4, 4, 128), f)
    for p in range(4):
        r, s = p // 2, p % 2
        qys = sorted({split(r + dy - 1)[1] for dy in range(3)}, key=lambda q: q != 0)
        qxs = sorted({split(s + dx - 1)[1] for dx in range(3)}, key=lambda q: q != 0)
        qs = [(qy, qx) for qy in qys for qx in qxs]
        qs.sort(key=lambda q: q != (0, 0))
        re2_q.append(qs)
        for qi, (qy, qx) in enumerate(qs):
            for pp in range(4):
                rp, sp = pp // 2, pp % 2
                dy = 2 * qy + rp - r + 1
                dx = 2 * qx + sp - s + 1
                if 0 <= dy < 3 and 0 <= dx < 3:
                    re2w[pp * 32 : (pp + 1) * 32, p, qi, :] = r2w[:, :, dy, dx].T
    d["re2w"] = re2w.reshape(128, 2048)
    d["re2b"] = np.asarray(inp["re_b2"], f).reshape(128, 1).copy()

    return d, re1_keys, re2_q


RE1_KEYS = None
RE2_Q = None


def _mapping():
    global RE1_KEYS, RE2_Q
    if RE1_KEYS is None:
        zeros = {k: np.zeros(v) for k, v in [
            ("dsc1_w1", (64, 256)), ("dsc1_b1", (64,)), ("dsc1_w2", (2304, 64)),
            ("dsc1_b2", (2304,)), ("dsc2_w1", (64, 256)), ("dsc2_b1", (64,)),
            ("dsc2_w2", (2304, 64)), ("dsc2_b2", (2304,)),
            ("align_w", (256, 512, 1, 1)), ("align_b", (256,)),
            ("up_w1", (1024, 256, 3, 3)), ("up_b1", (1024,)),
            ("up_w2", (128, 256, 1, 1)), ("up_b2", (128,)),
            ("re_w1", (32, 128, 3, 3)), ("re_b1", (32,)),
            ("re_w2", (128, 32, 3, 3)), ("re_b2", (128,)),
        ]}
        _, RE1_KEYS, RE2_Q = host_prep(zeros)
    return RE1_KEYS, RE2_Q


# ---------------------------------------------------------------- bass build
def img(ap):
    """[128, 64, 64] view of a flat [128, 4096] image."""
    return ap.rearrange("p (y x) -> p y x", y=64)


def pimg(ap):
    """View of a column-padded [128, 64*66] image (zero cols at x=0 and x=65)."""
    return ap.rearrange("p (y x) -> p y x", x=66)


def mm_views(src_ap, psum_ap, sy, sx, n):
    """(psum_out, rhs) for 8-row chunk n of a shifted conv tap on a col-padded
    source. Rows clipped by sy; columns handled by the zero pad. psum out is a
    contiguous 2D region (an f32r-matmul ISA requirement)."""
    y0 = max(0, -sy)
    y1 = 64 + min(0, -sy)
    r0 = max(8 * n, y0)
    r1 = min(8 * n + 8, y1)
    if r1 <= r0:
        return None, None
    v = pimg(src_ap)
    rhs = v[:, r0 + sy : r1 + sy, 1 + sx : 65 + sx]
    out = psum_ap[:, (r0 - 8 * n) * 64 : (r1 - 8 * n) * 64]
    return out, rhs


def build():
    re1_keys, re2_q = _mapping()
    nc = bacc.Bacc(trn_type="TRN2", target_bir_lowering=False, debug=False)

    x_d = [nc.dram_tensor(n, [256, 4224], F32R, kind="ExternalInput") for n in ("x1", "x2")]
    wd = {}
    for name, shape, dt in [
        ("w1T0", [128, 128], F32), ("w1T1", [128, 128], F32),
        ("b1_0", [64, 1], F32), ("b1_1", [64, 1], F32),
        ("w2r0", [64, 2304], BF16), ("w2r1", [64, 2304], BF16),
        ("b2r0", [128, 18], F32), ("b2r1", [128, 18], F32),
        ("ident", [128, 128], F32R),
        ("alignw", [128, 1024], F32R), ("alignb", [128, 2], F32),
        ("up1w", [128, 24576], BF16), ("up1b", [128, 8], F32),
        ("up2w", [128, 256], F32R), ("up2b", [128, 1], F32),
        ("re1w", [128, 2048], F32R), ("re1b", [128, 1], F32),
        ("re2w", [128, 2048], F32R), ("re2b", [128, 1], F32),
    ]:
        wd[name] = nc.dram_tensor(name, shape, dt, kind="ExternalInput")
    out_d = nc.dram_tensor("out", [128, 16384], F32, kind="ExternalOutput")

    with tile.TileContext(nc) as tc, ExitStack() as ctx:
        wpool = ctx.enter_context(tc.tile_pool(name="w", bufs=1))
        u1pool = ctx.enter_context(tc.tile_pool(name="u1w", bufs=4))
        big = ctx.enter_context(tc.tile_pool(name="big", bufs=6))
        upool = ctx.enter_context(tc.tile_pool(name="upos", bufs=8))
        dpool = ctx.enter_context(tc.tile_pool(name="dg", bufs=2))
        stg = ctx.enter_context(tc.tile_pool(name="stg", bufs=2))
        bnd = ctx.enter_context(tc.tile_pool(name="bnd", bufs=3))
        tiny = ctx.enter_context(tc.tile_pool(name="tiny", bufs=4))
        ps = ctx.enter_context(tc.tile_pool(name="ps", bufs=7, space="PSUM"))
        psc = ctx.enter_context(tc.tile_pool(name="psc", bufs=1, space="PSUM"))

        wt = {}

        def load_w(name):
            t = wpool.tile(list(wd[name].shape), wd[name].dtype, tag=name, name=name)
            nc.sync.dma_start(t[:], wd[name].ap())
            wt[name] = t

        def img_tile(name="it"):
            return big.tile([128, 4096], F32R, tag="big", name=name)

        def pad_tile(name="pt"):
            """[128, 64*66] tile; interior written by evictions, pad cols
            zeroed here (x DMAs bring zeros from the host instead)."""
            t = big.tile([128, 4224], F32R, tag="big", name=name)
            v = pimg(t[:].bitcast(F32))
            nc.gpsimd.memset(v[:, :, 0:1], 0.0)
            nc.gpsimd.memset(v[:, :, 65:66], 0.0)
            return t

        w2rt = {}

        def load_w2r(d):
            if d not in w2rt:
                t = u1pool.tile([64, 2304], BF16, tag="u1w", name="w2rt")
                nc.sync.dma_start(t[:], wd[f"w2r{d}"].ap())
                w2rt[d] = t
            return w2rt[d]

        # DMA order is the startup critical path: x1 chunks and the weights
        # cond1(x1) needs come first, everything else after.
        xin = [[None, None], [None, None]]

        def load_x(i, c, split=False):
            t = big.tile([128, 4224], F32R, tag="big", name="xin")
            if split:
                # halves: the channel-sum reduce pipelines with the DMA
                for h in range(2):
                    nc.sync.dma_start(
                        t[:, h * 2112 : (h + 1) * 2112],
                        x_d[i].ap()[c * 128 : (c + 1) * 128, h * 2112 : (h + 1) * 2112])
            else:
                nc.sync.dma_start(t[:], x_d[i].ap()[c * 128 : (c + 1) * 128, :])
            xin[i][c] = t

        load_x(0, 0)
        load_x(0, 1, split=True)
        for name in ("w1T0", "b1_0", "b2r0", "ident"):
            load_w(name)
        load_w2r(0)
        for name in ("w1T1", "b1_1", "b2r1", "alignb", "up2w", "up2b",
                     "re1b", "re2b", "up1b"):
            load_w(name)
        load_w2r(1)
        load_x(1, 0)
        load_x(1, 1)

        # ---------------- dsc stage ----------------
        d_init = [0]  # number of dpool buffers memset so far
        TAPS = (4, 0, 1, 2, 3, 5, 6, 7, 8)  # center first: full psum coverage

        def conditioning(d, gms):
            """gms: two [128,1] per-chunk channel-sum tiles -> per-chunk
            diagonal-tap matrices D [128, 9*128] (f32r) for the PE depthwise."""
            pgt = psc.tile([128, 9], F32, tag="psc", name="pgt")
            pg = pgt[0:64, 0:1]
            for c in range(2):
                nc.tensor.matmul(pg, wt[f"w1T{d}"][:, c * 64 : (c + 1) * 64],
                                 gms[c][:], start=(c == 0), stop=(c == 1))
            u = tiny.tile([64, 1], F32, tag="u")
            nc.scalar.activation(u[:], pg, AF.Identity, bias=wt[f"b1_{d}"][:])
            sq = tiny.tile([64, 1], F32, tag="sq")
            nc.scalar.activation(sq[:], u[:], AF.Square)
            v3 = tiny.tile([64, 1], F32, tag="v3")
            nc.vector.scalar_tensor_tensor(v3[:], sq[:], 0.044715, u[:], AL.mult, AL.mult)
            w_ = tiny.tile([64, 1], F32, tag="w_")
            nc.vector.tensor_tensor(w_[:], v3[:], u[:], AL.add)
            th = tiny.tile([64, 1], F32, tag="th")
            nc.scalar.activation(th[:], w_[:], AF.Tanh, scale=0.7978845608028654)
            hv = tiny.tile([64, 1], BF16, tag="hv")
            nc.vector.scalar_tensor_tensor(hv[:], th[:], 1.0, u[:], AL.add, AL.mult)

            w2r = load_w2r(d)
            D_l, wts_l = [], []
            for c in range(2):
                pw = psc.tile([128, 9], F32, tag="psc", name="pw")
                for k in range(9):
                    nc.tensor.matmul(pw[:, k : k + 1],
                                     w2r[:, (c * 9 + k) * 128 : (c * 9 + k + 1) * 128],
                                     hv[:], start=True, stop=True)
                raw = tiny.tile([128, 9], F32, tag="raw")
                nc.vector.tensor_tensor(raw[:], pw[:],
                                        wt[f"b2r{d}"][:, c * 9 : (c + 1) * 9], AL.add)
                mx = tiny.tile([128, 1], F32, tag="mx")
                nc.vector.tensor_reduce(mx[:], raw[:], axis=mybir.AxisListType.X, op=AL.max)
                ngm = tiny.tile([128, 1], F32, tag="ngm")
                nc.vector.tensor_scalar_mul(ngm[:], mx[:], -1.0)
                ex = tiny.tile([128, 9], F32, tag="ex")
                ssum = tiny.tile([128, 1], F32, tag="ssum")
                nc.scalar.activation(ex[:], raw[:], AF.Exp, bias=ngm[:], accum_out=ssum[:])
                rec = tiny.tile([128, 1], F32, tag="rec")
                nc.vector.reciprocal(rec[:], ssum[:])
                wts = tiny.tile([128, 9], F32, tag="wts")
                nc.vector.tensor_scalar_mul(wts[:], ex[:], rec[:])
                w4p = tiny.tile([128, 1], F32, tag="w4p")
                nc.vector.tensor_scalar_add(w4p[:], wts[:, 4:5], 1.0)
                # diagonal tap matrices (center tap has +1 residual folded in)
                D = dpool.tile([128, 1152], F32R, tag="D")
                if d_init[0] < 2:
                    nc.gpsimd.memset(D[:].bitcast(F32), 0.0)
                    d_init[0] += 1
                for k in TAPS:  # tap-consumption order: PE starts on the
                    ptr = w4p[:] if k == 4 else wts[:, k : k + 1]  # center tap
                    nc.vector.tensor_scalar_mul(                   # immediately
                        D[:, k * 128 : (k + 1) * 128], wt["ident"][:], ptr)
                D_l.append(D)
                wts_l.append(wts)
            return D_l, wts_l

        def dw_pe(src, dst, D, relu, gacc, wts=None):
            """dst = depthwise(src) (+x via center tap) on PE; evict via Act.
            With wts given (no-relu apps only), taps 0,1 run on DVE and tap 2
            on Pool as post-eviction MACs into dst, shrinking the PE share."""
            off = () if wts is None else (0, 1)
            for n in range(8):
                p = ps.tile([128, 512], F32, tag="ps")
                first = True
                for k in TAPS:
                    if k in off:
                        continue
                    sy, sx = k // 3 - 1, k % 3 - 1
                    o, rhs = mm_views(src[:], p[:], sy, sx, n)
                    if o is None:
                        continue
                    nc.tensor.matmul(o, D[:, k * 128 : (k + 1) * 128], rhs,
                                     start=first, stop=(k == TAPS[-1]),
                                     skip_group_check=True)
                    first = False
                if relu:
                    nc.scalar.activation(pimg(dst[:])[:, n * 8 : (n + 1) * 8, 1:65],
                                         p[:].rearrange("p (y x) -> p y x", y=8),
                                         AF.Relu, accum_out=gacc[:, n : n + 1])
                else:
                    nc.scalar.activation(dst[:, n * 512 : (n + 1) * 512], p[:],
                                         AF.Identity)
            for k in off:
                # per-partition-scalar MACs are DVE-only on hardware (Pool
                # lacks TensorScalarPtr); src pad columns supply the x-shift
                # zeros
                sy, sx = k // 3 - 1, k % 3 - 1
                sv = pimg(src[:])[:, 1 + sy : 64 + sy, 1 + sx : 65 + sx]
                dv = img(dst[:])[:, 1:64, :]
                nc.vector.scalar_tensor_tensor(dv, sv, wts[:, k : k + 1], dv,
                                               AL.mult, AL.add)

        # per-chunk channel sums of the inputs; x2's are emitted later so they
        # don't delay cond1(x1)'s D builds on the in-order DVE
        gms1 = [[None, None], [None, None]]

        def reduce_gms1(i, use_act=False):
            for c in range(2):
                g = tiny.tile([128, 1], F32, tag="gms")
                if use_act and c == 1:
                    # idle Act engine: in-place half-copies whose accumulators
                    # are the channel sums, each firing as its half-DMA lands
                    ga = tiny.tile([128, 2], F32, tag="gred", name="gred")
                    for h in range(2):
                        hv_ = xin[i][c][:, h * 2112 : (h + 1) * 2112]
                        nc.scalar.activation(hv_, hv_, AF.Copy,
                                             accum_out=ga[:, h : h + 1])
                    nc.vector.tensor_reduce(g[:], ga[:],
                                            axis=mybir.AxisListType.X, op=AL.add)
                else:
                    nc.vector.tensor_reduce(g[:], xin[i][c][:].bitcast(F32),
                                            axis=mybir.AxisListType.X, op=AL.add)
                gms1[i][c] = g

        reduce_gms1(0, use_act=True)

        mid = [[None, None], [None, None]]
        gacc = [[None, None], [None, None]]
        gms2 = [[None, None], [None, None]]
        y = [[None, None], [None, None]]

        def alloc_mid(i):
            for c in range(2):
                mid[i][c] = pad_tile(name="midt")
                gacc[i][c] = tiny.tile([128, 8], F32, tag="gacc", name="gacc")

        def reduce_gms2(i):
            for c in range(2):
                g2 = tiny.tile([128, 1], F32, tag="gms")
                nc.vector.tensor_reduce(g2[:], gacc[i][c][:],
                                        axis=mybir.AxisListType.X, op=AL.add)
                gms2[i][c] = g2

        # interleave conditioning (PE-tiny + DVE/Act chain) between the big
        # PE depthwise apps so PE never waits on a conditioning chain.
        D1x1, _ = conditioning(0, gms1[0])
        alloc_mid(0)
        reduce_gms1(1)
        dw_pe(xin[0][0], mid[0][0], D1x1[0], True, gacc[0][0])
        D1x2, _ = conditioning(0, gms1[1])
        alloc_mid(1)
        dw_pe(xin[0][1], mid[0][1], D1x1[1], True, gacc[0][1])
        dw_pe(xin[1][0], mid[1][0], D1x2[0], True, gacc[1][0])
        reduce_gms2(0)
        D2x1, wts2x1 = conditioning(1, gms2[0])
        dw_pe(xin[1][1], mid[1][1], D1x2[1], True, gacc[1][1])
        reduce_gms2(1)
        for c in range(2):
            y[0][c] = big.tile([128, 4096], F32R, tag="big", name="yt")
        dw_pe(mid[0][0], y[0][0], D2x1[0], False, None, wts=wts2x1[0][:])
        D2x2, wts2x2 = conditioning(1, gms2[1])
        dw_pe(mid[0][1], y[0][1], D2x1[1], False, None, wts=wts2x1[1][:])
        for c in range(2):
            y[1][c] = big.tile([128, 4096], F32R, tag="big", name="yt")
        dw_pe(mid[1][0], y[1][0], D2x2[0], False, None, wts=wts2x2[0][:])
        dw_pe(mid[1][1], y[1][1], D2x2[1], False, None)

        # ---------------- align 1x1 (2C -> C) + Winograd x-transform ----------------
        # U0 = odd_{j-1}-odd_j, U1 = even+odd, U2 = even-odd, U3 = even_j-even_{j+1}
        # (signs folded into the position weights), built per eviction chunk so
        # up1 can start as soon as the first rows exist.
        awt = u1pool.tile([128, 1024], F32R, tag="u1w", name="awt")
        nc.sync.dma_start(awt[:], wd["alignw"].ap())
        wt["alignw"] = awt
        U = [[upool.tile([128, 2048], BF16, tag="U", name="U") for _ in range(4)]
             for _ in range(2)]
        for mc in range(2):
            for n in range(8):
                p = ps.tile([128, 512], F32, tag="ps")
                for kc in range(4):
                    rhs = y[kc // 2][kc % 2][:, n * 512 : (n + 1) * 512]
                    nc.tensor.matmul(
                        p[:], wt["alignw"][:, (kc * 2 + mc) * 128 : (kc * 2 + mc + 1) * 128],
                        rhs, start=(kc == 0), stop=(kc == 3))
                aev = stg.tile([128, 512], F32, tag="cmb", name="aev", bufs=2)
                nc.scalar.activation(aev[:], p[:], AF.Identity,
                                     bias=wt["alignb"][:, mc : mc + 1])
                v = aev[:].rearrange("p (y j t) -> p y j t", j=32, t=2)
                ev, od = v[:, :, :, 0:1], v[:, :, :, 1:2]
                Uv = [U[mc][q][:].rearrange("p (y j t) -> p y j t", j=32, t=1)
                      [:, n * 8 : (n + 1) * 8, :, :] for q in range(4)]
                nc.gpsimd.tensor_tensor(Uv[1], ev, od, AL.add)
                nc.vector.tensor_tensor(Uv[2], ev, od, AL.subtract)
                nc.vector.tensor_tensor(Uv[0][:, :, 1:32], od[:, :, 0:31],
                                        od[:, :, 1:32], AL.subtract)
                nc.vector.tensor_scalar_mul(Uv[0][:, :, 0:1], od[:, :, 0:1], -1.0)
                nc.vector.tensor_tensor(Uv[3][:, :, 0:31], ev[:, :, 0:31],
                                        ev[:, :, 1:32], AL.subtract)
                nc.vector.tensor_scalar_mul(Uv[3][:, :, 31:32], ev[:, :, 31:32], 1.0)

        # ---------------- up1 (Winograd positions) + up2 (1x1) ----------------
        def wmm_views(U_t, ps_t, sl, sy, n):
            y0 = max(0, -sy)
            y1 = 64 + min(0, -sy)
            r0 = max(8 * n, y0)
            r1 = min(8 * n + 8, y1)
            if r1 <= r0:
                return None, None
            v = U_t[:].rearrange("p (y j) -> p y j", j=32)
            rhs = v[:, r0 + sy : r1 + sy, :]
            out = ps_t[:, sl + (r0 - 8 * n) * 32 : sl + (r1 - 8 * n) * 32]
            return out, rhs

        up2p = []
        for p4 in range(4):
            up2p.append(pad_tile(name="up2p"))

        def emit_up2(p4, n, stage):
            """up2 1x1 for chunk n; emitted one chunk late so PE never waits
            on the stage-combine chain."""
            p2 = ps.tile([128, 512], F32, tag="ps", name="p2")
            for kc in range(2):
                nc.tensor.matmul(p2[:], wt["up2w"][:, kc * 128 : (kc + 1) * 128],
                                 stage[kc][:], start=(kc == 0), stop=(kc == 1))
            nc.scalar.activation(pimg(up2p[p4][:])[:, n * 8 : (n + 1) * 8, 1:65],
                                 p2[:].rearrange("p (y x) -> p y x", y=8),
                                 AF.Identity, bias=wt["up2b"][:])

        pend = []
        for p4 in range(4):
            u1t = []
            for mcin in range(2):
                th = u1pool.tile([128, 3072], BF16, tag="u1w", name="u1t")
                off = (p4 * 2 + mcin) * 3072
                nc.sync.dma_start(th[:], wd["up1w"].ap()[:, off : off + 3072])
                u1t.append(th)
            for n in range(8):
                stage = []
                for mcin in range(2):
                    psA = ps.tile([128, 512], F32, tag="ps")
                    psB = ps.tile([128, 512], F32, tag="ps")
                    for pos in range(4):
                        # psA = (m0 | m3), psB = (m1 | m2): the combine then
                        # needs only one PSUM operand per instruction
                        tgt = psA if pos in (0, 3) else psB
                        sl = 256 * (0 if pos in (0, 1) else 1)
                        first = True
                        for dy in (1, 0, 2):  # sy=0 first: full slice coverage
                            sy = dy - 1
                            for kc in range(2):
                                o, rhs = wmm_views(U[kc][pos], tgt[:], sl, sy, n)
                                if o is None:
                                    continue
                                lhsT = u1t[mcin][:, ((pos * 3 + dy) * 2 + kc) * 128 :
                                                 ((pos * 3 + dy) * 2 + kc + 1) * 128]
                                nc.tensor.matmul(o, lhsT, rhs, start=first,
                                                 stop=(dy == 2 and kc == 1),
                                                 skip_group_check=True)
                                first = False
                    # out0 = m0+(m1+m2)+b, out1 = (m1-m2)+b-m3, x-interleaved.
                    # Act evicts both psum pairs (frees the banks fast; engine
                    # ops may read at most one PSUM operand anyway), then the
                    # combine is SBUF-only on Pool/DVE.
                    sb = stg.tile([128, 512], F32, tag="cmb", name="cmb", bufs=2)
                    nc.scalar.activation(sb[:], psB[:], AF.Identity)
                    sbA = stg.tile([128, 512], F32, tag="ret", name="sbA", bufs=3)
                    nc.scalar.activation(sbA[:], psA[:], AF.Identity)
                    ta = stg.tile([128, 256], F32, tag="t01", name="ta", bufs=2)
                    nc.gpsimd.tensor_tensor(ta[:], sb[:, 0:256], sb[:, 256:512],
                                            AL.add)
                    tb = stg.tile([128, 256], F32, tag="t12", name="tb", bufs=2)
                    nc.vector.tensor_tensor(tb[:], sb[:, 0:256], sb[:, 256:512],
                                            AL.subtract)
                    st = stg.tile([128, 512], F32R, tag="stg")
                    stv = st[:].rearrange("p (a t) -> p a t", t=2)
                    c1 = lambda ap: ap.rearrange("p (a t) -> p a t", t=1)
                    bptr = wt["up1b"][:, p4 * 2 + mcin : p4 * 2 + mcin + 1]
                    nc.vector.scalar_tensor_tensor(stv[:, :, 0:1], c1(ta[:]), bptr,
                                                   c1(sbA[:, 0:256]), AL.add, AL.add)
                    nc.vector.scalar_tensor_tensor(stv[:, :, 1:2], c1(tb[:]), bptr,
                                                   c1(sbA[:, 256:512]), AL.add,
                                                   AL.subtract)
                    stage.append(st)
                pend.append((n, stage))
                if len(pend) > 1:
                    emit_up2(p4, *pend.pop(0))
            while pend:
                emit_up2(p4, *pend.pop(0))

        # ---------------- re1 (polyphase 3x3, M-packed) ----------------
        re1t = pad_tile(name="re1t")
        for name in ("re1w", "re2w"):
            t = u1pool.tile([128, 2048], F32R, tag="u1w")
            nc.sync.dma_start(t[:], wd[name].ap())
            wt[name] = t
        for n in range(8):
            p = ps.tile([128, 512], F32, tag="ps")
            for ki, (pin, qy, qx) in enumerate(re1_keys):
                o, rhs = mm_views(up2p[pin][:], p[:], qy, qx, n)
                if o is None:
                    continue
                nc.tensor.matmul(o, wt["re1w"][:, ki * 128 : (ki + 1) * 128], rhs,
                                 start=(ki == 0), stop=(ki == len(re1_keys) - 1),
                                 skip_group_check=True)
            nc.scalar.activation(pimg(re1t[:])[:, n * 8 : (n + 1) * 8, 1:65],
                                 p[:].rearrange("p (y x) -> p y x", y=8),
                                 AF.Relu, bias=wt["re1b"][:])

        # ---------------- re2 (polyphase 3x3) + residual + interleave + out ----------------
        for n in range(8):
            pss = []
            for p4 in range(4):
                p = ps.tile([128, 512], F32, tag="ps")
                for qi, (qy, qx) in enumerate(re2_q[p4]):
                    o, rhs = mm_views(re1t[:], p[:], qy, qx, n)
                    if o is None:
                        continue
                    nc.tensor.matmul(o, wt["re2w"][:, (p4 * 4 + qi) * 128 :
                                                   (p4 * 4 + qi + 1) * 128], rhs,
                                     start=(qi == 0), stop=(qi == len(re2_q[p4]) - 1),
                                     skip_group_check=True)
                pss.append(p)
            # p4=3 detours via Act (psum evict + bias) so Pool (no PSUM
            # access) can do its residual add from SBUF; p4 0-2 are DVE
            # STTs straight from psum. Keeps every engine under PE's pace.
            ret = {}
            for p4 in (2, 3):
                t = stg.tile([128, 512], F32, tag="ret", name="ret", bufs=3)
                nc.scalar.activation(t[:], pss[p4][:], AF.Identity,
                                     bias=wt["re2b"][:])
                ret[p4] = t
            for hb in range(2):  # half-bands of 8 output rows (4 phase rows)
                band = bnd.tile([128, 1024], F32, tag="bnd")
                bv = band[:].rearrange("p (y r x s) -> p y r x s", y=4, r=2, s=2)
                for p4 in range(4):
                    r, s = p4 // 2, p4 % 2
                    y0 = n * 8 + hb * 4
                    up_v = pimg(up2p[p4][:].bitcast(F32))[:, y0 : y0 + 4, 1:65]
                    if p4 >= 2:
                        nc.gpsimd.tensor_tensor(
                            bv[:, :, r, :, s],
                            ret[p4][:, hb * 256 : (hb + 1) * 256].rearrange(
                                "p (y x) -> p y x", y=4),
                            up_v, AL.add)
                    else:
                        nc.vector.scalar_tensor_tensor(
                            bv[:, :, r, :, s],
                            pss[p4][:, hb * 256 : (hb + 1) * 256].rearrange(
                                "p (y x) -> p y x", y=4),
                            wt["re2b"][:], up_v, AL.add, AL.add)
                nc.sync.dma_start(
                    out_d.ap()[:, (2 * n + hb) * 1024 : (2 * n + hb + 1) * 1024],
                    band[:])

    nc.compile()
    return nc


_NC = None


def _get_nc():
    global _NC
    if _NC is None:
        _NC = build()
    return _NC


def make_in_maps(inputs):
    w, _, _ = host_prep(inputs)
    def hostpad(x):
        x = np.asarray(x, np.float32).reshape(NC, 256, 64, 64)
        xp = np.zeros((NC, 256, 64, 66), np.float32)
        xp[:, :, :, 1:65] = x
        return np.ascontiguousarray(xp.reshape(NC, 256, 4224))

    x1 = hostpad(inputs["x1"])
    x2 = hostpad(inputs["x2"])
    in_maps = []
    for i in range(NC):
        m = {"x1": x1[i], "x2": x2[i]}
        m.update(w)
        in_maps.append(m)
    return in_maps


def kernel(**inputs):
    nc = _get_nc()
    in_maps = make_in_maps(inputs)
    res = run_bass_kernel_spmd(nc, in_maps, core_ids=list(range(NC)))
    out = np.stack([res.results[i]["out"].reshape(128, 128, 128) for i in range(NC)])
    return out.astype(np.float32)
